# revision 69
# baseline (speedup 1.0000x reference)
"""GCN (2x GCNConv + mean-pool + fc + LayerNorm) on 8 Trainium2 NeuronCores.

One-hot matmul aggregation replaces DMA scatter-adds entirely:

conv1 (pull): per-core in-edges on a cross-core-common segment grid
(64-node dst block x src%4 slot, segment length = max over cores). Each
128-edge chunk is gathered with one 512B descriptor per edge from a
4-row-packed bf16 x' table (the edge's row is slot src%4). For each
(chunk x segment) overlap ("unit"), PE accumulates
zT[64f, 64d] += M[128e, 64f].T @ S[128e, 64d], where S is a one-hot built
from a per-unit dstloc column: batched is_equal-vs-iota on DVE, with a
share built on the Activation engine as relu(1 - (iota - dstloc)^2).
The implicit self-loop is a second matmul against a transposed prescaled
x table, so it never enters the edge stream, and mm1 needs no transpose
(zT is the lhsT directly).

conv2 (push): per-core out-edges on a common global 64-node dst-block
grid, blocks ordered (quarter, range, local) so each node-quarter of the
TRANSPOSED partials u2T completes early; gathers hit the local hpart rows
(h1' = relu(h1)*dinv, prescaled). FOUR chunked ReduceScatters fire as
their quarter's writes land, overlapping the next quarter's compute, and
mm2 consumes each reduced chunk one quarter later (no transposes: z2T is
the lhsT; the self term adds from an SBUF-resident transposed hpsbT).
Pooling is the PSUM-accumulated one-hot graph matmul; pooled sums are
AllReduced; the tiny fc+LayerNorm head is computed redundantly.
"""
import sys

if '/opt/trn_rl_repo' not in sys.path:
    sys.path.insert(0, '/opt/trn_rl_repo')

import ml_dtypes
import numpy as np

import concourse.bacc as bacc
import concourse.mybir as mybir
from concourse.bass import _add_dep_helper
from concourse.tile import TileContext
from concourse.bass_utils import run_bass_kernel_spmd

# ---------------------------------------------------------------- constants
N = 100000
E = 800000
IN = 64
HID = 128
G = 256
NC = 8
RPC = N // NC              # 12500 nodes per core
NB = 98                    # 128-node dst blocks per core (12544 padded)
NPC = NB * 128             # 12544
DW = 64                    # aggregation dst-block width
NB64 = NPC // DW           # 196 64-blocks per core
NBG64 = NC * NB64          # 1568 global 64-blocks
QLB = (56, 56, 56, 28)     # conv2 RS chunks (in 64-blocks per core)
LN_EPS = 1e-5
W1CH = 16                  # conv1 gather window (chunks per call)
W2CH = 32                  # conv2 gather window
UB = 32                    # S-build batch (units per instruction)
ACT1 = 10 ** 9             # conv1: every ACT1-th S-batch goes to Act engine
ACT2 = 10 ** 9             # conv2: every ACT2-th S-batch goes to Act engine
G1 = 4                     # conv1 retire group (128-blocks)
G2 = 7                     # conv2 retire group (64-blocks; divides 56)
G3 = 7                     # mm2 group (128-blocks)
F32 = mybir.dt.float32
FP8 = mybir.dt.float8e4
BF16 = mybir.dt.bfloat16
I16 = mybir.dt.int16


def _wrap16(a, cols):
    """[n] -> [128, cols] int16: element i -> [i%16, i//16], tiled x8."""
    out = np.zeros((16, cols), np.int16)
    w = a.reshape(-1, 16).T
    out[:, : w.shape[1]] = w
    return np.tile(out, (8, 1))


def _common_grid(counts):
    seg = counts.max(axis=0).astype(np.int64)
    off = np.concatenate([[0], np.cumsum(seg)])
    epad = int((off[-1] + 127) // 128 * 128)
    return seg, off, epad


def _make_units(seg, off, nblk, seg_per_blk):
    """(chunk, blk, lo, hi) overlaps of 128-edge chunks with segments.

    Segment k belongs to block k // seg_per_blk (blocks in segment order).
    Returns units, per-unit start/stop (first/last unit of its block), and
    per-block has_units.
    """
    units = []
    blk_first = {}
    blk_last = {}
    for k in range(len(seg)):
        if seg[k] == 0:
            continue
        blk = k // seg_per_blk
        a, b = int(off[k]), int(off[k] + seg[k])
        for cc in range(a // 128, (b - 1) // 128 + 1):
            lo, hi = max(a, cc * 128), min(b, (cc + 1) * 128)
            u = len(units)
            units.append((cc, blk, lo, hi))
            if blk not in blk_first:
                blk_first[blk] = u
            blk_last[blk] = u
    start = [False] * len(units)
    stop = [False] * len(units)
    for blk, u in blk_first.items():
        start[u] = True
    for blk, u in blk_last.items():
        stop[u] = True
    has = [blk in blk_first for blk in range(nblk)]
    return units, start, stop, has


def _q_of_lb(lb):
    """conv2 node-quarter of a local 64-block index."""
    acc = 0
    for q, n in enumerate(QLB):
        if lb < acc + n:
            return q
        acc += n
    raise ValueError(lb)


# conv2 ordered block list: (quarter, range, lb64) lexicographic
_B2ORDER = sorted(range(NBG64),
                  key=lambda b: (_q_of_lb(b % NB64), b // NB64, b % NB64))
_Q0 = [0, 56, 112, 168]            # quarter start lb64


def _host_prep(edge_index, batch):
    src = np.asarray(edge_index[0], np.int64)
    dst = np.asarray(edge_index[1], np.int64)
    deg = (np.bincount(dst, minlength=N) + 1.0).astype(np.float32)
    dinv = (1.0 / np.sqrt(deg)).astype(np.float32)

    # ---------------- conv1: (blk64, slot) grid
    e_core1 = []
    cnt1 = np.zeros((NC, NB64 * 4), np.int64)
    for c in range(NC):
        m = (dst >= c * RPC) & (dst < (c + 1) * RPC)
        s, d = src[m], dst[m] - c * RPC
        key = (d // DW) * 4 + (s % 4)
        cnt1[c] = np.bincount(key, minlength=NB64 * 4)
        order = np.lexsort((d, key))
        e_core1.append((s[order], d[order], key[order]))
    seg1, off1, epad1 = _common_grid(cnt1)
    units1, ustart1, ustop1, has1 = _make_units(seg1, off1, NB64, 4)
    nch1 = epad1 // 128
    nu1 = len(units1)
    nu1p = (nu1 + UB - 1) // UB * UB
    uslot1 = []
    for u, (cc, blk, lo, hi) in enumerate(units1):
        base = blk * 4
        q = 0
        for qq in range(4):
            a, b = off1[base + qq], off1[base + qq] + seg1[base + qq]
            if a <= lo < b:
                q = qq
                break
        uslot1.append(q)

    # ---------------- conv2: global 64-blocks in _B2ORDER
    e_core2 = []
    cnt2 = np.zeros((NC, NBG64), np.int64)   # indexed by ORDER position
    b2pos = np.empty(NBG64, np.int64)        # global blk64 -> order position
    for i, b in enumerate(_B2ORDER):
        b2pos[b] = i
    for c in range(NC):
        m = (src >= c * RPC) & (src < (c + 1) * RPC)
        s, d = src[m] - c * RPC, dst[m]
        rr = d // RPC                      # owner range of the dst
        dloc = d - rr * RPC                # dst local to its owner core
        key = b2pos[rr * NB64 + dloc // DW]
        cnt2[c] = np.bincount(key, minlength=NBG64)
        order = np.lexsort((dloc, key))
        e_core2.append((s[order], dloc[order], key[order]))
    seg2, off2, epad2 = _common_grid(cnt2)
    units2, ustart2, ustop2, has2 = _make_units(seg2, off2, NBG64, 1)
    nch2 = epad2 // 128
    nu2 = len(units2)
    nu2p = (nu2 + UB - 1) // UB * UB

    # ---------------- per-core streams
    per_core = []
    for c in range(NC):
        s, d, key = e_core1[c]
        pos = np.empty(len(s), np.int64)
        ptr = 0
        for k in np.unique(key):
            n = int(cnt1[c][k])
            pos[ptr:ptr + n] = off1[k] + np.arange(n)
            ptr += n
        gi1 = np.zeros(epad1, np.int16)
        gi1[pos] = (s // 4).astype(np.int16)
        sd1 = np.full(epad1, -1.0, np.float32)
        sd1[pos] = (d % DW).astype(np.float32)
        sl1 = np.full(epad1, -1, np.int8)
        sl1[pos] = (s % 4).astype(np.int8)
        dl1c = np.full((nu1p, 128), -1.0, np.float32)
        for u, (cc, blk, lo, hi) in enumerate(units1):
            p0 = lo - cc * 128
            q = uslot1[u]
            dl1c[u, p0:hi - cc * 128] = np.where(
                sl1[lo:hi] == q, sd1[lo:hi], -1.0)

        s2, d2, key2 = e_core2[c]
        pos2 = np.empty(len(s2), np.int64)
        ptr = 0
        for k in np.unique(key2):
            n = int(cnt2[c][k])
            pos2[ptr:ptr + n] = off2[k] + np.arange(n)
            ptr += n
        gi2 = np.zeros(epad2, np.int16)
        gi2[pos2] = s2.astype(np.int16)
        sd2 = np.full(epad2, -1.0, np.float32)
        sd2[pos2] = (d2 % DW).astype(np.float32)
        dl2c = np.full((nu2p, 128), -1.0, np.float32)
        for u, (cc, blk, lo, hi) in enumerate(units2):
            p0 = lo - cc * 128
            dl2c[u, p0:hi - cc * 128] = sd2[lo:hi]

        dv = np.zeros(NPC, np.float32)
        dv[:RPC] = dinv[c * RPC:(c + 1) * RPC]
        dvt = dv.reshape(NB, 128).T.copy()
        gid = np.asarray(batch, np.int64)
        gv = np.full(NPC, -1.0, np.float32)
        gv[:RPC] = gid[c * RPC:(c + 1) * RPC].astype(np.float32)

        per_core.append({
            "gi1": _wrap16(gi1, epad1 // 16),
            "gi2": _wrap16(gi2, epad2 // 16),
            "dl1": dl1c.T.astype(ml_dtypes.bfloat16).copy(),
            "dl2": dl2c.T.astype(ml_dtypes.bfloat16).copy(),
            "dinv1": dvt,
            "dinv2": (dvt * dvt).copy(),
            "gid": gv.reshape(NB, 128).T.copy(),
        })

    gidn = np.asarray(batch, np.int64)
    cntg = np.bincount(gidn, minlength=G).astype(np.float32)
    inv_cnt_w = (1.0 / np.maximum(cntg, 1.0)).reshape(2, 128).T.copy()

    meta = {"epad1": epad1, "nch1": nch1, "units1": units1,
            "ustart1": ustart1, "ustop1": ustop1, "has1": has1, "nu1p": nu1p,
            "uslot1": uslot1,
            "epad2": epad2, "nch2": nch2, "units2": units2,
            "ustart2": ustart2, "ustop2": ustop2, "has2": has2, "nu2p": nu2p}
    return per_core, inv_cnt_w, dinv, meta


def _build(meta, bias_zero=True, stage=5):
    nc = bacc.Bacc("TRN2", target_bir_lowering=False, debug=False,
                   num_devices=NC, num_swdge_queues=1,
                   dynamic_dma_scratch_size=32768)
    epad1, nch1 = meta["epad1"], meta["nch1"]
    units1, ustart1, ustop1 = meta["units1"], meta["ustart1"], meta["ustop1"]
    has1, nu1p, uslot1 = meta["has1"], meta["nu1p"], meta["uslot1"]
    epad2, nch2 = meta["epad2"], meta["nch2"]
    units2, ustart2, ustop2 = meta["units2"], meta["ustart2"], meta["ustop2"]
    has2, nu2p = meta["has2"], meta["nu2p"]
    ecols1, ecols2 = epad1 // 16, epad2 // 16

    # ------------------------------------------------ I/O declarations
    xb4_d = nc.dram_tensor("xb4", [N // 4, 256], BF16, kind="ExternalInput")
    xst_d = nc.dram_tensor("xst", [IN, NPC], BF16, kind="ExternalInput")
    w1_d = nc.dram_tensor("w1", [IN, HID], BF16, kind="ExternalInput")
    w2_d = nc.dram_tensor("w2", [HID, HID], BF16, kind="ExternalInput")
    wfc_d = nc.dram_tensor("wfc", [HID, HID], F32, kind="ExternalInput")
    bfcr_d = nc.dram_tensor("bfcr", [128, HID], F32, kind="ExternalInput")
    gamr_d = nc.dram_tensor("gamr", [128, HID], F32, kind="ExternalInput")
    betr_d = nc.dram_tensor("betr", [128, HID], F32, kind="ExternalInput")
    if not bias_zero:
        b1r_d = nc.dram_tensor("b1r", [128, HID], F32, kind="ExternalInput")
        b2r_d = nc.dram_tensor("b2r", [128, HID], F32, kind="ExternalInput")
    gi1_d = nc.dram_tensor("gi1", [128, ecols1], I16, kind="ExternalInput")
    gi2_d = nc.dram_tensor("gi2", [128, ecols2], I16, kind="ExternalInput")
    dl1_d = nc.dram_tensor("dl1", [128, nu1p], BF16, kind="ExternalInput")
    dl2_d = nc.dram_tensor("dl2", [128, nu2p], BF16, kind="ExternalInput")
    gid_d = nc.dram_tensor("gid", [128, NB], F32, kind="ExternalInput")
    dinv1_d = nc.dram_tensor("dinv1", [128, NB], F32, kind="ExternalInput")
    dinv2_d = nc.dram_tensor("dinv2", [128, NB], F32, kind="ExternalInput")
    icnt_d = nc.dram_tensor("icnt", [128, 2], F32, kind="ExternalInput")

    hpart = nc.dram_tensor("hpart", [NPC, HID], BF16)
    u2t_k = [nc.dram_tensor(f"u2t{k}", [NC * HID, QLB[k] * DW], FP8)
             for k in range(4)]
    u2o_k = [nc.dram_tensor(f"u2o{k}", [HID, QLB[k] * DW], FP8)
             for k in range(4)]
    pool_loc = nc.dram_tensor("pool_loc", [G, HID], BF16)
    pool_glob = nc.dram_tensor("pool_glob", [G, HID], BF16,
                               addr_space="Shared")
    y_d = nc.dram_tensor("y", [G, HID], F32, kind="ExternalOutput")
    if stage == 2:
        dbg_h = nc.dram_tensor("dbg_h", [NPC, HID], BF16,
                               kind="ExternalOutput")
    if stage == 4:
        dbg_v = [nc.dram_tensor(f"dbg_v{k}", [HID, QLB[k] * DW], FP8,
                                kind="ExternalOutput") for k in range(4)]
    if stage == 3:
        dbg_u = [nc.dram_tensor(f"dbg_u{k}", [NC * HID, QLB[k] * DW], FP8,
                                kind="ExternalOutput") for k in range(4)]

    eye_d = nc.inline_tensor(np.eye(128, dtype=np.float32), name="eye128")
    eyeb_d = nc.inline_tensor(np.eye(128, dtype=ml_dtypes.bfloat16),
                              name="eye128b")
    iotaU_np = np.tile(np.arange(DW, dtype=np.float32),
                       (128, UB)).astype(ml_dtypes.bfloat16)
    iotaU_d = nc.inline_tensor(iotaU_np, name="iotaU")
    iotaF_d = nc.inline_tensor(
        np.tile(np.arange(DW, dtype=np.float32),
                (128, 1)).astype(ml_dtypes.bfloat16), name="iotaF")
    iota256_d = nc.inline_tensor(
        np.tile(np.arange(256, dtype=np.float32), (128, 1)), name="iota256")


    xb4 = xb4_d.ap()
    hp_rows = hpart.ap()
    CORES = [list(range(NC))]

    # persistent SBUF
    gi1_s = nc.alloc_sbuf_tensor("gi1_s", [128, ecols1], I16)
    gi2_s = nc.alloc_sbuf_tensor("gi2_s", [128, ecols2], I16)
    dl1_s = nc.alloc_sbuf_tensor("dl1_s", [128, nu1p], BF16)
    dl2_s = nc.alloc_sbuf_tensor("dl2_s", [128, nu2p], BF16)
    xst_s = nc.alloc_sbuf_tensor("xst_s", [IN, NPC], BF16)
    hpsbT = nc.alloc_sbuf_tensor("hpsbT", [128, NB * 128], BF16)

    with TileContext(nc) as tc:
        with tc.tile_pool(name="init", bufs=1) as ipool:
            nc.sync.dma_start(out=gi1_s[:], in_=gi1_d[:])
            nc.sync.dma_start(out=gi2_s[:], in_=gi2_d[:])
            nc.sync.dma_start(out=dl1_s[:], in_=dl1_d[:])
            nc.sync.dma_start(out=dl2_s[:], in_=dl2_d[:])
            nc.sync.dma_start(out=xst_s[:], in_=xst_d[:])
            eye_t = ipool.tile([128, 128], F32)
            nc.sync.dma_start(out=eye_t[:], in_=eye_d[:])
            eyeb_t = ipool.tile([128, 128], BF16)
            nc.sync.dma_start(out=eyeb_t[:], in_=eyeb_d[:])
            iotaU_t = ipool.tile([128, UB * DW], BF16)
            nc.sync.dma_start(out=iotaU_t[:], in_=iotaU_d[:])
            iotaF_t = ipool.tile([128, DW], BF16)
            nc.sync.dma_start(out=iotaF_t[:], in_=iotaF_d[:])
            iota256_t = ipool.tile([128, 256], F32)
            nc.sync.dma_start(out=iota256_t[:], in_=iota256_d[:])

            w1_t = ipool.tile([IN, HID], BF16)
            nc.sync.dma_start(out=w1_t[:], in_=w1_d[:])
            w2_t = ipool.tile([HID, HID], BF16)
            nc.sync.dma_start(out=w2_t[:], in_=w2_d[:])
            gid_t = ipool.tile([128, NB], F32)
            nc.sync.dma_start(out=gid_t[:], in_=gid_d[:])

            dinv1_t = ipool.tile([128, NB], F32)
            nc.sync.dma_start(out=dinv1_t[:], in_=dinv1_d[:])
            dinv2_t = ipool.tile([128, NB], F32)
            nc.sync.dma_start(out=dinv2_t[:], in_=dinv2_d[:])
            ones_t = ipool.tile([128, 1], BF16)
            nc.vector.memset(ones_t[:], 1.0)
            if not bias_zero:
                b1r_t = ipool.tile([128, HID], F32)
                nc.sync.dma_start(out=b1r_t[:], in_=b1r_d[:])
                b2r_t = ipool.tile([128, HID], F32)
                nc.sync.dma_start(out=b2r_t[:], in_=b2r_d[:])

            hp_r = hpart.ap().rearrange("(a p) f -> p a f", p=128)
            u2t_r = [t.ap().rearrange("(r p) n -> r p n", p=HID)
                     for t in u2t_k]

            # ======================= conv1 =======================
            with (
                tc.tile_pool(name="g1", bufs=4) as gpool,
                tc.tile_pool(name="s1", bufs=6) as spool,
                tc.tile_pool(name="r1", bufs=3) as rpool,
                tc.tile_pool(name="pa1", bufs=3, space="PSUM") as papool,
                tc.tile_pool(name="ph1", bufs=2, space="PSUM") as phpool,
                tc.tile_pool(name="pt1", bufs=2, space="PSUM") as ptpool,
            ):
                mt = {}

                def gather1(w0):
                    wh = min(w0 + W1CH, nch1)
                    t = gpool.tile([128, W1CH, 256], BF16, tag="m1")
                    nc.gpsimd.dma_gather(
                        t[:, :wh - w0, :], xb4, gi1_s[:, w0 * 8:wh * 8],
                        (wh - w0) * 128, (wh - w0) * 128, 256,
                        queue_num=0, single_packet=False)
                    mt.clear()
                    mt[w0] = t

                stile = {}

                def sbuild(u0, nu, dl_s, act_every, sb_i):
                    uh = min(u0 + UB, nu)
                    t = spool.tile([128, UB, DW], BF16, tag="s")
                    if sb_i % act_every == act_every - 1:
                        for j in range(uh - u0):
                            tq = spool.tile([128, DW], BF16, tag="tq")
                            nc.scalar.activation(
                                tq[:], iotaF_t[:],
                                mybir.ActivationFunctionType.Square,
                                bias=dl_s[:, u0 + j:u0 + j + 1], scale=-1.0)
                            nc.scalar.activation(
                                t[:, j, :], tq[:],
                                mybir.ActivationFunctionType.Relu,
                                bias=ones_t[:], scale=-1.0)
                    else:
                        nc.vector.tensor_tensor(
                            t[:, :uh - u0, :],
                            dl_s[:, u0:uh].rearrange("p (u x) -> p u x", x=1)
                            .broadcast_to([128, uh - u0, DW]),
                            iotaU_t[:, :(uh - u0) * DW]
                            .rearrange("p (u x) -> p u x", x=DW),
                            mybir.AluOpType.is_equal)
                    stile.clear()
                    stile[u0] = t

                # conv1: retire group = G1 128-blocks = 2*G1 64-blocks
                B64G = 2 * G1

                def retire1(g, aggP):
                    b0 = g * G1                      # first 128-block
                    nb_ = min(G1, NB - b0)
                    zs = rpool.tile([64, G1 * 128], BF16, tag="zsb")
                    if aggP is not None:
                        nc.scalar.activation(
                            zs[:, :nb_ * 128], aggP[:, :nb_ * 128],
                            mybir.ActivationFunctionType.Copy)
                    for j64 in range(nb_ * 2):
                        if not has1[g * B64G + j64]:
                            nc.vector.memset(
                                zs[:, j64 * DW:(j64 + 1) * DW], 0.0)
                    hps = phpool.tile([128, G1, 128], F32, tag="hps")
                    hg = rpool.tile([128, G1, 128], BF16, tag="hg")
                    for j in range(nb_):
                        blk = b0 + j
                        nc.tensor.matmul(hps[:, j, :],
                                         zs[:, j * 128:(j + 1) * 128],
                                         w1_t[:], start=True, stop=False)
                        nc.tensor.matmul(
                            hps[:, j, :],
                            xst_s[:, blk * 128:(blk + 1) * 128],
                            w1_t[:], start=False, stop=True)
                        if bias_zero:
                            nc.scalar.activation(
                                hg[:, j, :], hps[:, j, :],
                                mybir.ActivationFunctionType.Relu,
                                scale=dinv2_t[:, blk:blk + 1])
                        else:
                            hb = rpool.tile([128, HID], F32, tag="hb")
                            nc.vector.tensor_scalar(
                                hb[:], hps[:, j, :],
                                dinv1_t[:, blk:blk + 1], None,
                                mybir.AluOpType.mult)
                            nc.vector.tensor_add(hb[:], hb[:], b1r_t[:])
                            hr = rpool.tile([128, HID], F32, tag="hr")
                            nc.scalar.activation(
                                hr[:], hb[:],
                                mybir.ActivationFunctionType.Relu)
                            nc.vector.tensor_scalar(
                                hg[:, j, :], hr[:],
                                dinv1_t[:, blk:blk + 1], None,
                                mybir.AluOpType.mult)
                        tp = ptpool.tile([128, 128], BF16, tag="tp")
                        nc.tensor.transpose(tp[:], hg[:, j, :], eyeb_t[:])
                        nc.scalar.activation(
                            hpsbT[:, blk * 128:(blk + 1) * 128], tp[:],
                            mybir.ActivationFunctionType.Copy)
                    nc.sync.dma_start(out=hp_r[:, b0:b0 + nb_, :],
                                        in_=hg[:, :nb_, :])

                aggP = None
                cur_grp = -1
                retired = set()
                sb_i = 0
                for u, (cc, blk, lo, hi) in enumerate(units1):
                    w0 = cc // W1CH * W1CH
                    if w0 not in mt:
                        gather1(w0)
                    u0 = u // UB * UB
                    if u0 not in stile:
                        sbuild(u0, len(units1), dl1_s, ACT1, sb_i)
                        sb_i += 1
                    g = blk // B64G
                    if g != cur_grp:
                        if cur_grp >= 0:
                            retire1(cur_grp, aggP)
                            retired.add(cur_grp)
                        cur_grp = g
                        aggP = papool.tile([64, B64G * DW], F32, tag="agg")
                    j = blk - g * B64G
                    q = uslot1[u]
                    nc.tensor.matmul(
                        aggP[:, j * DW:(j + 1) * DW],
                        mt[w0][:, cc - w0, 64 * q:64 * q + 64],
                        stile[u0][:, u - u0, :],
                        start=ustart1[u], stop=ustop1[u])
                if cur_grp >= 0:
                    retire1(cur_grp, aggP)
                    retired.add(cur_grp)
                for g in range((NB + G1 - 1) // G1):
                    if g not in retired:
                        retire1(g, None)

            if stage == 2:
                nc.sync.dma_start(out=dbg_h[:], in_=hpart[:])

            # ============== conv2 + chunked RS + pipelined mm2 ==========
            if stage >= 3:
                pools2 = [
                    tc.tile_pool(name="g2", bufs=4),
                    tc.tile_pool(name="s2", bufs=6),
                    tc.tile_pool(name="r2", bufs=4),
                    tc.tile_pool(name="pa2", bufs=3, space="PSUM"),
                    tc.tile_pool(name="mm2", bufs=3),
                    tc.tile_pool(name="ps2", bufs=1, space="PSUM"),
                    tc.tile_pool(name="pacc", bufs=1, space="PSUM"),
                ]
                gpool, spool, rpool, papool, mpool, ppool, apool = [
                    p.__enter__() for p in pools2]
                mt = {}
                stile = {}

                def gather2(w0):
                    wh = min(w0 + W2CH, nch2)
                    t = gpool.tile([128, W2CH, HID], BF16, tag="m2")
                    nc.gpsimd.dma_gather(
                        t[:, :wh - w0, :], hp_rows,
                        gi2_s[:, w0 * 8:wh * 8],
                        (wh - w0) * 128, (wh - w0) * 128, HID,
                        queue_num=0, single_packet=False)
                    mt.clear()
                    mt[w0] = t

                def sbuild2(u0, sb_i):
                    uh = min(u0 + UB, len(units2))
                    t = spool.tile([128, UB, DW], BF16, tag="s")
                    if sb_i % ACT2 == ACT2 - 1:
                        for j in range(uh - u0):
                            tq = spool.tile([128, DW], BF16, tag="tq")
                            nc.scalar.activation(
                                tq[:], iotaF_t[:],
                                mybir.ActivationFunctionType.Square,
                                bias=dl2_s[:, u0 + j:u0 + j + 1],
                                scale=-1.0)
                            nc.scalar.activation(
                                t[:, j, :], tq[:],
                                mybir.ActivationFunctionType.Relu,
                                bias=ones_t[:], scale=-1.0)
                    else:
                        nc.vector.tensor_tensor(
                            t[:, :uh - u0, :],
                            dl2_s[:, u0:uh].rearrange("p (u x) -> p u x", x=1)
                            .broadcast_to([128, uh - u0, DW]),
                            iotaU_t[:, :(uh - u0) * DW]
                            .rearrange("p (u x) -> p u x", x=DW),
                            mybir.AluOpType.is_equal)
                    stile.clear()
                    stile[u0] = t

                ret_i = [0]
                qdmas = [[], [], [], []]     # retire DMA insts per quarter
                stage_state = {}             # (k, r) -> [tile, filled_set]

                flushed_qr = set()

                def flush_qr(k, r):
                    tile, filled = stage_state.pop((k, r))
                    ngrp = QLB[k] // G2
                    for gl in range(ngrp):
                        if gl not in filled:
                            nc.vector.memset(
                                tile[:, gl * G2 * DW:(gl + 1) * G2 * DW],
                                0.0)
                    dma = nc.gpsimd.dma_start(out=u2t_r[k][r, :, :],
                                              in_=tile[:, :QLB[k] * DW])
                    qdmas[k].append(dma)
                    flushed_qr.add((k, r))

                def retire2(g, aggP):
                    # g: group index over ordered blocks (G2 64-blocks)
                    ob0 = g * G2
                    b_glob = _B2ORDER[ob0]
                    r = b_glob // NB64
                    lb = b_glob % NB64
                    k = _q_of_lb(lb)
                    gl = (lb - _Q0[k]) // G2         # group within (k, r)
                    ngrp = QLB[k] // G2
                    if (k, r) not in stage_state:
                        tag = "u2sA" if QLB[k] == 56 else "u2sB"
                        st_t = rpool.tile([128, QLB[k] * DW], FP8, tag=tag,
                                          name=tag)
                        stage_state[(k, r)] = [st_t, set()]
                    tile, filled = stage_state[(k, r)]
                    sl = tile[:, gl * G2 * DW:(gl + 1) * G2 * DW]
                    eng = [nc.scalar, nc.vector, nc.scalar][ret_i[0] % 3]
                    ret_i[0] += 1
                    if eng is nc.scalar:
                        nc.scalar.activation(
                            sl, aggP[:], mybir.ActivationFunctionType.Copy)
                    else:
                        nc.vector.tensor_copy(sl, aggP[:])
                    for j in range(G2):
                        if not has2[ob0 + j]:
                            nc.vector.memset(
                                tile[:, (gl * G2 + j) * DW:
                                     (gl * G2 + j + 1) * DW], 0.0)
                    filled.add(gl)
                    if len(filled) == ngrp:
                        flush_qr(k, r)

                # ---- mm2 chunk consumer (pooled psum held across chunks)
                pooled = [apool.tile([128, HID], F32, tag=f"pool{h}",
                                     name=f"pooled{h}")
                          for h in range(2)]
                NB128Q = [q // 2 for q in QLB]       # 128-blocks per chunk
                LB128Q = [q // 2 for q in _Q0]       # first 128-block

                def mm2_chunk(k):
                    u2o = u2o_k[k].ap()
                    first = (k == 0)
                    last = (k == 3)
                    nblk = NB128Q[k]
                    for gg in range(nblk // G3):
                        b0 = LB128Q[k] + gg * G3     # absolute 128-block
                        c0 = gg * G3 * 128
                        ga = mpool.tile([128, G3 * 128], FP8, tag="ga")
                        gd = nc.sync.dma_start(out=ga[:],
                                               in_=u2o[:, c0:c0 + G3 * 128])
                        _add_dep_helper(gd.ins, rs_cc[k].ins, True,
                                        f"mm2 chunk {k} reads RS{k}")
                        sel7 = mpool.tile([128, G3, 256], BF16, tag="sel7")
                        nc.vector.tensor_tensor(
                            sel7[:],
                            gid_t[:, b0:b0 + G3]
                            .rearrange("p (u x) -> p u x", x=1)
                            .broadcast_to([128, G3, 256]),
                            iota256_t[:]
                            .rearrange("p (u x) -> p u x", u=1)
                            .broadcast_to([128, G3, 256]),
                            mybir.AluOpType.is_equal)
                        z = mpool.tile([128, G3 * 128], BF16, tag="z2")
                        nc.vector.tensor_add(
                            z[:], ga[:],
                            hpsbT[:, b0 * 128:(b0 + G3) * 128])
                        h2p = ppool.tile([128, G3, HID], F32, tag="h2p")
                        for j in range(G3):
                            blk = b0 + j
                            nc.tensor.matmul(
                                h2p[:, j, :], z[:, j * 128:(j + 1) * 128],
                                w2_t[:], start=True, stop=True)
                            h2s = mpool.tile([128, HID], BF16, tag="h2s")
                            if bias_zero:
                                nc.scalar.activation(
                                    h2s[:], h2p[:, j, :],
                                    mybir.ActivationFunctionType.Relu,
                                    scale=dinv1_t[:, blk:blk + 1])
                            else:
                                hb2 = mpool.tile([128, HID], F32, tag="hb2")
                                nc.vector.tensor_scalar(
                                    hb2[:], h2p[:, j, :],
                                    dinv1_t[:, blk:blk + 1], None,
                                    mybir.AluOpType.mult)
                                nc.vector.tensor_add(hb2[:], hb2[:],
                                                     b2r_t[:])
                                nc.scalar.activation(
                                    h2s[:], hb2[:],
                                    mybir.ActivationFunctionType.Relu)
                            st = first and gg == 0 and j == 0
                            sp = last and gg == nblk // G3 - 1 and j == G3 - 1
                            for hh in range(2):
                                nc.tensor.matmul(
                                    pooled[hh][:],
                                    sel7[:, j, hh * 128:(hh + 1) * 128],
                                    h2s[:], start=st, stop=sp)

                # ---- conv2 main loop with interleaved RS / mm2
                q_last_grp = []                      # last retire group per q
                acc = 0
                for k in range(4):
                    q_last_grp.append((acc + QLB[k]) // G2 - 1)
                    acc += QLB[k]

                rs_emitted = []
                rs_cc = {}

                def emit_rs(k):
                    cc = nc.gpsimd.collective_compute(
                        "ReduceScatter", mybir.AluOpType.add, CORES,
                        [u2t_k[k][:]], [u2o_k[k][:]])
                    for d in qdmas[k]:
                        _add_dep_helper(cc.ins, d.ins, True,
                                        f"RS{k} waits quarter writes")
                    if rs_emitted:
                        _add_dep_helper(cc.ins, rs_cc[rs_emitted[-1]].ins,
                                        True, "collective order")
                    rs_cc[k] = cc
                    rs_emitted.append(k)

                aggP = None
                cur_grp = -1
                retired2 = set()
                sb_i = 0

                rs_ready = []

                def retire_and_track(g, aggP):
                    retire2(g, aggP)
                    retired2.add(g)
                    for k in range(4):
                        if g == q_last_grp[k]:
                            lo_g = q_last_grp[k - 1] + 1 if k else 0
                            if all(gg in retired2
                                   for gg in range(lo_g, g + 1)):
                                rs_ready.append(k)

                def maybe_emit_pending(blk):
                    # fire a ready RS once the loop is half-way through the
                    # NEXT quarter (gather descs pre-generated = DMA runway)
                    if not rs_ready:
                        return
                    k = rs_ready[0]
                    b_glob = _B2ORDER[blk // G2 * G2]
                    kq = _q_of_lb(b_glob % NB64)
                    r = b_glob // NB64
                    if kq > k + 1 or (kq == k + 1 and r >= 4):
                        rs_ready.pop(0)
                        emit_rs(k)
                        if stage >= 5 and k >= 1:
                            mm2_chunk(k - 1)

                for u, (cc, blk, lo, hi) in enumerate(units2):
                    w0 = cc // W2CH * W2CH
                    if w0 not in mt:
                        gather2(w0)
                    u0 = u // UB * UB
                    if u0 not in stile:
                        sbuild2(u0, sb_i)
                        sb_i += 1
                    g = blk // G2
                    maybe_emit_pending(blk)
                    if g != cur_grp:
                        if cur_grp >= 0:
                            retire_and_track(cur_grp, aggP)
                        cur_grp = g
                        aggP = papool.tile([128, G2 * DW], F32, tag="agg2")
                    j = blk - g * G2
                    nc.tensor.matmul(
                        aggP[:, j * DW:(j + 1) * DW], mt[w0][:, cc - w0, :],
                        stile[u0][:, u - u0, :],
                        start=ustart2[u], stop=ustop2[u])
                if cur_grp >= 0:
                    retire_and_track(cur_grp, aggP)
                # flush any incomplete / absent (quarter, range) staging
                for k in list(rs_ready):
                    pass
                for k in range(4):
                    for r in range(NC):
                        if (k, r) in stage_state:
                            flush_qr(k, r)
                        elif (k, r) not in flushed_qr:
                            tag = "u2sA" if QLB[k] == 56 else "u2sB"
                            st_t = rpool.tile([128, QLB[k] * DW], FP8,
                                              tag=tag, name=tag)
                            stage_state[(k, r)] = [st_t, set()]
                            flush_qr(k, r)
                    if k not in rs_emitted:
                        emit_rs(k)
                        if stage >= 5 and k >= 1:
                            mm2_chunk(k - 1)

                if stage == 4:
                    for k in range(4):
                        dd = nc.sync.dma_start(out=dbg_v[k][:],
                                               in_=u2o_k[k][:])
                        _add_dep_helper(dd.ins, rs_cc[k].ins, True, "dbg")
                if stage == 3:
                    for k in range(4):
                        dd = nc.sync.dma_start(out=dbg_u[k][:],
                                               in_=u2t_k[k][:])
                        for d in qdmas[k]:
                            _add_dep_helper(dd.ins, d.ins, True, "dbgu")

                pl_dma = None
                if stage >= 5:
                    mm2_chunk(3)
                    pl_r = pool_loc.ap().rearrange("(h p) f -> p h f", p=128)
                    pl_s = mpool.tile([128, 2, HID], BF16, tag="pls")
                    nc.vector.tensor_copy(pl_s[:, 0, :], pooled[0][:])
                    nc.vector.tensor_copy(pl_s[:, 1, :], pooled[1][:])
                    pl_dma = nc.sync.dma_start(out=pl_r[:], in_=pl_s[:])

                for p in reversed(pools2):
                    p.__exit__(None, None, None)

            if stage >= 5:
                ar_cc = nc.gpsimd.collective_compute(
                    "AllReduce", mybir.AluOpType.add, CORES,
                    [pool_loc[:]], [pool_glob[:]],
                )
                _add_dep_helper(ar_cc.ins, pl_dma.ins, True,
                                "AR waits pooled write")
                _add_dep_helper(ar_cc.ins, rs_cc[3].ins, True,
                                "collective order")

                # ---------------- head: mean-div, fc, LayerNorm (tiny)
                pg_r = pool_glob.ap().rearrange("(h p) f -> p h f", p=128)
                y_r = y_d.ap().rearrange("(h p) f -> p h f", p=128)
                with (
                    tc.tile_pool(name="head", bufs=1) as hpool,
                    tc.tile_pool(name="psh", bufs=2, space="PSUM") as hps,
                ):
                    wfc_t = hpool.tile([HID, HID], F32)
                    nc.sync.dma_start(out=wfc_t[:], in_=wfc_d[:])
                    bfcr_t = hpool.tile([128, HID], F32)
                    nc.sync.dma_start(out=bfcr_t[:], in_=bfcr_d[:])
                    gamr_t = hpool.tile([128, HID], F32)
                    nc.sync.dma_start(out=gamr_t[:], in_=gamr_d[:])
                    betr_t = hpool.tile([128, HID], F32)
                    nc.sync.dma_start(out=betr_t[:], in_=betr_d[:])
                    icnt_t = hpool.tile([128, 2], F32)
                    nc.sync.dma_start(out=icnt_t[:], in_=icnt_d[:])
                    eps_t = hpool.tile([128, 1], F32)
                    nc.vector.memset(eps_t[:], LN_EPS)
                    yo = hpool.tile([128, 2, HID], F32)
                    for hh in range(2):
                        pgb = hpool.tile([128, HID], BF16, tag="pgb")
                        pgd = nc.sync.dma_start(out=pgb[:],
                                                in_=pg_r[:, hh, :])
                        _add_dep_helper(pgd.ins, ar_cc.ins, True,
                                        "head reads AllReduce output")
                        pg_s = hpool.tile([128, HID], F32, tag="pg")
                        nc.vector.tensor_scalar(
                            pg_s[:], pgb[:], icnt_t[:, hh:hh + 1], None,
                            mybir.AluOpType.mult)
                        pgT_p = hps.tile([HID, 128], F32, tag="pgT")
                        nc.tensor.transpose(pgT_p[:], pg_s[:], eye_t[:])
                        pgT_s = hpool.tile([HID, 128], F32, tag="pgTs")
                        nc.vector.tensor_copy(pgT_s[:], pgT_p[:])
                        y_p = hps.tile([128, HID], F32, tag="yp")
                        nc.tensor.matmul(y_p[:], pgT_s[:], wfc_t[:])
                        y_s = hpool.tile([128, HID], F32, tag="ys")
                        nc.vector.tensor_add(y_s[:], y_p[:], bfcr_t[:])
                        mu = hpool.tile([128, 1], F32, tag="mu")
                        nc.vector.tensor_reduce(mu[:], y_s[:],
                                                mybir.AxisListType.XYZW,
                                                mybir.AluOpType.add)
                        nc.vector.tensor_scalar(mu[:], mu[:], -1.0 / HID,
                                                None, mybir.AluOpType.mult)
                        cen = hpool.tile([128, HID], F32, tag="cen")
                        nc.vector.tensor_scalar(cen[:], y_s[:], mu[:], None,
                                                mybir.AluOpType.add)
                        sq = hpool.tile([128, HID], F32, tag="sq")
                        nc.vector.tensor_mul(sq[:], cen[:], cen[:])
                        var = hpool.tile([128, 1], F32, tag="var")
                        nc.vector.tensor_reduce(var[:], sq[:],
                                                mybir.AxisListType.XYZW,
                                                mybir.AluOpType.add)
                        std = hpool.tile([128, 1], F32, tag="std")
                        nc.scalar.activation(
                            std[:], var[:],
                            mybir.ActivationFunctionType.Sqrt,
                            bias=eps_t[:], scale=1.0 / HID)
                        rstd = hpool.tile([128, 1], F32, tag="rstd")
                        nc.vector.reciprocal(rstd[:], std[:])
                        nc.vector.tensor_scalar(cen[:], cen[:], rstd[:],
                                                None, mybir.AluOpType.mult)
                        nc.vector.tensor_mul(cen[:], cen[:], gamr_t[:])
                        nc.vector.tensor_add(yo[:, hh, :], cen[:], betr_t[:])
                    nc.sync.dma_start(out=y_r[:], in_=yo[:])

    nc.compile()
    return nc


_CACHE = {}


def make_in_maps(x, edge_index, batch, W1, b1, W2, b2, Wfc, bfc, gamma, beta,
                 per_core=None, inv_cnt_w=None, dinv=None, meta=None):
    if per_core is None:
        per_core, inv_cnt_w, dinv, meta = _host_prep(
            np.asarray(edge_index), np.asarray(batch))
    x = np.asarray(x, np.float32)
    xp = x * dinv[:, None]
    xb4 = xp.astype(ml_dtypes.bfloat16).reshape(N // 4, 256)
    xself = (xp * dinv[:, None]).astype(np.float32)
    rep = lambda v: np.tile(np.asarray(v, np.float32)[None, :], (128, 1))
    bias_zero = (not np.any(np.asarray(b1))) and (not np.any(np.asarray(b2)))
    shared = {
        "xb4": xb4,
        "w1": np.asarray(W1, np.float32).astype(ml_dtypes.bfloat16),
        "w2": np.asarray(W2, np.float32).astype(ml_dtypes.bfloat16),
        "wfc": np.asarray(Wfc, np.float32),
        "bfcr": rep(bfc),
        "gamr": rep(gamma), "betr": rep(beta),
        "icnt": inv_cnt_w,
    }
    if not bias_zero:
        shared["b1r"] = rep(b1)
        shared["b2r"] = rep(b2)
    in_maps = []
    for c in range(NC):
        m = dict(shared)
        xs = np.zeros((IN, NPC), np.float32)
        xs[:, :RPC] = xself[c * RPC:(c + 1) * RPC].T
        m["xst"] = xs.astype(ml_dtypes.bfloat16)
        for k in ("gi1", "gi2", "dl1", "dl2", "gid",
                  "dinv1", "dinv2"):
            m[k] = per_core[c][k]
        in_maps.append(m)
    return in_maps, bias_zero, meta


def kernel(x, edge_index, batch, W1, b1, W2, b2, Wfc, bfc, gamma, beta,
           _stage=5, _full_results=False):
    per_core, inv_cnt_w, dinv, meta = _host_prep(np.asarray(edge_index),
                                                 np.asarray(batch))
    in_maps, bias_zero, meta = make_in_maps(
        x, edge_index, batch, W1, b1, W2, b2, Wfc, bfc, gamma, beta,
        per_core, inv_cnt_w, dinv, meta)
    key = (meta["epad1"], meta["epad2"], meta["nu1p"], meta["nu2p"],
           bias_zero, _stage)
    if key not in _CACHE:
        _CACHE[key] = _build(meta, bias_zero, _stage)
    nc = _CACHE[key]

    res = run_bass_kernel_spmd(nc, in_maps, list(range(NC)))
    if _full_results:
        return res.results
    return res.results[0]["y"]


# revision 72
# speedup vs baseline: 1.0172x; 1.0172x over previous
"""GCN (2x GCNConv + mean-pool + fc + LayerNorm) on 8 Trainium2 NeuronCores.

One-hot matmul aggregation replaces DMA scatter-adds entirely:

conv1 (pull): per-core in-edges on a cross-core-common segment grid
(64-node dst block x src%4 slot, segment length = max over cores). Each
128-edge chunk is gathered with one 512B descriptor per edge from a
4-row-packed bf16 x' table (the edge's row is slot src%4). For each
(chunk x segment) overlap ("unit"), PE accumulates
zT[64f, 64d] += M[128e, 64f].T @ S[128e, 64d], where S is a one-hot built
from a per-unit dstloc column: batched is_equal-vs-iota on DVE, with a
share built on the Activation engine as relu(1 - (iota - dstloc)^2).
The implicit self-loop is a second matmul against a transposed prescaled
x table, so it never enters the edge stream, and mm1 needs no transpose
(zT is the lhsT directly).

conv2 (push): per-core out-edges on a common global 64-node dst-block
grid, blocks ordered (quarter, range, local) so each node-quarter of the
TRANSPOSED partials u2T completes early; gathers hit the local hpart rows
(h1' = relu(h1)*dinv, prescaled). FOUR chunked ReduceScatters fire as
their quarter's writes land, overlapping the next quarter's compute, and
mm2 consumes each reduced chunk one quarter later (no transposes: z2T is
the lhsT; the self term adds from an SBUF-resident transposed hpsbT).
Pooling is the PSUM-accumulated one-hot graph matmul; pooled sums are
AllReduced; the tiny fc+LayerNorm head is computed redundantly.
"""
import sys

if '/opt/trn_rl_repo' not in sys.path:
    sys.path.insert(0, '/opt/trn_rl_repo')

import ml_dtypes
import numpy as np

import concourse.bacc as bacc
import concourse.mybir as mybir
from concourse.bass import _add_dep_helper
from concourse.tile import TileContext
from concourse.bass_utils import run_bass_kernel_spmd

# ---------------------------------------------------------------- constants
N = 100000
E = 800000
IN = 64
HID = 128
G = 256
NC = 8
RPC = N // NC              # 12500 nodes per core
NB = 98                    # 128-node dst blocks per core (12544 padded)
NPC = NB * 128             # 12544
DW = 64                    # aggregation dst-block width
NB64 = NPC // DW           # 196 64-blocks per core
NBG64 = NC * NB64          # 1568 global 64-blocks
QLB = (56, 56, 56, 28)     # conv2 RS chunks (in 64-blocks per core)
LN_EPS = 1e-5
W1CH = 16                  # conv1 gather window (chunks per call)
W2CH = 32                  # conv2 gather window
UB = 32                    # S-build batch (units per instruction)
ACT1 = 10 ** 9             # conv1: every ACT1-th S-batch goes to Act engine
ACT2 = 10 ** 9             # conv2: every ACT2-th S-batch goes to Act engine
G1 = 4                     # conv1 retire group (128-blocks)
G2 = 7                     # conv2 retire group (64-blocks; divides 56)
G3 = 7                     # mm2 group (128-blocks)
F32 = mybir.dt.float32
FP8 = mybir.dt.float8e4
BF16 = mybir.dt.bfloat16
I16 = mybir.dt.int16


def _wrap16(a, cols):
    """[n] -> [128, cols] int16: element i -> [i%16, i//16], tiled x8."""
    out = np.zeros((16, cols), np.int16)
    w = a.reshape(-1, 16).T
    out[:, : w.shape[1]] = w
    return np.tile(out, (8, 1))


def _common_grid(counts):
    seg = counts.max(axis=0).astype(np.int64)
    off = np.concatenate([[0], np.cumsum(seg)])
    epad = int((off[-1] + 127) // 128 * 128)
    return seg, off, epad


def _make_units(seg, off, nblk, seg_per_blk):
    """(chunk, blk, lo, hi) overlaps of 128-edge chunks with segments.

    Segment k belongs to block k // seg_per_blk (blocks in segment order).
    Returns units, per-unit start/stop (first/last unit of its block), and
    per-block has_units.
    """
    units = []
    blk_first = {}
    blk_last = {}
    for k in range(len(seg)):
        if seg[k] == 0:
            continue
        blk = k // seg_per_blk
        a, b = int(off[k]), int(off[k] + seg[k])
        for cc in range(a // 128, (b - 1) // 128 + 1):
            lo, hi = max(a, cc * 128), min(b, (cc + 1) * 128)
            u = len(units)
            units.append((cc, blk, lo, hi))
            if blk not in blk_first:
                blk_first[blk] = u
            blk_last[blk] = u
    start = [False] * len(units)
    stop = [False] * len(units)
    for blk, u in blk_first.items():
        start[u] = True
    for blk, u in blk_last.items():
        stop[u] = True
    has = [blk in blk_first for blk in range(nblk)]
    return units, start, stop, has


def _q_of_lb(lb):
    """conv2 node-quarter of a local 64-block index."""
    acc = 0
    for q, n in enumerate(QLB):
        if lb < acc + n:
            return q
        acc += n
    raise ValueError(lb)


# conv2 ordered block list: (quarter, range, lb64) lexicographic
_B2ORDER = sorted(range(NBG64),
                  key=lambda b: (_q_of_lb(b % NB64), b // NB64, b % NB64))
_Q0 = [0, 56, 112, 168]            # quarter start lb64


def _host_prep(edge_index, batch):
    src = np.asarray(edge_index[0], np.int64)
    dst = np.asarray(edge_index[1], np.int64)
    deg = (np.bincount(dst, minlength=N) + 1.0).astype(np.float32)
    dinv = (1.0 / np.sqrt(deg)).astype(np.float32)

    # ---------------- conv1: (blk64, slot) grid
    e_core1 = []
    cnt1 = np.zeros((NC, NB64 * 4), np.int64)
    for c in range(NC):
        m = (dst >= c * RPC) & (dst < (c + 1) * RPC)
        s, d = src[m], dst[m] - c * RPC
        key = (d // DW) * 4 + (s % 4)
        cnt1[c] = np.bincount(key, minlength=NB64 * 4)
        order = np.lexsort((d, key))
        e_core1.append((s[order], d[order], key[order]))
    seg1, off1, epad1 = _common_grid(cnt1)
    units1, ustart1, ustop1, has1 = _make_units(seg1, off1, NB64, 4)
    nch1 = epad1 // 128
    nu1 = len(units1)
    nu1p = (nu1 + UB - 1) // UB * UB
    uslot1 = []
    for u, (cc, blk, lo, hi) in enumerate(units1):
        base = blk * 4
        q = 0
        for qq in range(4):
            a, b = off1[base + qq], off1[base + qq] + seg1[base + qq]
            if a <= lo < b:
                q = qq
                break
        uslot1.append(q)

    # ---------------- conv2: global 64-blocks in _B2ORDER
    e_core2 = []
    cnt2 = np.zeros((NC, NBG64), np.int64)   # indexed by ORDER position
    b2pos = np.empty(NBG64, np.int64)        # global blk64 -> order position
    for i, b in enumerate(_B2ORDER):
        b2pos[b] = i
    for c in range(NC):
        m = (src >= c * RPC) & (src < (c + 1) * RPC)
        s, d = src[m] - c * RPC, dst[m]
        rr = d // RPC                      # owner range of the dst
        dloc = d - rr * RPC                # dst local to its owner core
        key = b2pos[rr * NB64 + dloc // DW]
        cnt2[c] = np.bincount(key, minlength=NBG64)
        order = np.lexsort((dloc, key))
        e_core2.append((s[order], dloc[order], key[order]))
    seg2, off2, epad2 = _common_grid(cnt2)
    units2, ustart2, ustop2, has2 = _make_units(seg2, off2, NBG64, 1)
    nch2 = epad2 // 128
    nu2 = len(units2)
    nu2p = (nu2 + UB - 1) // UB * UB

    # ---------------- per-core streams
    per_core = []
    for c in range(NC):
        s, d, key = e_core1[c]
        pos = np.empty(len(s), np.int64)
        ptr = 0
        for k in np.unique(key):
            n = int(cnt1[c][k])
            pos[ptr:ptr + n] = off1[k] + np.arange(n)
            ptr += n
        gi1 = np.zeros(epad1, np.int16)
        gi1[pos] = (s // 4).astype(np.int16)
        sd1 = np.full(epad1, -1.0, np.float32)
        sd1[pos] = (d % DW).astype(np.float32)
        sl1 = np.full(epad1, -1, np.int8)
        sl1[pos] = (s % 4).astype(np.int8)
        dl1c = np.full((nu1p, 128), -1.0, np.float32)
        for u, (cc, blk, lo, hi) in enumerate(units1):
            p0 = lo - cc * 128
            q = uslot1[u]
            dl1c[u, p0:hi - cc * 128] = np.where(
                sl1[lo:hi] == q, sd1[lo:hi], -1.0)

        s2, d2, key2 = e_core2[c]
        pos2 = np.empty(len(s2), np.int64)
        ptr = 0
        for k in np.unique(key2):
            n = int(cnt2[c][k])
            pos2[ptr:ptr + n] = off2[k] + np.arange(n)
            ptr += n
        gi2 = np.zeros(epad2, np.int16)
        gi2[pos2] = s2.astype(np.int16)
        sd2 = np.full(epad2, -1.0, np.float32)
        sd2[pos2] = (d2 % DW).astype(np.float32)
        dl2c = np.full((nu2p, 128), -1.0, np.float32)
        for u, (cc, blk, lo, hi) in enumerate(units2):
            p0 = lo - cc * 128
            dl2c[u, p0:hi - cc * 128] = sd2[lo:hi]

        dv = np.zeros(NPC, np.float32)
        dv[:RPC] = dinv[c * RPC:(c + 1) * RPC]
        dvt = dv.reshape(NB, 128).T.copy()
        gid = np.asarray(batch, np.int64)
        gv = np.full(NPC, -1.0, np.float32)
        gv[:RPC] = gid[c * RPC:(c + 1) * RPC].astype(np.float32)

        per_core.append({
            "gi1": _wrap16(gi1, epad1 // 16),
            "gi2": _wrap16(gi2, epad2 // 16),
            "dl1": dl1c.T.astype(ml_dtypes.bfloat16).copy(),
            "dl2": dl2c.T.astype(ml_dtypes.bfloat16).copy(),
            "dinv1": dvt,
            "dinv2": (dvt * dvt).copy(),
            "gid": gv.reshape(NB, 128).T.copy(),
        })

    gidn = np.asarray(batch, np.int64)
    cntg = np.bincount(gidn, minlength=G).astype(np.float32)
    inv_cnt_w = (1.0 / np.maximum(cntg, 1.0)).reshape(2, 128).T.copy()

    meta = {"epad1": epad1, "nch1": nch1, "units1": units1,
            "ustart1": ustart1, "ustop1": ustop1, "has1": has1, "nu1p": nu1p,
            "uslot1": uslot1,
            "epad2": epad2, "nch2": nch2, "units2": units2,
            "ustart2": ustart2, "ustop2": ustop2, "has2": has2, "nu2p": nu2p}
    return per_core, inv_cnt_w, dinv, meta


def _build(meta, bias_zero=True, stage=5):
    nc = bacc.Bacc("TRN2", target_bir_lowering=False, debug=False,
                   num_devices=NC, num_swdge_queues=1,
                   dynamic_dma_scratch_size=32768)
    epad1, nch1 = meta["epad1"], meta["nch1"]
    units1, ustart1, ustop1 = meta["units1"], meta["ustart1"], meta["ustop1"]
    has1, nu1p, uslot1 = meta["has1"], meta["nu1p"], meta["uslot1"]
    epad2, nch2 = meta["epad2"], meta["nch2"]
    units2, ustart2, ustop2 = meta["units2"], meta["ustart2"], meta["ustop2"]
    has2, nu2p = meta["has2"], meta["nu2p"]
    ecols1, ecols2 = epad1 // 16, epad2 // 16

    # ------------------------------------------------ I/O declarations
    xb4_d = nc.dram_tensor("xb4", [N // 4, 256], BF16, kind="ExternalInput")
    xst_d = nc.dram_tensor("xst", [IN, NPC], BF16, kind="ExternalInput")
    w1_d = nc.dram_tensor("w1", [IN, HID], BF16, kind="ExternalInput")
    w2_d = nc.dram_tensor("w2", [HID, HID], BF16, kind="ExternalInput")
    wfc_d = nc.dram_tensor("wfc", [HID, HID], F32, kind="ExternalInput")
    bfcr_d = nc.dram_tensor("bfcr", [128, HID], F32, kind="ExternalInput")
    gamr_d = nc.dram_tensor("gamr", [128, HID], F32, kind="ExternalInput")
    betr_d = nc.dram_tensor("betr", [128, HID], F32, kind="ExternalInput")
    if not bias_zero:
        b1r_d = nc.dram_tensor("b1r", [128, HID], F32, kind="ExternalInput")
        b2r_d = nc.dram_tensor("b2r", [128, HID], F32, kind="ExternalInput")
    gi1_d = nc.dram_tensor("gi1", [128, ecols1], I16, kind="ExternalInput")
    gi2_d = nc.dram_tensor("gi2", [128, ecols2], I16, kind="ExternalInput")
    dl1_d = nc.dram_tensor("dl1", [128, nu1p], BF16, kind="ExternalInput")
    dl2_d = nc.dram_tensor("dl2", [128, nu2p], BF16, kind="ExternalInput")
    gid_d = nc.dram_tensor("gid", [128, NB], F32, kind="ExternalInput")
    dinv1_d = nc.dram_tensor("dinv1", [128, NB], F32, kind="ExternalInput")
    dinv2_d = nc.dram_tensor("dinv2", [128, NB], F32, kind="ExternalInput")
    icnt_d = nc.dram_tensor("icnt", [128, 2], F32, kind="ExternalInput")

    hpart = nc.dram_tensor("hpart", [NPC, HID], BF16)
    u2t_k = [nc.dram_tensor(f"u2t{k}", [NC * HID, QLB[k] * DW], FP8)
             for k in range(4)]
    u2o_k = [nc.dram_tensor(f"u2o{k}", [HID, QLB[k] * DW], FP8)
             for k in range(4)]
    pool_loc = nc.dram_tensor("pool_loc", [G, HID], BF16)
    pool_glob = nc.dram_tensor("pool_glob", [G, HID], BF16,
                               addr_space="Shared")
    y_d = nc.dram_tensor("y", [G, HID], F32, kind="ExternalOutput")
    if stage == 2:
        dbg_h = nc.dram_tensor("dbg_h", [NPC, HID], BF16,
                               kind="ExternalOutput")
    if stage == 4:
        dbg_v = [nc.dram_tensor(f"dbg_v{k}", [HID, QLB[k] * DW], FP8,
                                kind="ExternalOutput") for k in range(4)]
    if stage == 3:
        dbg_u = [nc.dram_tensor(f"dbg_u{k}", [NC * HID, QLB[k] * DW], FP8,
                                kind="ExternalOutput") for k in range(4)]

    eye_d = nc.inline_tensor(np.eye(128, dtype=np.float32), name="eye128")
    eyeb_d = nc.inline_tensor(np.eye(128, dtype=ml_dtypes.bfloat16),
                              name="eye128b")
    iotaU_np = np.tile(np.arange(DW, dtype=np.float32),
                       (128, UB)).astype(ml_dtypes.bfloat16)
    iotaU_d = nc.inline_tensor(iotaU_np, name="iotaU")
    iotaF_d = nc.inline_tensor(
        np.tile(np.arange(DW, dtype=np.float32),
                (128, 1)).astype(ml_dtypes.bfloat16), name="iotaF")
    iota256_d = nc.inline_tensor(
        np.tile(np.arange(256, dtype=np.float32), (128, 1)), name="iota256")


    xb4 = xb4_d.ap()
    hp_rows = hpart.ap()
    CORES = [list(range(NC))]

    # persistent SBUF
    gi1_s = nc.alloc_sbuf_tensor("gi1_s", [128, ecols1], I16)
    gi2_s = nc.alloc_sbuf_tensor("gi2_s", [128, ecols2], I16)
    dl1_s = nc.alloc_sbuf_tensor("dl1_s", [128, nu1p], BF16)
    dl2_s = nc.alloc_sbuf_tensor("dl2_s", [128, nu2p], BF16)
    xst_s = nc.alloc_sbuf_tensor("xst_s", [IN, NPC], BF16)
    hpsbT = nc.alloc_sbuf_tensor("hpsbT", [128, NB * 128], BF16)

    with TileContext(nc) as tc:
        with tc.tile_pool(name="init", bufs=1) as ipool:
            nc.sync.dma_start(out=gi1_s[:], in_=gi1_d[:])
            nc.sync.dma_start(out=gi2_s[:], in_=gi2_d[:])
            nc.sync.dma_start(out=dl1_s[:], in_=dl1_d[:])
            nc.sync.dma_start(out=dl2_s[:], in_=dl2_d[:])
            nc.sync.dma_start(out=xst_s[:], in_=xst_d[:])
            eye_t = ipool.tile([128, 128], F32)
            nc.sync.dma_start(out=eye_t[:], in_=eye_d[:])
            eyeb_t = ipool.tile([128, 128], BF16)
            nc.sync.dma_start(out=eyeb_t[:], in_=eyeb_d[:])
            iotaU_t = ipool.tile([128, UB * DW], BF16)
            nc.sync.dma_start(out=iotaU_t[:], in_=iotaU_d[:])
            iotaF_t = ipool.tile([128, DW], BF16)
            nc.sync.dma_start(out=iotaF_t[:], in_=iotaF_d[:])
            iota256_t = ipool.tile([128, 256], F32)
            nc.sync.dma_start(out=iota256_t[:], in_=iota256_d[:])

            w1_t = ipool.tile([IN, HID], BF16)
            nc.sync.dma_start(out=w1_t[:], in_=w1_d[:])
            w2_t = ipool.tile([HID, HID], BF16)
            nc.sync.dma_start(out=w2_t[:], in_=w2_d[:])
            gid_t = ipool.tile([128, NB], F32)
            nc.sync.dma_start(out=gid_t[:], in_=gid_d[:])

            dinv1_t = ipool.tile([128, NB], F32)
            nc.sync.dma_start(out=dinv1_t[:], in_=dinv1_d[:])
            dinv2_t = ipool.tile([128, NB], F32)
            nc.sync.dma_start(out=dinv2_t[:], in_=dinv2_d[:])
            ones_t = ipool.tile([128, 1], BF16)
            nc.vector.memset(ones_t[:], 1.0)
            if not bias_zero:
                b1r_t = ipool.tile([128, HID], F32)
                nc.sync.dma_start(out=b1r_t[:], in_=b1r_d[:])
                b2r_t = ipool.tile([128, HID], F32)
                nc.sync.dma_start(out=b2r_t[:], in_=b2r_d[:])

            hp_r = hpart.ap().rearrange("(a p) f -> p a f", p=128)
            u2t_r = [t.ap().rearrange("(r p) n -> r p n", p=HID)
                     for t in u2t_k]

            # ======================= conv1 =======================
            with (
                tc.tile_pool(name="g1", bufs=4) as gpool,
                tc.tile_pool(name="s1", bufs=6) as spool,
                tc.tile_pool(name="r1", bufs=3) as rpool,
                tc.tile_pool(name="pa1", bufs=3, space="PSUM") as papool,
                tc.tile_pool(name="ph1", bufs=2, space="PSUM") as phpool,
                tc.tile_pool(name="pt1", bufs=2, space="PSUM") as ptpool,
            ):
                mt = {}

                def gather1(w0):
                    wh = min(w0 + W1CH, nch1)
                    t = gpool.tile([128, W1CH, 256], BF16, tag="m1")
                    nc.gpsimd.dma_gather(
                        t[:, :wh - w0, :], xb4, gi1_s[:, w0 * 8:wh * 8],
                        (wh - w0) * 128, (wh - w0) * 128, 256,
                        queue_num=0, single_packet=False)
                    mt.clear()
                    mt[w0] = t

                stile = {}

                def sbuild(u0, nu, dl_s, act_every, sb_i):
                    uh = min(u0 + UB, nu)
                    t = spool.tile([128, UB, DW], BF16, tag="s")
                    if sb_i % act_every == act_every - 1:
                        for j in range(uh - u0):
                            tq = spool.tile([128, DW], BF16, tag="tq")
                            nc.scalar.activation(
                                tq[:], iotaF_t[:],
                                mybir.ActivationFunctionType.Square,
                                bias=dl_s[:, u0 + j:u0 + j + 1], scale=-1.0)
                            nc.scalar.activation(
                                t[:, j, :], tq[:],
                                mybir.ActivationFunctionType.Relu,
                                bias=ones_t[:], scale=-1.0)
                    else:
                        nc.vector.tensor_tensor(
                            t[:, :uh - u0, :],
                            dl_s[:, u0:uh].rearrange("p (u x) -> p u x", x=1)
                            .broadcast_to([128, uh - u0, DW]),
                            iotaU_t[:, :(uh - u0) * DW]
                            .rearrange("p (u x) -> p u x", x=DW),
                            mybir.AluOpType.is_equal)
                    stile.clear()
                    stile[u0] = t

                # conv1: retire group = G1 128-blocks = 2*G1 64-blocks
                B64G = 2 * G1

                def retire1(g, aggP):
                    b0 = g * G1                      # first 128-block
                    nb_ = min(G1, NB - b0)
                    zs = rpool.tile([64, G1 * 128], BF16, tag="zsb")
                    if aggP is not None:
                        nc.scalar.activation(
                            zs[:, :nb_ * 128], aggP[:, :nb_ * 128],
                            mybir.ActivationFunctionType.Copy)
                    for j64 in range(nb_ * 2):
                        if not has1[g * B64G + j64]:
                            nc.vector.memset(
                                zs[:, j64 * DW:(j64 + 1) * DW], 0.0)
                    hps = phpool.tile([128, G1, 128], F32, tag="hps")
                    hg = rpool.tile([128, G1, 128], BF16, tag="hg")
                    for j in range(nb_):
                        blk = b0 + j
                        nc.tensor.matmul(hps[:, j, :],
                                         zs[:, j * 128:(j + 1) * 128],
                                         w1_t[:], start=True, stop=False)
                        nc.tensor.matmul(
                            hps[:, j, :],
                            xst_s[:, blk * 128:(blk + 1) * 128],
                            w1_t[:], start=False, stop=True)
                        if bias_zero:
                            nc.scalar.activation(
                                hg[:, j, :], hps[:, j, :],
                                mybir.ActivationFunctionType.Relu,
                                scale=dinv2_t[:, blk:blk + 1])
                        else:
                            hb = rpool.tile([128, HID], F32, tag="hb")
                            nc.vector.tensor_scalar(
                                hb[:], hps[:, j, :],
                                dinv1_t[:, blk:blk + 1], None,
                                mybir.AluOpType.mult)
                            nc.vector.tensor_add(hb[:], hb[:], b1r_t[:])
                            hr = rpool.tile([128, HID], F32, tag="hr")
                            nc.scalar.activation(
                                hr[:], hb[:],
                                mybir.ActivationFunctionType.Relu)
                            nc.vector.tensor_scalar(
                                hg[:, j, :], hr[:],
                                dinv1_t[:, blk:blk + 1], None,
                                mybir.AluOpType.mult)
                        tp = ptpool.tile([128, 128], BF16, tag="tp")
                        nc.tensor.transpose(tp[:], hg[:, j, :], eyeb_t[:])
                        nc.scalar.activation(
                            hpsbT[:, blk * 128:(blk + 1) * 128], tp[:],
                            mybir.ActivationFunctionType.Copy)
                    nc.sync.dma_start(out=hp_r[:, b0:b0 + nb_, :],
                                        in_=hg[:, :nb_, :])

                aggP = None
                cur_grp = -1
                retired = set()
                sb_i = 0
                for u, (cc, blk, lo, hi) in enumerate(units1):
                    w0 = cc // W1CH * W1CH
                    if w0 not in mt:
                        gather1(w0)
                    u0 = u // UB * UB
                    if u0 not in stile:
                        sbuild(u0, len(units1), dl1_s, ACT1, sb_i)
                        sb_i += 1
                    g = blk // B64G
                    if g != cur_grp:
                        if cur_grp >= 0:
                            retire1(cur_grp, aggP)
                            retired.add(cur_grp)
                        cur_grp = g
                        aggP = papool.tile([64, B64G * DW], F32, tag="agg")
                    j = blk - g * B64G
                    q = uslot1[u]
                    nc.tensor.matmul(
                        aggP[:, j * DW:(j + 1) * DW],
                        mt[w0][:, cc - w0, 64 * q:64 * q + 64],
                        stile[u0][:, u - u0, :],
                        start=ustart1[u], stop=ustop1[u])
                if cur_grp >= 0:
                    retire1(cur_grp, aggP)
                    retired.add(cur_grp)
                for g in range((NB + G1 - 1) // G1):
                    if g not in retired:
                        retire1(g, None)

            if stage == 2:
                nc.sync.dma_start(out=dbg_h[:], in_=hpart[:])

            # ============== conv2 + chunked RS + pipelined mm2 ==========
            if stage >= 3:
                pools2 = [
                    tc.tile_pool(name="g2", bufs=4),
                    tc.tile_pool(name="s2", bufs=6),
                    tc.tile_pool(name="r2", bufs=4),
                    tc.tile_pool(name="pa2", bufs=3, space="PSUM"),
                    tc.tile_pool(name="mm2", bufs=3),
                    tc.tile_pool(name="ps2", bufs=1, space="PSUM"),
                    tc.tile_pool(name="pacc", bufs=1, space="PSUM"),
                ]
                gpool, spool, rpool, papool, mpool, ppool, apool = [
                    p.__enter__() for p in pools2]
                mt = {}
                stile = {}

                def gather2(w0):
                    wh = min(w0 + W2CH, nch2)
                    t = gpool.tile([128, W2CH, HID], BF16, tag="m2")
                    nc.gpsimd.dma_gather(
                        t[:, :wh - w0, :], hp_rows,
                        gi2_s[:, w0 * 8:wh * 8],
                        (wh - w0) * 128, (wh - w0) * 128, HID,
                        queue_num=0, single_packet=False)
                    mt.clear()
                    mt[w0] = t

                def sbuild2(u0, sb_i):
                    uh = min(u0 + UB, len(units2))
                    t = spool.tile([128, UB, DW], BF16, tag="s")
                    if sb_i % ACT2 == ACT2 - 1:
                        for j in range(uh - u0):
                            tq = spool.tile([128, DW], BF16, tag="tq")
                            nc.scalar.activation(
                                tq[:], iotaF_t[:],
                                mybir.ActivationFunctionType.Square,
                                bias=dl2_s[:, u0 + j:u0 + j + 1],
                                scale=-1.0)
                            nc.scalar.activation(
                                t[:, j, :], tq[:],
                                mybir.ActivationFunctionType.Relu,
                                bias=ones_t[:], scale=-1.0)
                    else:
                        nc.vector.tensor_tensor(
                            t[:, :uh - u0, :],
                            dl2_s[:, u0:uh].rearrange("p (u x) -> p u x", x=1)
                            .broadcast_to([128, uh - u0, DW]),
                            iotaU_t[:, :(uh - u0) * DW]
                            .rearrange("p (u x) -> p u x", x=DW),
                            mybir.AluOpType.is_equal)
                    stile.clear()
                    stile[u0] = t

                ret_i = [0]
                qdmas = [[], [], [], []]     # retire DMA insts per quarter
                stage_state = {}             # (k, r) -> [tile, filled_set]

                flushed_qr = set()

                def flush_qr(k, r):
                    tile, filled = stage_state.pop((k, r))
                    ngrp = QLB[k] // G2
                    for gl in range(ngrp):
                        if gl not in filled:
                            nc.vector.memset(
                                tile[:, gl * G2 * DW:(gl + 1) * G2 * DW],
                                0.0)
                    dma = nc.gpsimd.dma_start(out=u2t_r[k][r, :, :],
                                              in_=tile[:, :QLB[k] * DW])
                    qdmas[k].append(dma)
                    flushed_qr.add((k, r))

                def retire2(g, aggP):
                    # g: group index over ordered blocks (G2 64-blocks)
                    ob0 = g * G2
                    b_glob = _B2ORDER[ob0]
                    r = b_glob // NB64
                    lb = b_glob % NB64
                    k = _q_of_lb(lb)
                    gl = (lb - _Q0[k]) // G2         # group within (k, r)
                    ngrp = QLB[k] // G2
                    if (k, r) not in stage_state:
                        tag = "u2sA" if QLB[k] == 56 else "u2sB"
                        st_t = rpool.tile([128, QLB[k] * DW], FP8, tag=tag,
                                          name=tag)
                        stage_state[(k, r)] = [st_t, set()]
                    tile, filled = stage_state[(k, r)]
                    sl = tile[:, gl * G2 * DW:(gl + 1) * G2 * DW]
                    eng = [nc.scalar, nc.scalar, nc.scalar, nc.scalar,
                           nc.vector][ret_i[0] % 5]
                    ret_i[0] += 1
                    if eng is nc.scalar:
                        nc.scalar.activation(
                            sl, aggP[:], mybir.ActivationFunctionType.Copy)
                    else:
                        nc.vector.tensor_copy(sl, aggP[:])
                    for j in range(G2):
                        if not has2[ob0 + j]:
                            nc.vector.memset(
                                tile[:, (gl * G2 + j) * DW:
                                     (gl * G2 + j + 1) * DW], 0.0)
                    filled.add(gl)
                    if len(filled) == ngrp:
                        flush_qr(k, r)

                # ---- mm2 chunk consumer (pooled psum held across chunks)
                pooled = [apool.tile([128, HID], F32, tag=f"pool{h}",
                                     name=f"pooled{h}")
                          for h in range(2)]
                NB128Q = [q // 2 for q in QLB]       # 128-blocks per chunk
                LB128Q = [q // 2 for q in _Q0]       # first 128-block

                def mm2_chunk(k):
                    u2o = u2o_k[k].ap()
                    first = (k == 0)
                    last = (k == 3)
                    nblk = NB128Q[k]
                    for gg in range(nblk // G3):
                        b0 = LB128Q[k] + gg * G3     # absolute 128-block
                        c0 = gg * G3 * 128
                        ga = mpool.tile([128, G3 * 128], FP8, tag="ga")
                        gd = nc.sync.dma_start(out=ga[:],
                                               in_=u2o[:, c0:c0 + G3 * 128])
                        _add_dep_helper(gd.ins, rs_cc[k].ins, True,
                                        f"mm2 chunk {k} reads RS{k}")
                        sel7 = mpool.tile([128, G3, 256], BF16, tag="sel7")
                        nc.vector.tensor_tensor(
                            sel7[:],
                            gid_t[:, b0:b0 + G3]
                            .rearrange("p (u x) -> p u x", x=1)
                            .broadcast_to([128, G3, 256]),
                            iota256_t[:]
                            .rearrange("p (u x) -> p u x", u=1)
                            .broadcast_to([128, G3, 256]),
                            mybir.AluOpType.is_equal)
                        z = mpool.tile([128, G3 * 128], BF16, tag="z2")
                        nc.vector.tensor_add(
                            z[:], ga[:],
                            hpsbT[:, b0 * 128:(b0 + G3) * 128])
                        h2p = ppool.tile([128, G3, HID], F32, tag="h2p")
                        for j in range(G3):
                            blk = b0 + j
                            nc.tensor.matmul(
                                h2p[:, j, :], z[:, j * 128:(j + 1) * 128],
                                w2_t[:], start=True, stop=True)
                            h2s = mpool.tile([128, HID], BF16, tag="h2s")
                            if bias_zero:
                                nc.scalar.activation(
                                    h2s[:], h2p[:, j, :],
                                    mybir.ActivationFunctionType.Relu,
                                    scale=dinv1_t[:, blk:blk + 1])
                            else:
                                hb2 = mpool.tile([128, HID], F32, tag="hb2")
                                nc.vector.tensor_scalar(
                                    hb2[:], h2p[:, j, :],
                                    dinv1_t[:, blk:blk + 1], None,
                                    mybir.AluOpType.mult)
                                nc.vector.tensor_add(hb2[:], hb2[:],
                                                     b2r_t[:])
                                nc.scalar.activation(
                                    h2s[:], hb2[:],
                                    mybir.ActivationFunctionType.Relu)
                            st = first and gg == 0 and j == 0
                            sp = last and gg == nblk // G3 - 1 and j == G3 - 1
                            for hh in range(2):
                                nc.tensor.matmul(
                                    pooled[hh][:],
                                    sel7[:, j, hh * 128:(hh + 1) * 128],
                                    h2s[:], start=st, stop=sp)

                # ---- conv2 main loop with interleaved RS / mm2
                q_last_grp = []                      # last retire group per q
                acc = 0
                for k in range(4):
                    q_last_grp.append((acc + QLB[k]) // G2 - 1)
                    acc += QLB[k]

                rs_emitted = []
                rs_cc = {}

                def emit_rs(k):
                    cc = nc.gpsimd.collective_compute(
                        "ReduceScatter", mybir.AluOpType.add, CORES,
                        [u2t_k[k][:]], [u2o_k[k][:]])
                    for d in qdmas[k]:
                        _add_dep_helper(cc.ins, d.ins, True,
                                        f"RS{k} waits quarter writes")
                    if rs_emitted:
                        _add_dep_helper(cc.ins, rs_cc[rs_emitted[-1]].ins,
                                        True, "collective order")
                    rs_cc[k] = cc
                    rs_emitted.append(k)

                aggP = None
                cur_grp = -1
                retired2 = set()
                sb_i = 0

                rs_ready = []

                def retire_and_track(g, aggP):
                    retire2(g, aggP)
                    retired2.add(g)
                    for k in range(4):
                        if g == q_last_grp[k]:
                            lo_g = q_last_grp[k - 1] + 1 if k else 0
                            if all(gg in retired2
                                   for gg in range(lo_g, g + 1)):
                                rs_ready.append(k)

                def maybe_emit_pending(blk):
                    # fire a ready RS once the loop is half-way through the
                    # NEXT quarter (gather descs pre-generated = DMA runway)
                    if not rs_ready:
                        return
                    k = rs_ready[0]
                    b_glob = _B2ORDER[blk // G2 * G2]
                    kq = _q_of_lb(b_glob % NB64)
                    r = b_glob // NB64
                    if kq > k + 1 or (kq == k + 1 and r >= 4):
                        rs_ready.pop(0)
                        emit_rs(k)
                        if stage >= 5 and k >= 1:
                            mm2_chunk(k - 1)

                for u, (cc, blk, lo, hi) in enumerate(units2):
                    w0 = cc // W2CH * W2CH
                    if w0 not in mt:
                        gather2(w0)
                    u0 = u // UB * UB
                    if u0 not in stile:
                        sbuild2(u0, sb_i)
                        sb_i += 1
                    g = blk // G2
                    maybe_emit_pending(blk)
                    if g != cur_grp:
                        if cur_grp >= 0:
                            retire_and_track(cur_grp, aggP)
                        cur_grp = g
                        aggP = papool.tile([128, G2 * DW], F32, tag="agg2")
                    j = blk - g * G2
                    nc.tensor.matmul(
                        aggP[:, j * DW:(j + 1) * DW], mt[w0][:, cc - w0, :],
                        stile[u0][:, u - u0, :],
                        start=ustart2[u], stop=ustop2[u])
                if cur_grp >= 0:
                    retire_and_track(cur_grp, aggP)
                # flush any incomplete / absent (quarter, range) staging
                for k in list(rs_ready):
                    pass
                for k in range(4):
                    for r in range(NC):
                        if (k, r) in stage_state:
                            flush_qr(k, r)
                        elif (k, r) not in flushed_qr:
                            tag = "u2sA" if QLB[k] == 56 else "u2sB"
                            st_t = rpool.tile([128, QLB[k] * DW], FP8,
                                              tag=tag, name=tag)
                            stage_state[(k, r)] = [st_t, set()]
                            flush_qr(k, r)
                    if k not in rs_emitted:
                        emit_rs(k)
                        if stage >= 5 and k >= 1:
                            mm2_chunk(k - 1)

                if stage == 4:
                    for k in range(4):
                        dd = nc.sync.dma_start(out=dbg_v[k][:],
                                               in_=u2o_k[k][:])
                        _add_dep_helper(dd.ins, rs_cc[k].ins, True, "dbg")
                if stage == 3:
                    for k in range(4):
                        dd = nc.sync.dma_start(out=dbg_u[k][:],
                                               in_=u2t_k[k][:])
                        for d in qdmas[k]:
                            _add_dep_helper(dd.ins, d.ins, True, "dbgu")

                pl_dma = None
                if stage >= 5:
                    mm2_chunk(3)
                    pl_r = pool_loc.ap().rearrange("(h p) f -> p h f", p=128)
                    pl_s = mpool.tile([128, 2, HID], BF16, tag="pls")
                    nc.vector.tensor_copy(pl_s[:, 0, :], pooled[0][:])
                    nc.vector.tensor_copy(pl_s[:, 1, :], pooled[1][:])
                    pl_dma = nc.sync.dma_start(out=pl_r[:], in_=pl_s[:])

                for p in reversed(pools2):
                    p.__exit__(None, None, None)

            if stage >= 5:
                ar_cc = nc.gpsimd.collective_compute(
                    "AllReduce", mybir.AluOpType.add, CORES,
                    [pool_loc[:]], [pool_glob[:]],
                )
                _add_dep_helper(ar_cc.ins, pl_dma.ins, True,
                                "AR waits pooled write")
                _add_dep_helper(ar_cc.ins, rs_cc[3].ins, True,
                                "collective order")

                # ---------------- head: mean-div, fc, LayerNorm (tiny)
                pg_r = pool_glob.ap().rearrange("(h p) f -> p h f", p=128)
                y_r = y_d.ap().rearrange("(h p) f -> p h f", p=128)
                with (
                    tc.tile_pool(name="head", bufs=1) as hpool,
                    tc.tile_pool(name="psh", bufs=2, space="PSUM") as hps,
                ):
                    wfc_t = hpool.tile([HID, HID], F32)
                    nc.sync.dma_start(out=wfc_t[:], in_=wfc_d[:])
                    bfcr_t = hpool.tile([128, HID], F32)
                    nc.sync.dma_start(out=bfcr_t[:], in_=bfcr_d[:])
                    gamr_t = hpool.tile([128, HID], F32)
                    nc.sync.dma_start(out=gamr_t[:], in_=gamr_d[:])
                    betr_t = hpool.tile([128, HID], F32)
                    nc.sync.dma_start(out=betr_t[:], in_=betr_d[:])
                    icnt_t = hpool.tile([128, 2], F32)
                    nc.sync.dma_start(out=icnt_t[:], in_=icnt_d[:])
                    eps_t = hpool.tile([128, 1], F32)
                    nc.vector.memset(eps_t[:], LN_EPS)
                    yo = hpool.tile([128, 2, HID], F32)
                    for hh in range(2):
                        pgb = hpool.tile([128, HID], BF16, tag="pgb")
                        pgd = nc.sync.dma_start(out=pgb[:],
                                                in_=pg_r[:, hh, :])
                        _add_dep_helper(pgd.ins, ar_cc.ins, True,
                                        "head reads AllReduce output")
                        pg_s = hpool.tile([128, HID], F32, tag="pg")
                        nc.vector.tensor_scalar(
                            pg_s[:], pgb[:], icnt_t[:, hh:hh + 1], None,
                            mybir.AluOpType.mult)
                        pgT_p = hps.tile([HID, 128], F32, tag="pgT")
                        nc.tensor.transpose(pgT_p[:], pg_s[:], eye_t[:])
                        pgT_s = hpool.tile([HID, 128], F32, tag="pgTs")
                        nc.vector.tensor_copy(pgT_s[:], pgT_p[:])
                        y_p = hps.tile([128, HID], F32, tag="yp")
                        nc.tensor.matmul(y_p[:], pgT_s[:], wfc_t[:])
                        y_s = hpool.tile([128, HID], F32, tag="ys")
                        nc.vector.tensor_add(y_s[:], y_p[:], bfcr_t[:])
                        mu = hpool.tile([128, 1], F32, tag="mu")
                        nc.vector.tensor_reduce(mu[:], y_s[:],
                                                mybir.AxisListType.XYZW,
                                                mybir.AluOpType.add)
                        nc.vector.tensor_scalar(mu[:], mu[:], -1.0 / HID,
                                                None, mybir.AluOpType.mult)
                        cen = hpool.tile([128, HID], F32, tag="cen")
                        nc.vector.tensor_scalar(cen[:], y_s[:], mu[:], None,
                                                mybir.AluOpType.add)
                        sq = hpool.tile([128, HID], F32, tag="sq")
                        nc.vector.tensor_mul(sq[:], cen[:], cen[:])
                        var = hpool.tile([128, 1], F32, tag="var")
                        nc.vector.tensor_reduce(var[:], sq[:],
                                                mybir.AxisListType.XYZW,
                                                mybir.AluOpType.add)
                        std = hpool.tile([128, 1], F32, tag="std")
                        nc.scalar.activation(
                            std[:], var[:],
                            mybir.ActivationFunctionType.Sqrt,
                            bias=eps_t[:], scale=1.0 / HID)
                        rstd = hpool.tile([128, 1], F32, tag="rstd")
                        nc.vector.reciprocal(rstd[:], std[:])
                        nc.vector.tensor_scalar(cen[:], cen[:], rstd[:],
                                                None, mybir.AluOpType.mult)
                        nc.vector.tensor_mul(cen[:], cen[:], gamr_t[:])
                        nc.vector.tensor_add(yo[:, hh, :], cen[:], betr_t[:])
                    nc.sync.dma_start(out=y_r[:], in_=yo[:])

    nc.compile()
    return nc


_CACHE = {}


def make_in_maps(x, edge_index, batch, W1, b1, W2, b2, Wfc, bfc, gamma, beta,
                 per_core=None, inv_cnt_w=None, dinv=None, meta=None):
    if per_core is None:
        per_core, inv_cnt_w, dinv, meta = _host_prep(
            np.asarray(edge_index), np.asarray(batch))
    x = np.asarray(x, np.float32)
    xp = x * dinv[:, None]
    xb4 = xp.astype(ml_dtypes.bfloat16).reshape(N // 4, 256)
    xself = (xp * dinv[:, None]).astype(np.float32)
    rep = lambda v: np.tile(np.asarray(v, np.float32)[None, :], (128, 1))
    bias_zero = (not np.any(np.asarray(b1))) and (not np.any(np.asarray(b2)))
    shared = {
        "xb4": xb4,
        "w1": np.asarray(W1, np.float32).astype(ml_dtypes.bfloat16),
        "w2": np.asarray(W2, np.float32).astype(ml_dtypes.bfloat16),
        "wfc": np.asarray(Wfc, np.float32),
        "bfcr": rep(bfc),
        "gamr": rep(gamma), "betr": rep(beta),
        "icnt": inv_cnt_w,
    }
    if not bias_zero:
        shared["b1r"] = rep(b1)
        shared["b2r"] = rep(b2)
    in_maps = []
    for c in range(NC):
        m = dict(shared)
        xs = np.zeros((IN, NPC), np.float32)
        xs[:, :RPC] = xself[c * RPC:(c + 1) * RPC].T
        m["xst"] = xs.astype(ml_dtypes.bfloat16)
        for k in ("gi1", "gi2", "dl1", "dl2", "gid",
                  "dinv1", "dinv2"):
            m[k] = per_core[c][k]
        in_maps.append(m)
    return in_maps, bias_zero, meta


def kernel(x, edge_index, batch, W1, b1, W2, b2, Wfc, bfc, gamma, beta,
           _stage=5, _full_results=False):
    per_core, inv_cnt_w, dinv, meta = _host_prep(np.asarray(edge_index),
                                                 np.asarray(batch))
    in_maps, bias_zero, meta = make_in_maps(
        x, edge_index, batch, W1, b1, W2, b2, Wfc, bfc, gamma, beta,
        per_core, inv_cnt_w, dinv, meta)
    key = (meta["epad1"], meta["epad2"], meta["nu1p"], meta["nu2p"],
           bias_zero, _stage)
    if key not in _CACHE:
        _CACHE[key] = _build(meta, bias_zero, _stage)
    nc = _CACHE[key]

    res = run_bass_kernel_spmd(nc, in_maps, list(range(NC)))
    if _full_results:
        return res.results
    return res.results[0]["y"]


# revision 76
# speedup vs baseline: 1.0175x; 1.0003x over previous
"""GCN (2x GCNConv + mean-pool + fc + LayerNorm) on 8 Trainium2 NeuronCores.

One-hot matmul aggregation replaces DMA scatter-adds entirely:

conv1 (pull): per-core in-edges on a cross-core-common segment grid
(64-node dst block x src%4 slot, segment length = max over cores). Each
128-edge chunk is gathered with one 512B descriptor per edge from a
4-row-packed bf16 x' table (the edge's row is slot src%4). For each
(chunk x segment) overlap ("unit"), PE accumulates
zT[64f, 64d] += M[128e, 64f].T @ S[128e, 64d], where S is a one-hot built
from a per-unit dstloc column: batched is_equal-vs-iota on DVE, with a
share built on the Activation engine as relu(1 - (iota - dstloc)^2).
The implicit self-loop is a second matmul against a transposed prescaled
x table, so it never enters the edge stream, and mm1 needs no transpose
(zT is the lhsT directly).

conv2 (push): per-core out-edges on a common global 64-node dst-block
grid, blocks ordered (quarter, range, local) so each node-quarter of the
TRANSPOSED partials u2T completes early; gathers hit the local hpart rows
(h1' = relu(h1)*dinv, prescaled). FOUR chunked ReduceScatters fire as
their quarter's writes land, overlapping the next quarter's compute, and
mm2 consumes each reduced chunk one quarter later (no transposes: z2T is
the lhsT; the self term adds from an SBUF-resident transposed hpsbT).
Pooling is the PSUM-accumulated one-hot graph matmul; pooled sums are
AllReduced; the tiny fc+LayerNorm head is computed redundantly.
"""
import sys

if '/opt/trn_rl_repo' not in sys.path:
    sys.path.insert(0, '/opt/trn_rl_repo')

import ml_dtypes
import numpy as np

import concourse.bacc as bacc
import concourse.mybir as mybir
from concourse.bass import _add_dep_helper
from concourse.tile import TileContext
from concourse.bass_utils import run_bass_kernel_spmd

# ---------------------------------------------------------------- constants
N = 100000
E = 800000
IN = 64
HID = 128
G = 256
NC = 8
RPC = N // NC              # 12500 nodes per core
NB = 98                    # 128-node dst blocks per core (12544 padded)
NPC = NB * 128             # 12544
DW = 64                    # aggregation dst-block width
NB64 = NPC // DW           # 196 64-blocks per core
NBG64 = NC * NB64          # 1568 global 64-blocks
QLB = (56, 56, 56, 28)     # conv2 RS chunks (in 64-blocks per core)
LN_EPS = 1e-5
W1CH = 16                  # conv1 gather window (chunks per call)
W2CH = 32                  # conv2 gather window
UB = 32                    # S-build batch (units per instruction)
ACT1 = 10 ** 9             # conv1: every ACT1-th S-batch goes to Act engine
ACT2 = 10 ** 9             # conv2: every ACT2-th S-batch goes to Act engine
G1 = 4                     # conv1 retire group (128-blocks)
G2 = 7                     # conv2 retire group (64-blocks; divides 56)
G3 = 7                     # mm2 group (128-blocks)
F32 = mybir.dt.float32
FP8 = mybir.dt.float8e4
BF16 = mybir.dt.bfloat16
I16 = mybir.dt.int16


def _wrap16(a, cols):
    """[n] -> [128, cols] int16: element i -> [i%16, i//16], tiled x8."""
    out = np.zeros((16, cols), np.int16)
    w = a.reshape(-1, 16).T
    out[:, : w.shape[1]] = w
    return np.tile(out, (8, 1))


def _common_grid(counts):
    seg = counts.max(axis=0).astype(np.int64)
    off = np.concatenate([[0], np.cumsum(seg)])
    epad = int((off[-1] + 127) // 128 * 128)
    return seg, off, epad


def _make_units(seg, off, nblk, seg_per_blk):
    """(chunk, blk, lo, hi) overlaps of 128-edge chunks with segments.

    Segment k belongs to block k // seg_per_blk (blocks in segment order).
    Returns units, per-unit start/stop (first/last unit of its block), and
    per-block has_units.
    """
    units = []
    blk_first = {}
    blk_last = {}
    for k in range(len(seg)):
        if seg[k] == 0:
            continue
        blk = k // seg_per_blk
        a, b = int(off[k]), int(off[k] + seg[k])
        for cc in range(a // 128, (b - 1) // 128 + 1):
            lo, hi = max(a, cc * 128), min(b, (cc + 1) * 128)
            u = len(units)
            units.append((cc, blk, lo, hi))
            if blk not in blk_first:
                blk_first[blk] = u
            blk_last[blk] = u
    start = [False] * len(units)
    stop = [False] * len(units)
    for blk, u in blk_first.items():
        start[u] = True
    for blk, u in blk_last.items():
        stop[u] = True
    has = [blk in blk_first for blk in range(nblk)]
    return units, start, stop, has


def _q_of_lb(lb):
    """conv2 node-quarter of a local 64-block index."""
    acc = 0
    for q, n in enumerate(QLB):
        if lb < acc + n:
            return q
        acc += n
    raise ValueError(lb)


# conv2 ordered block list: (quarter, range, lb64) lexicographic
_B2ORDER = sorted(range(NBG64),
                  key=lambda b: (_q_of_lb(b % NB64), b // NB64, b % NB64))
_Q0 = [0, 56, 112, 168]            # quarter start lb64


def _host_prep(edge_index, batch):
    src = np.asarray(edge_index[0], np.int64)
    dst = np.asarray(edge_index[1], np.int64)
    deg = (np.bincount(dst, minlength=N) + 1.0).astype(np.float32)
    dinv = (1.0 / np.sqrt(deg)).astype(np.float32)

    # ---------------- conv1: (blk64, slot) grid
    e_core1 = []
    cnt1 = np.zeros((NC, NB64 * 4), np.int64)
    for c in range(NC):
        m = (dst >= c * RPC) & (dst < (c + 1) * RPC)
        s, d = src[m], dst[m] - c * RPC
        key = (d // DW) * 4 + (s % 4)
        cnt1[c] = np.bincount(key, minlength=NB64 * 4)
        order = np.lexsort((d, key))
        e_core1.append((s[order], d[order], key[order]))
    seg1, off1, epad1 = _common_grid(cnt1)
    units1, ustart1, ustop1, has1 = _make_units(seg1, off1, NB64, 4)
    nch1 = epad1 // 128
    nu1 = len(units1)
    nu1p = (nu1 + UB - 1) // UB * UB
    uslot1 = []
    for u, (cc, blk, lo, hi) in enumerate(units1):
        base = blk * 4
        q = 0
        for qq in range(4):
            a, b = off1[base + qq], off1[base + qq] + seg1[base + qq]
            if a <= lo < b:
                q = qq
                break
        uslot1.append(q)

    # ---------------- conv2: global 64-blocks in _B2ORDER
    e_core2 = []
    cnt2 = np.zeros((NC, NBG64), np.int64)   # indexed by ORDER position
    b2pos = np.empty(NBG64, np.int64)        # global blk64 -> order position
    for i, b in enumerate(_B2ORDER):
        b2pos[b] = i
    for c in range(NC):
        m = (src >= c * RPC) & (src < (c + 1) * RPC)
        s, d = src[m] - c * RPC, dst[m]
        rr = d // RPC                      # owner range of the dst
        dloc = d - rr * RPC                # dst local to its owner core
        key = b2pos[rr * NB64 + dloc // DW]
        cnt2[c] = np.bincount(key, minlength=NBG64)
        order = np.lexsort((dloc, key))
        e_core2.append((s[order], dloc[order], key[order]))
    seg2, off2, epad2 = _common_grid(cnt2)
    units2, ustart2, ustop2, has2 = _make_units(seg2, off2, NBG64, 1)
    nch2 = epad2 // 128
    nu2 = len(units2)
    nu2p = (nu2 + UB - 1) // UB * UB

    # ---------------- per-core streams
    per_core = []
    for c in range(NC):
        s, d, key = e_core1[c]
        pos = np.empty(len(s), np.int64)
        ptr = 0
        for k in np.unique(key):
            n = int(cnt1[c][k])
            pos[ptr:ptr + n] = off1[k] + np.arange(n)
            ptr += n
        gi1 = np.zeros(epad1, np.int16)
        gi1[pos] = (s // 4).astype(np.int16)
        sd1 = np.full(epad1, -1.0, np.float32)
        sd1[pos] = (d % DW).astype(np.float32)
        sl1 = np.full(epad1, -1, np.int8)
        sl1[pos] = (s % 4).astype(np.int8)
        dl1c = np.full((nu1p, 128), -1.0, np.float32)
        for u, (cc, blk, lo, hi) in enumerate(units1):
            p0 = lo - cc * 128
            q = uslot1[u]
            dl1c[u, p0:hi - cc * 128] = np.where(
                sl1[lo:hi] == q, sd1[lo:hi], -1.0)

        s2, d2, key2 = e_core2[c]
        pos2 = np.empty(len(s2), np.int64)
        ptr = 0
        for k in np.unique(key2):
            n = int(cnt2[c][k])
            pos2[ptr:ptr + n] = off2[k] + np.arange(n)
            ptr += n
        gi2 = np.zeros(epad2, np.int16)
        gi2[pos2] = s2.astype(np.int16)
        sd2 = np.full(epad2, -1.0, np.float32)
        sd2[pos2] = (d2 % DW).astype(np.float32)
        dl2c = np.full((nu2p, 128), -1.0, np.float32)
        for u, (cc, blk, lo, hi) in enumerate(units2):
            p0 = lo - cc * 128
            dl2c[u, p0:hi - cc * 128] = sd2[lo:hi]

        dv = np.zeros(NPC, np.float32)
        dv[:RPC] = dinv[c * RPC:(c + 1) * RPC]
        dvt = dv.reshape(NB, 128).T.copy()
        gid = np.asarray(batch, np.int64)
        gv = np.full(NPC, -1.0, np.float32)
        gv[:RPC] = gid[c * RPC:(c + 1) * RPC].astype(np.float32)

        per_core.append({
            "gi1": _wrap16(gi1, epad1 // 16),
            "gi2": _wrap16(gi2, epad2 // 16),
            "dl1": dl1c.T.astype(ml_dtypes.bfloat16).copy(),
            "dl2": dl2c.T.astype(ml_dtypes.bfloat16).copy(),
            "dinv1": dvt,
            "dinv2": (dvt * dvt).copy(),
            "gid": gv.reshape(NB, 128).T.copy(),
        })

    gidn = np.asarray(batch, np.int64)
    cntg = np.bincount(gidn, minlength=G).astype(np.float32)
    inv_cnt_w = (1.0 / np.maximum(cntg, 1.0)).reshape(2, 128).T.copy()

    meta = {"epad1": epad1, "nch1": nch1, "units1": units1,
            "ustart1": ustart1, "ustop1": ustop1, "has1": has1, "nu1p": nu1p,
            "uslot1": uslot1,
            "epad2": epad2, "nch2": nch2, "units2": units2,
            "ustart2": ustart2, "ustop2": ustop2, "has2": has2, "nu2p": nu2p}
    return per_core, inv_cnt_w, dinv, meta


def _build(meta, bias_zero=True, stage=5):
    nc = bacc.Bacc("TRN2", target_bir_lowering=False, debug=False,
                   num_devices=NC, num_swdge_queues=1,
                   dynamic_dma_scratch_size=32768)
    epad1, nch1 = meta["epad1"], meta["nch1"]
    units1, ustart1, ustop1 = meta["units1"], meta["ustart1"], meta["ustop1"]
    has1, nu1p, uslot1 = meta["has1"], meta["nu1p"], meta["uslot1"]
    epad2, nch2 = meta["epad2"], meta["nch2"]
    units2, ustart2, ustop2 = meta["units2"], meta["ustart2"], meta["ustop2"]
    has2, nu2p = meta["has2"], meta["nu2p"]
    ecols1, ecols2 = epad1 // 16, epad2 // 16

    # ------------------------------------------------ I/O declarations
    xb4_d = nc.dram_tensor("xb4", [N // 4, 256], BF16, kind="ExternalInput")
    xst_d = nc.dram_tensor("xst", [IN, NPC], BF16, kind="ExternalInput")
    w1_d = nc.dram_tensor("w1", [IN, HID], BF16, kind="ExternalInput")
    w2_d = nc.dram_tensor("w2", [HID, HID], BF16, kind="ExternalInput")
    wfc_d = nc.dram_tensor("wfc", [HID, HID], F32, kind="ExternalInput")
    bfcr_d = nc.dram_tensor("bfcr", [128, HID], F32, kind="ExternalInput")
    gamr_d = nc.dram_tensor("gamr", [128, HID], F32, kind="ExternalInput")
    betr_d = nc.dram_tensor("betr", [128, HID], F32, kind="ExternalInput")
    if not bias_zero:
        b1r_d = nc.dram_tensor("b1r", [128, HID], F32, kind="ExternalInput")
        b2r_d = nc.dram_tensor("b2r", [128, HID], F32, kind="ExternalInput")
    gi1_d = nc.dram_tensor("gi1", [128, ecols1], I16, kind="ExternalInput")
    gi2_d = nc.dram_tensor("gi2", [128, ecols2], I16, kind="ExternalInput")
    dl1_d = nc.dram_tensor("dl1", [128, nu1p], BF16, kind="ExternalInput")
    dl2_d = nc.dram_tensor("dl2", [128, nu2p], BF16, kind="ExternalInput")
    gid_d = nc.dram_tensor("gid", [128, NB], F32, kind="ExternalInput")
    dinv1_d = nc.dram_tensor("dinv1", [128, NB], F32, kind="ExternalInput")
    dinv2_d = nc.dram_tensor("dinv2", [128, NB], F32, kind="ExternalInput")
    icnt_d = nc.dram_tensor("icnt", [128, 2], F32, kind="ExternalInput")

    hpart = nc.dram_tensor("hpart", [NPC, HID], BF16)
    u2t_k = [nc.dram_tensor(f"u2t{k}", [NC * HID, QLB[k] * DW], FP8)
             for k in range(4)]
    u2o_k = [nc.dram_tensor(f"u2o{k}", [HID, QLB[k] * DW], FP8)
             for k in range(4)]
    pool_loc = nc.dram_tensor("pool_loc", [G, HID], BF16)
    pool_glob = nc.dram_tensor("pool_glob", [G, HID], BF16,
                               addr_space="Shared")
    y_d = nc.dram_tensor("y", [G, HID], F32, kind="ExternalOutput")
    if stage == 2:
        dbg_h = nc.dram_tensor("dbg_h", [NPC, HID], BF16,
                               kind="ExternalOutput")
    if stage == 4:
        dbg_v = [nc.dram_tensor(f"dbg_v{k}", [HID, QLB[k] * DW], FP8,
                                kind="ExternalOutput") for k in range(4)]
    if stage == 3:
        dbg_u = [nc.dram_tensor(f"dbg_u{k}", [NC * HID, QLB[k] * DW], FP8,
                                kind="ExternalOutput") for k in range(4)]

    eye_d = nc.inline_tensor(np.eye(128, dtype=np.float32), name="eye128")
    eyeb_d = nc.inline_tensor(np.eye(128, dtype=ml_dtypes.bfloat16),
                              name="eye128b")
    iotaU_np = np.tile(np.arange(DW, dtype=np.float32),
                       (128, UB)).astype(ml_dtypes.bfloat16)
    iotaU_d = nc.inline_tensor(iotaU_np, name="iotaU")
    iotaF_d = nc.inline_tensor(
        np.tile(np.arange(DW, dtype=np.float32),
                (128, 1)).astype(ml_dtypes.bfloat16), name="iotaF")
    iota256_d = nc.inline_tensor(
        np.tile(np.arange(256, dtype=np.float32), (128, 1)), name="iota256")


    xb4 = xb4_d.ap()
    hp_rows = hpart.ap()
    CORES = [list(range(NC))]

    # persistent SBUF
    gi1_s = nc.alloc_sbuf_tensor("gi1_s", [128, ecols1], I16)
    gi2_s = nc.alloc_sbuf_tensor("gi2_s", [128, ecols2], I16)
    dl1_s = nc.alloc_sbuf_tensor("dl1_s", [128, nu1p], BF16)
    dl2_s = nc.alloc_sbuf_tensor("dl2_s", [128, nu2p], BF16)
    xst_s = nc.alloc_sbuf_tensor("xst_s", [IN, NPC], BF16)
    hpsbT = nc.alloc_sbuf_tensor("hpsbT", [128, NB * 128], BF16)

    with TileContext(nc) as tc:
        with tc.tile_pool(name="init", bufs=1) as ipool:
            nc.sync.dma_start(out=gi1_s[:], in_=gi1_d[:])
            nc.sync.dma_start(out=gi2_s[:], in_=gi2_d[:])
            nc.sync.dma_start(out=dl1_s[:], in_=dl1_d[:])
            nc.sync.dma_start(out=dl2_s[:], in_=dl2_d[:])
            nc.sync.dma_start(out=xst_s[:], in_=xst_d[:])
            eye_t = ipool.tile([128, 128], F32)
            nc.sync.dma_start(out=eye_t[:], in_=eye_d[:])
            eyeb_t = ipool.tile([128, 128], BF16)
            nc.sync.dma_start(out=eyeb_t[:], in_=eyeb_d[:])
            iotaU_t = ipool.tile([128, UB * DW], BF16)
            nc.sync.dma_start(out=iotaU_t[:], in_=iotaU_d[:])
            iotaF_t = ipool.tile([128, DW], BF16)
            nc.sync.dma_start(out=iotaF_t[:], in_=iotaF_d[:])
            iota256_t = ipool.tile([128, 256], F32)
            nc.sync.dma_start(out=iota256_t[:], in_=iota256_d[:])

            w1_t = ipool.tile([IN, HID], BF16)
            nc.sync.dma_start(out=w1_t[:], in_=w1_d[:])
            w2_t = ipool.tile([HID, HID], BF16)
            nc.sync.dma_start(out=w2_t[:], in_=w2_d[:])
            gid_t = ipool.tile([128, NB], F32)
            nc.sync.dma_start(out=gid_t[:], in_=gid_d[:])

            dinv1_t = ipool.tile([128, NB], F32)
            nc.sync.dma_start(out=dinv1_t[:], in_=dinv1_d[:])
            dinv2_t = ipool.tile([128, NB], F32)
            nc.sync.dma_start(out=dinv2_t[:], in_=dinv2_d[:])
            ones_t = ipool.tile([128, 1], BF16)
            nc.vector.memset(ones_t[:], 1.0)
            if not bias_zero:
                b1r_t = ipool.tile([128, HID], F32)
                nc.sync.dma_start(out=b1r_t[:], in_=b1r_d[:])
                b2r_t = ipool.tile([128, HID], F32)
                nc.sync.dma_start(out=b2r_t[:], in_=b2r_d[:])

            hp_r = hpart.ap().rearrange("(a p) f -> p a f", p=128)
            u2t_r = [t.ap().rearrange("(r p) n -> r p n", p=HID)
                     for t in u2t_k]

            # ======================= conv1 =======================
            with (
                tc.tile_pool(name="g1", bufs=4) as gpool,
                tc.tile_pool(name="s1", bufs=6) as spool,
                tc.tile_pool(name="r1", bufs=3) as rpool,
                tc.tile_pool(name="pa1", bufs=3, space="PSUM") as papool,
                tc.tile_pool(name="ph1", bufs=2, space="PSUM") as phpool,
                tc.tile_pool(name="pt1", bufs=3, space="PSUM") as ptpool,
            ):
                mt = {}

                def gather1(w0):
                    wh = min(w0 + W1CH, nch1)
                    t = gpool.tile([128, W1CH, 256], BF16, tag="m1")
                    nc.gpsimd.dma_gather(
                        t[:, :wh - w0, :], xb4, gi1_s[:, w0 * 8:wh * 8],
                        (wh - w0) * 128, (wh - w0) * 128, 256,
                        queue_num=0, single_packet=False)
                    mt.clear()
                    mt[w0] = t

                stile = {}

                def sbuild(u0, nu, dl_s, act_every, sb_i):
                    uh = min(u0 + UB, nu)
                    t = spool.tile([128, UB, DW], BF16, tag="s")
                    if sb_i % act_every == act_every - 1:
                        for j in range(uh - u0):
                            tq = spool.tile([128, DW], BF16, tag="tq")
                            nc.scalar.activation(
                                tq[:], iotaF_t[:],
                                mybir.ActivationFunctionType.Square,
                                bias=dl_s[:, u0 + j:u0 + j + 1], scale=-1.0)
                            nc.scalar.activation(
                                t[:, j, :], tq[:],
                                mybir.ActivationFunctionType.Relu,
                                bias=ones_t[:], scale=-1.0)
                    else:
                        nc.vector.tensor_tensor(
                            t[:, :uh - u0, :],
                            dl_s[:, u0:uh].rearrange("p (u x) -> p u x", x=1)
                            .broadcast_to([128, uh - u0, DW]),
                            iotaU_t[:, :(uh - u0) * DW]
                            .rearrange("p (u x) -> p u x", x=DW),
                            mybir.AluOpType.is_equal)
                    stile.clear()
                    stile[u0] = t

                # conv1: retire group = G1 128-blocks = 2*G1 64-blocks
                B64G = 2 * G1

                r1_i = [0]

                def retire1(g, aggP):
                    b0 = g * G1                      # first 128-block
                    nb_ = min(G1, NB - b0)
                    zs = rpool.tile([64, G1 * 128], BF16, tag="zsb")
                    if aggP is not None:
                        r1_i[0] += 1
                        if r1_i[0] % 3 == 0:
                            nc.vector.tensor_copy(zs[:, :nb_ * 128],
                                                  aggP[:, :nb_ * 128])
                        else:
                            nc.scalar.activation(
                                zs[:, :nb_ * 128], aggP[:, :nb_ * 128],
                                mybir.ActivationFunctionType.Copy)
                    for j64 in range(nb_ * 2):
                        if not has1[g * B64G + j64]:
                            nc.vector.memset(
                                zs[:, j64 * DW:(j64 + 1) * DW], 0.0)
                    hps = phpool.tile([128, G1, 128], F32, tag="hps")
                    hg = rpool.tile([128, G1, 128], BF16, tag="hg")
                    for j in range(nb_):
                        blk = b0 + j
                        nc.tensor.matmul(hps[:, j, :],
                                         zs[:, j * 128:(j + 1) * 128],
                                         w1_t[:], start=True, stop=False)
                        nc.tensor.matmul(
                            hps[:, j, :],
                            xst_s[:, blk * 128:(blk + 1) * 128],
                            w1_t[:], start=False, stop=True)
                        if bias_zero:
                            nc.scalar.activation(
                                hg[:, j, :], hps[:, j, :],
                                mybir.ActivationFunctionType.Relu,
                                scale=dinv2_t[:, blk:blk + 1])
                        else:
                            hb = rpool.tile([128, HID], F32, tag="hb")
                            nc.vector.tensor_scalar(
                                hb[:], hps[:, j, :],
                                dinv1_t[:, blk:blk + 1], None,
                                mybir.AluOpType.mult)
                            nc.vector.tensor_add(hb[:], hb[:], b1r_t[:])
                            hr = rpool.tile([128, HID], F32, tag="hr")
                            nc.scalar.activation(
                                hr[:], hb[:],
                                mybir.ActivationFunctionType.Relu)
                            nc.vector.tensor_scalar(
                                hg[:, j, :], hr[:],
                                dinv1_t[:, blk:blk + 1], None,
                                mybir.AluOpType.mult)
                        tp = ptpool.tile([128, 128], BF16, tag="tp")
                        nc.tensor.transpose(tp[:], hg[:, j, :], eyeb_t[:])
                        if blk % 3 == 2:
                            nc.vector.tensor_copy(
                                hpsbT[:, blk * 128:(blk + 1) * 128], tp[:])
                        else:
                            nc.scalar.activation(
                                hpsbT[:, blk * 128:(blk + 1) * 128], tp[:],
                                mybir.ActivationFunctionType.Copy)
                    nc.sync.dma_start(out=hp_r[:, b0:b0 + nb_, :],
                                        in_=hg[:, :nb_, :])

                aggP = None
                cur_grp = -1
                retired = set()
                sb_i = 0
                for u, (cc, blk, lo, hi) in enumerate(units1):
                    w0 = cc // W1CH * W1CH
                    if w0 not in mt:
                        gather1(w0)
                    u0 = u // UB * UB
                    if u0 not in stile:
                        sbuild(u0, len(units1), dl1_s, ACT1, sb_i)
                        sb_i += 1
                    g = blk // B64G
                    if g != cur_grp:
                        if cur_grp >= 0:
                            retire1(cur_grp, aggP)
                            retired.add(cur_grp)
                        cur_grp = g
                        aggP = papool.tile([64, B64G * DW], F32, tag="agg")
                    j = blk - g * B64G
                    q = uslot1[u]
                    nc.tensor.matmul(
                        aggP[:, j * DW:(j + 1) * DW],
                        mt[w0][:, cc - w0, 64 * q:64 * q + 64],
                        stile[u0][:, u - u0, :],
                        start=ustart1[u], stop=ustop1[u])
                if cur_grp >= 0:
                    retire1(cur_grp, aggP)
                    retired.add(cur_grp)
                for g in range((NB + G1 - 1) // G1):
                    if g not in retired:
                        retire1(g, None)

            if stage == 2:
                nc.sync.dma_start(out=dbg_h[:], in_=hpart[:])

            # ============== conv2 + chunked RS + pipelined mm2 ==========
            if stage >= 3:
                pools2 = [
                    tc.tile_pool(name="g2", bufs=4),
                    tc.tile_pool(name="s2", bufs=6),
                    tc.tile_pool(name="r2", bufs=4),
                    tc.tile_pool(name="pa2", bufs=3, space="PSUM"),
                    tc.tile_pool(name="mm2", bufs=3),
                    tc.tile_pool(name="ps2", bufs=1, space="PSUM"),
                    tc.tile_pool(name="pacc", bufs=1, space="PSUM"),
                ]
                gpool, spool, rpool, papool, mpool, ppool, apool = [
                    p.__enter__() for p in pools2]
                mt = {}
                stile = {}

                def gather2(w0):
                    wh = min(w0 + W2CH, nch2)
                    t = gpool.tile([128, W2CH, HID], BF16, tag="m2")
                    nc.gpsimd.dma_gather(
                        t[:, :wh - w0, :], hp_rows,
                        gi2_s[:, w0 * 8:wh * 8],
                        (wh - w0) * 128, (wh - w0) * 128, HID,
                        queue_num=0, single_packet=False)
                    mt.clear()
                    mt[w0] = t

                def sbuild2(u0, sb_i):
                    uh = min(u0 + UB, len(units2))
                    t = spool.tile([128, UB, DW], BF16, tag="s")
                    if sb_i % ACT2 == ACT2 - 1:
                        for j in range(uh - u0):
                            tq = spool.tile([128, DW], BF16, tag="tq")
                            nc.scalar.activation(
                                tq[:], iotaF_t[:],
                                mybir.ActivationFunctionType.Square,
                                bias=dl2_s[:, u0 + j:u0 + j + 1],
                                scale=-1.0)
                            nc.scalar.activation(
                                t[:, j, :], tq[:],
                                mybir.ActivationFunctionType.Relu,
                                bias=ones_t[:], scale=-1.0)
                    else:
                        nc.vector.tensor_tensor(
                            t[:, :uh - u0, :],
                            dl2_s[:, u0:uh].rearrange("p (u x) -> p u x", x=1)
                            .broadcast_to([128, uh - u0, DW]),
                            iotaU_t[:, :(uh - u0) * DW]
                            .rearrange("p (u x) -> p u x", x=DW),
                            mybir.AluOpType.is_equal)
                    stile.clear()
                    stile[u0] = t

                ret_i = [0]
                qdmas = [[], [], [], []]     # retire DMA insts per quarter
                stage_state = {}             # (k, r) -> [tile, filled_set]

                flushed_qr = set()

                def flush_qr(k, r):
                    tile, filled = stage_state.pop((k, r))
                    ngrp = QLB[k] // G2
                    for gl in range(ngrp):
                        if gl not in filled:
                            nc.vector.memset(
                                tile[:, gl * G2 * DW:(gl + 1) * G2 * DW],
                                0.0)
                    dma = nc.gpsimd.dma_start(out=u2t_r[k][r, :, :],
                                              in_=tile[:, :QLB[k] * DW])
                    qdmas[k].append(dma)
                    flushed_qr.add((k, r))

                def retire2(g, aggP):
                    # g: group index over ordered blocks (G2 64-blocks)
                    ob0 = g * G2
                    b_glob = _B2ORDER[ob0]
                    r = b_glob // NB64
                    lb = b_glob % NB64
                    k = _q_of_lb(lb)
                    gl = (lb - _Q0[k]) // G2         # group within (k, r)
                    ngrp = QLB[k] // G2
                    if (k, r) not in stage_state:
                        tag = "u2sA" if QLB[k] == 56 else "u2sB"
                        st_t = rpool.tile([128, QLB[k] * DW], FP8, tag=tag,
                                          name=tag)
                        stage_state[(k, r)] = [st_t, set()]
                    tile, filled = stage_state[(k, r)]
                    sl = tile[:, gl * G2 * DW:(gl + 1) * G2 * DW]
                    eng = [nc.scalar, nc.scalar, nc.scalar, nc.scalar,
                           nc.vector][ret_i[0] % 5]
                    ret_i[0] += 1
                    if eng is nc.scalar:
                        nc.scalar.activation(
                            sl, aggP[:], mybir.ActivationFunctionType.Copy)
                    else:
                        nc.vector.tensor_copy(sl, aggP[:])
                    for j in range(G2):
                        if not has2[ob0 + j]:
                            nc.vector.memset(
                                tile[:, (gl * G2 + j) * DW:
                                     (gl * G2 + j + 1) * DW], 0.0)
                    filled.add(gl)
                    if len(filled) == ngrp:
                        flush_qr(k, r)

                # ---- mm2 chunk consumer (pooled psum held across chunks)
                pooled = [apool.tile([128, HID], F32, tag=f"pool{h}",
                                     name=f"pooled{h}")
                          for h in range(2)]
                NB128Q = [q // 2 for q in QLB]       # 128-blocks per chunk
                LB128Q = [q // 2 for q in _Q0]       # first 128-block

                def mm2_chunk(k):
                    u2o = u2o_k[k].ap()
                    first = (k == 0)
                    last = (k == 3)
                    nblk = NB128Q[k]
                    for gg in range(nblk // G3):
                        b0 = LB128Q[k] + gg * G3     # absolute 128-block
                        c0 = gg * G3 * 128
                        ga = mpool.tile([128, G3 * 128], FP8, tag="ga")
                        gd = nc.sync.dma_start(out=ga[:],
                                               in_=u2o[:, c0:c0 + G3 * 128])
                        _add_dep_helper(gd.ins, rs_cc[k].ins, True,
                                        f"mm2 chunk {k} reads RS{k}")
                        sel7 = mpool.tile([128, G3, 256], BF16, tag="sel7")
                        nc.vector.tensor_tensor(
                            sel7[:],
                            gid_t[:, b0:b0 + G3]
                            .rearrange("p (u x) -> p u x", x=1)
                            .broadcast_to([128, G3, 256]),
                            iota256_t[:]
                            .rearrange("p (u x) -> p u x", u=1)
                            .broadcast_to([128, G3, 256]),
                            mybir.AluOpType.is_equal)
                        z = mpool.tile([128, G3 * 128], BF16, tag="z2")
                        nc.vector.tensor_add(
                            z[:], ga[:],
                            hpsbT[:, b0 * 128:(b0 + G3) * 128])
                        h2p = ppool.tile([128, G3, HID], F32, tag="h2p")
                        for j in range(G3):
                            blk = b0 + j
                            nc.tensor.matmul(
                                h2p[:, j, :], z[:, j * 128:(j + 1) * 128],
                                w2_t[:], start=True, stop=True)
                            h2s = mpool.tile([128, HID], BF16, tag="h2s")
                            if bias_zero:
                                nc.scalar.activation(
                                    h2s[:], h2p[:, j, :],
                                    mybir.ActivationFunctionType.Relu,
                                    scale=dinv1_t[:, blk:blk + 1])
                            else:
                                hb2 = mpool.tile([128, HID], F32, tag="hb2")
                                nc.vector.tensor_scalar(
                                    hb2[:], h2p[:, j, :],
                                    dinv1_t[:, blk:blk + 1], None,
                                    mybir.AluOpType.mult)
                                nc.vector.tensor_add(hb2[:], hb2[:],
                                                     b2r_t[:])
                                nc.scalar.activation(
                                    h2s[:], hb2[:],
                                    mybir.ActivationFunctionType.Relu)
                            st = first and gg == 0 and j == 0
                            sp = last and gg == nblk // G3 - 1 and j == G3 - 1
                            for hh in range(2):
                                nc.tensor.matmul(
                                    pooled[hh][:],
                                    sel7[:, j, hh * 128:(hh + 1) * 128],
                                    h2s[:], start=st, stop=sp)

                # ---- conv2 main loop with interleaved RS / mm2
                q_last_grp = []                      # last retire group per q
                acc = 0
                for k in range(4):
                    q_last_grp.append((acc + QLB[k]) // G2 - 1)
                    acc += QLB[k]

                rs_emitted = []
                rs_cc = {}

                def emit_rs(k):
                    cc = nc.gpsimd.collective_compute(
                        "ReduceScatter", mybir.AluOpType.add, CORES,
                        [u2t_k[k][:]], [u2o_k[k][:]])
                    for d in qdmas[k]:
                        _add_dep_helper(cc.ins, d.ins, True,
                                        f"RS{k} waits quarter writes")
                    if rs_emitted:
                        _add_dep_helper(cc.ins, rs_cc[rs_emitted[-1]].ins,
                                        True, "collective order")
                    rs_cc[k] = cc
                    rs_emitted.append(k)

                aggP = None
                cur_grp = -1
                retired2 = set()
                sb_i = 0

                rs_ready = []

                def retire_and_track(g, aggP):
                    retire2(g, aggP)
                    retired2.add(g)
                    for k in range(4):
                        if g == q_last_grp[k]:
                            lo_g = q_last_grp[k - 1] + 1 if k else 0
                            if all(gg in retired2
                                   for gg in range(lo_g, g + 1)):
                                rs_ready.append(k)

                def maybe_emit_pending(blk):
                    # fire a ready RS once the loop is half-way through the
                    # NEXT quarter (gather descs pre-generated = DMA runway)
                    if not rs_ready:
                        return
                    k = rs_ready[0]
                    b_glob = _B2ORDER[blk // G2 * G2]
                    kq = _q_of_lb(b_glob % NB64)
                    r = b_glob // NB64
                    if kq > k + 1 or (kq == k + 1 and r >= 4):
                        rs_ready.pop(0)
                        emit_rs(k)
                        if stage >= 5 and k >= 1:
                            mm2_chunk(k - 1)

                for u, (cc, blk, lo, hi) in enumerate(units2):
                    w0 = cc // W2CH * W2CH
                    if w0 not in mt:
                        gather2(w0)
                    u0 = u // UB * UB
                    if u0 not in stile:
                        sbuild2(u0, sb_i)
                        sb_i += 1
                    g = blk // G2
                    maybe_emit_pending(blk)
                    if g != cur_grp:
                        if cur_grp >= 0:
                            retire_and_track(cur_grp, aggP)
                        cur_grp = g
                        aggP = papool.tile([128, G2 * DW], F32, tag="agg2")
                    j = blk - g * G2
                    nc.tensor.matmul(
                        aggP[:, j * DW:(j + 1) * DW], mt[w0][:, cc - w0, :],
                        stile[u0][:, u - u0, :],
                        start=ustart2[u], stop=ustop2[u])
                if cur_grp >= 0:
                    retire_and_track(cur_grp, aggP)
                # flush any incomplete / absent (quarter, range) staging
                for k in list(rs_ready):
                    pass
                for k in range(4):
                    for r in range(NC):
                        if (k, r) in stage_state:
                            flush_qr(k, r)
                        elif (k, r) not in flushed_qr:
                            tag = "u2sA" if QLB[k] == 56 else "u2sB"
                            st_t = rpool.tile([128, QLB[k] * DW], FP8,
                                              tag=tag, name=tag)
                            stage_state[(k, r)] = [st_t, set()]
                            flush_qr(k, r)
                    if k not in rs_emitted:
                        emit_rs(k)
                        if stage >= 5 and k >= 1:
                            mm2_chunk(k - 1)

                if stage == 4:
                    for k in range(4):
                        dd = nc.sync.dma_start(out=dbg_v[k][:],
                                               in_=u2o_k[k][:])
                        _add_dep_helper(dd.ins, rs_cc[k].ins, True, "dbg")
                if stage == 3:
                    for k in range(4):
                        dd = nc.sync.dma_start(out=dbg_u[k][:],
                                               in_=u2t_k[k][:])
                        for d in qdmas[k]:
                            _add_dep_helper(dd.ins, d.ins, True, "dbgu")

                pl_dma = None
                if stage >= 5:
                    mm2_chunk(3)
                    pl_r = pool_loc.ap().rearrange("(h p) f -> p h f", p=128)
                    pl_s = mpool.tile([128, 2, HID], BF16, tag="pls")
                    nc.vector.tensor_copy(pl_s[:, 0, :], pooled[0][:])
                    nc.vector.tensor_copy(pl_s[:, 1, :], pooled[1][:])
                    pl_dma = nc.sync.dma_start(out=pl_r[:], in_=pl_s[:])

                for p in reversed(pools2):
                    p.__exit__(None, None, None)

            if stage >= 5:
                ar_cc = nc.gpsimd.collective_compute(
                    "AllReduce", mybir.AluOpType.add, CORES,
                    [pool_loc[:]], [pool_glob[:]],
                )
                _add_dep_helper(ar_cc.ins, pl_dma.ins, True,
                                "AR waits pooled write")
                _add_dep_helper(ar_cc.ins, rs_cc[3].ins, True,
                                "collective order")

                # ---------------- head: mean-div, fc, LayerNorm (tiny)
                pg_r = pool_glob.ap().rearrange("(h p) f -> p h f", p=128)
                y_r = y_d.ap().rearrange("(h p) f -> p h f", p=128)
                with (
                    tc.tile_pool(name="head", bufs=1) as hpool,
                    tc.tile_pool(name="psh", bufs=2, space="PSUM") as hps,
                ):
                    wfc_t = hpool.tile([HID, HID], F32)
                    nc.sync.dma_start(out=wfc_t[:], in_=wfc_d[:])
                    bfcr_t = hpool.tile([128, HID], F32)
                    nc.sync.dma_start(out=bfcr_t[:], in_=bfcr_d[:])
                    gamr_t = hpool.tile([128, HID], F32)
                    nc.sync.dma_start(out=gamr_t[:], in_=gamr_d[:])
                    betr_t = hpool.tile([128, HID], F32)
                    nc.sync.dma_start(out=betr_t[:], in_=betr_d[:])
                    icnt_t = hpool.tile([128, 2], F32)
                    nc.sync.dma_start(out=icnt_t[:], in_=icnt_d[:])
                    eps_t = hpool.tile([128, 1], F32)
                    nc.vector.memset(eps_t[:], LN_EPS)
                    yo = hpool.tile([128, 2, HID], F32)
                    for hh in range(2):
                        pgb = hpool.tile([128, HID], BF16, tag="pgb")
                        pgd = nc.sync.dma_start(out=pgb[:],
                                                in_=pg_r[:, hh, :])
                        _add_dep_helper(pgd.ins, ar_cc.ins, True,
                                        "head reads AllReduce output")
                        pg_s = hpool.tile([128, HID], F32, tag="pg")
                        nc.vector.tensor_scalar(
                            pg_s[:], pgb[:], icnt_t[:, hh:hh + 1], None,
                            mybir.AluOpType.mult)
                        pgT_p = hps.tile([HID, 128], F32, tag="pgT")
                        nc.tensor.transpose(pgT_p[:], pg_s[:], eye_t[:])
                        pgT_s = hpool.tile([HID, 128], F32, tag="pgTs")
                        nc.vector.tensor_copy(pgT_s[:], pgT_p[:])
                        y_p = hps.tile([128, HID], F32, tag="yp")
                        nc.tensor.matmul(y_p[:], pgT_s[:], wfc_t[:])
                        y_s = hpool.tile([128, HID], F32, tag="ys")
                        nc.vector.tensor_add(y_s[:], y_p[:], bfcr_t[:])
                        mu = hpool.tile([128, 1], F32, tag="mu")
                        nc.vector.tensor_reduce(mu[:], y_s[:],
                                                mybir.AxisListType.XYZW,
                                                mybir.AluOpType.add)
                        nc.vector.tensor_scalar(mu[:], mu[:], -1.0 / HID,
                                                None, mybir.AluOpType.mult)
                        cen = hpool.tile([128, HID], F32, tag="cen")
                        nc.vector.tensor_scalar(cen[:], y_s[:], mu[:], None,
                                                mybir.AluOpType.add)
                        sq = hpool.tile([128, HID], F32, tag="sq")
                        nc.vector.tensor_mul(sq[:], cen[:], cen[:])
                        var = hpool.tile([128, 1], F32, tag="var")
                        nc.vector.tensor_reduce(var[:], sq[:],
                                                mybir.AxisListType.XYZW,
                                                mybir.AluOpType.add)
                        std = hpool.tile([128, 1], F32, tag="std")
                        nc.scalar.activation(
                            std[:], var[:],
                            mybir.ActivationFunctionType.Sqrt,
                            bias=eps_t[:], scale=1.0 / HID)
                        rstd = hpool.tile([128, 1], F32, tag="rstd")
                        nc.vector.reciprocal(rstd[:], std[:])
                        nc.vector.tensor_scalar(cen[:], cen[:], rstd[:],
                                                None, mybir.AluOpType.mult)
                        nc.vector.tensor_mul(cen[:], cen[:], gamr_t[:])
                        nc.vector.tensor_add(yo[:, hh, :], cen[:], betr_t[:])
                    nc.sync.dma_start(out=y_r[:], in_=yo[:])

    nc.compile()
    return nc


_CACHE = {}


def make_in_maps(x, edge_index, batch, W1, b1, W2, b2, Wfc, bfc, gamma, beta,
                 per_core=None, inv_cnt_w=None, dinv=None, meta=None):
    if per_core is None:
        per_core, inv_cnt_w, dinv, meta = _host_prep(
            np.asarray(edge_index), np.asarray(batch))
    x = np.asarray(x, np.float32)
    xp = x * dinv[:, None]
    xb4 = xp.astype(ml_dtypes.bfloat16).reshape(N // 4, 256)
    xself = (xp * dinv[:, None]).astype(np.float32)
    rep = lambda v: np.tile(np.asarray(v, np.float32)[None, :], (128, 1))
    bias_zero = (not np.any(np.asarray(b1))) and (not np.any(np.asarray(b2)))
    shared = {
        "xb4": xb4,
        "w1": np.asarray(W1, np.float32).astype(ml_dtypes.bfloat16),
        "w2": np.asarray(W2, np.float32).astype(ml_dtypes.bfloat16),
        "wfc": np.asarray(Wfc, np.float32),
        "bfcr": rep(bfc),
        "gamr": rep(gamma), "betr": rep(beta),
        "icnt": inv_cnt_w,
    }
    if not bias_zero:
        shared["b1r"] = rep(b1)
        shared["b2r"] = rep(b2)
    in_maps = []
    for c in range(NC):
        m = dict(shared)
        xs = np.zeros((IN, NPC), np.float32)
        xs[:, :RPC] = xself[c * RPC:(c + 1) * RPC].T
        m["xst"] = xs.astype(ml_dtypes.bfloat16)
        for k in ("gi1", "gi2", "dl1", "dl2", "gid",
                  "dinv1", "dinv2"):
            m[k] = per_core[c][k]
        in_maps.append(m)
    return in_maps, bias_zero, meta


def kernel(x, edge_index, batch, W1, b1, W2, b2, Wfc, bfc, gamma, beta,
           _stage=5, _full_results=False):
    per_core, inv_cnt_w, dinv, meta = _host_prep(np.asarray(edge_index),
                                                 np.asarray(batch))
    in_maps, bias_zero, meta = make_in_maps(
        x, edge_index, batch, W1, b1, W2, b2, Wfc, bfc, gamma, beta,
        per_core, inv_cnt_w, dinv, meta)
    key = (meta["epad1"], meta["epad2"], meta["nu1p"], meta["nu2p"],
           bias_zero, _stage)
    if key not in _CACHE:
        _CACHE[key] = _build(meta, bias_zero, _stage)
    nc = _CACHE[key]

    res = run_bass_kernel_spmd(nc, in_maps, list(range(NC)))
    if _full_results:
        return res.results
    return res.results[0]["y"]


# revision 82
# speedup vs baseline: 1.0272x; 1.0095x over previous
"""GCN (2x GCNConv + mean-pool + fc + LayerNorm) on 8 Trainium2 NeuronCores.

One-hot matmul aggregation replaces DMA scatter-adds entirely:

conv1 (pull): per-core in-edges on a cross-core-common segment grid
(64-node dst block x src%4 slot, segment length = max over cores). Each
128-edge chunk is gathered with one 512B descriptor per edge from a
4-row-packed bf16 x' table (the edge's row is slot src%4). For each
(chunk x segment) overlap ("unit"), PE accumulates
zT[64f, 64d] += M[128e, 64f].T @ S[128e, 64d], where S is a one-hot built
from a per-unit dstloc column: batched is_equal-vs-iota on DVE, with a
share built on the Activation engine as relu(1 - (iota - dstloc)^2).
The implicit self-loop is a second matmul against a transposed prescaled
x table, so it never enters the edge stream, and mm1 needs no transpose
(zT is the lhsT directly).

conv2 (push): per-core out-edges on a common global 64-node dst-block
grid, blocks ordered (quarter, range, local) so each node-quarter of the
TRANSPOSED partials u2T completes early; gathers hit the local hpart rows
(h1' = relu(h1)*dinv, prescaled). FOUR chunked ReduceScatters fire as
their quarter's writes land, overlapping the next quarter's compute, and
mm2 consumes each reduced chunk one quarter later (no transposes: z2T is
the lhsT; the self term adds from an SBUF-resident transposed hpsbT).
Pooling is the PSUM-accumulated one-hot graph matmul; pooled sums are
AllReduced; the tiny fc+LayerNorm head is computed redundantly.
"""
import sys

if '/opt/trn_rl_repo' not in sys.path:
    sys.path.insert(0, '/opt/trn_rl_repo')

import ml_dtypes
import numpy as np

import concourse.bacc as bacc
import concourse.mybir as mybir
from concourse.bass import _add_dep_helper
from concourse.tile import TileContext
from concourse.bass_utils import run_bass_kernel_spmd

# ---------------------------------------------------------------- constants
N = 100000
E = 800000
IN = 64
HID = 128
G = 256
NC = 8
RPC = N // NC              # 12500 nodes per core
NB = 98                    # 128-node dst blocks per core (12544 padded)
NPC = NB * 128             # 12544
DW = 64                    # aggregation dst-block width
NB64 = NPC // DW           # 196 64-blocks per core
NBG64 = NC * NB64          # 1568 global 64-blocks
QLB = (56, 56, 56, 28)     # conv2 RS chunks (in 64-blocks per core)
LN_EPS = 1e-5
W1CH = 16                  # conv1 gather window (chunks per call)
W2CH = 32                  # conv2 gather window
UB = 32                    # S-build batch (units per instruction)
ACT1 = 10 ** 9             # conv1: every ACT1-th S-batch goes to Act engine
ACT2 = 10 ** 9             # conv2: every ACT2-th S-batch goes to Act engine
G1 = 4                     # conv1 retire group (128-blocks)
G2 = 7                     # conv2 retire group (64-blocks; divides 56)
G3 = 7                     # mm2 group (128-blocks)
F32 = mybir.dt.float32
FP8 = mybir.dt.float8e4
BF16 = mybir.dt.bfloat16
I16 = mybir.dt.int16


def _wrap16(a, cols):
    """[n] -> [128, cols] int16: element i -> [i%16, i//16], tiled x8."""
    out = np.zeros((16, cols), np.int16)
    w = a.reshape(-1, 16).T
    out[:, : w.shape[1]] = w
    return np.tile(out, (8, 1))


def _common_grid(counts):
    seg = counts.max(axis=0).astype(np.int64)
    off = np.concatenate([[0], np.cumsum(seg)])
    epad = int((off[-1] + 127) // 128 * 128)
    return seg, off, epad


def _make_units(seg, off, nblk, seg_per_blk):
    """(chunk, blk, lo, hi) overlaps of 128-edge chunks with segments.

    Segment k belongs to block k // seg_per_blk (blocks in segment order).
    Returns units, per-unit start/stop (first/last unit of its block), and
    per-block has_units.
    """
    units = []
    blk_first = {}
    blk_last = {}
    for k in range(len(seg)):
        if seg[k] == 0:
            continue
        blk = k // seg_per_blk
        a, b = int(off[k]), int(off[k] + seg[k])
        for cc in range(a // 128, (b - 1) // 128 + 1):
            lo, hi = max(a, cc * 128), min(b, (cc + 1) * 128)
            u = len(units)
            units.append((cc, blk, lo, hi))
            if blk not in blk_first:
                blk_first[blk] = u
            blk_last[blk] = u
    start = [False] * len(units)
    stop = [False] * len(units)
    for blk, u in blk_first.items():
        start[u] = True
    for blk, u in blk_last.items():
        stop[u] = True
    has = [blk in blk_first for blk in range(nblk)]
    return units, start, stop, has


def _q_of_lb(lb):
    """conv2 node-quarter of a local 64-block index."""
    acc = 0
    for q, n in enumerate(QLB):
        if lb < acc + n:
            return q
        acc += n
    raise ValueError(lb)


# conv2 ordered block list: (quarter, range, lb64) lexicographic
_B2ORDER = sorted(range(NBG64),
                  key=lambda b: (_q_of_lb(b % NB64), b // NB64, b % NB64))
_Q0 = [0, 56, 112, 168]            # quarter start lb64


def _host_prep(edge_index, batch):
    src = np.asarray(edge_index[0], np.int64)
    dst = np.asarray(edge_index[1], np.int64)
    deg = (np.bincount(dst, minlength=N) + 1.0).astype(np.float32)
    dinv = (1.0 / np.sqrt(deg)).astype(np.float32)

    # ---------------- conv1: (blk64, slot) grid
    e_core1 = []
    cnt1 = np.zeros((NC, NB64 * 4), np.int64)
    for c in range(NC):
        m = (dst >= c * RPC) & (dst < (c + 1) * RPC)
        s, d = src[m], dst[m] - c * RPC
        key = (d // DW) * 4 + (s % 4)
        cnt1[c] = np.bincount(key, minlength=NB64 * 4)
        order = np.lexsort((d, key))
        e_core1.append((s[order], d[order], key[order]))
    seg1, off1, epad1 = _common_grid(cnt1)
    units1, ustart1, ustop1, has1 = _make_units(seg1, off1, NB64, 4)
    nch1 = epad1 // 128
    nu1 = len(units1)
    nu1p = (nu1 + UB - 1) // UB * UB
    uslot1 = []
    for u, (cc, blk, lo, hi) in enumerate(units1):
        base = blk * 4
        q = 0
        for qq in range(4):
            a, b = off1[base + qq], off1[base + qq] + seg1[base + qq]
            if a <= lo < b:
                q = qq
                break
        uslot1.append(q)

    # ---------------- conv2: global 64-blocks in _B2ORDER
    e_core2 = []
    cnt2 = np.zeros((NC, NBG64), np.int64)   # indexed by ORDER position
    b2pos = np.empty(NBG64, np.int64)        # global blk64 -> order position
    for i, b in enumerate(_B2ORDER):
        b2pos[b] = i
    for c in range(NC):
        m = (src >= c * RPC) & (src < (c + 1) * RPC)
        s, d = src[m] - c * RPC, dst[m]
        rr = d // RPC                      # owner range of the dst
        dloc = d - rr * RPC                # dst local to its owner core
        key = b2pos[rr * NB64 + dloc // DW]
        cnt2[c] = np.bincount(key, minlength=NBG64)
        order = np.lexsort((dloc, key))
        e_core2.append((s[order], dloc[order], key[order]))
    seg2, off2, epad2 = _common_grid(cnt2)
    units2, ustart2, ustop2, has2 = _make_units(seg2, off2, NBG64, 1)
    nch2 = epad2 // 128
    nu2 = len(units2)
    nu2p = (nu2 + UB - 1) // UB * UB

    # ---------------- per-core streams
    per_core = []
    for c in range(NC):
        s, d, key = e_core1[c]
        pos = np.empty(len(s), np.int64)
        ptr = 0
        for k in np.unique(key):
            n = int(cnt1[c][k])
            pos[ptr:ptr + n] = off1[k] + np.arange(n)
            ptr += n
        gi1 = np.zeros(epad1, np.int16)
        gi1[pos] = (s // 4).astype(np.int16)
        sd1 = np.full(epad1, -1.0, np.float32)
        sd1[pos] = (d % DW).astype(np.float32)
        sl1 = np.full(epad1, -1, np.int8)
        sl1[pos] = (s % 4).astype(np.int8)
        dl1c = np.full((nu1p, 128), -1.0, np.float32)
        for u, (cc, blk, lo, hi) in enumerate(units1):
            p0 = lo - cc * 128
            q = uslot1[u]
            dl1c[u, p0:hi - cc * 128] = np.where(
                sl1[lo:hi] == q, sd1[lo:hi], -1.0)

        s2, d2, key2 = e_core2[c]
        pos2 = np.empty(len(s2), np.int64)
        ptr = 0
        for k in np.unique(key2):
            n = int(cnt2[c][k])
            pos2[ptr:ptr + n] = off2[k] + np.arange(n)
            ptr += n
        gi2 = np.zeros(epad2, np.int16)
        gi2[pos2] = ((s2 % 128) * NB + s2 // 128).astype(np.int16)
        sd2 = np.full(epad2, -1.0, np.float32)
        sd2[pos2] = (d2 % DW).astype(np.float32)
        dl2c = np.full((nu2p, 128), -1.0, np.float32)
        for u, (cc, blk, lo, hi) in enumerate(units2):
            p0 = lo - cc * 128
            dl2c[u, p0:hi - cc * 128] = sd2[lo:hi]

        dv = np.zeros(NPC, np.float32)
        dv[:RPC] = dinv[c * RPC:(c + 1) * RPC]
        dvt = dv.reshape(NB, 128).T.copy()
        gid = np.asarray(batch, np.int64)
        gv = np.full(NPC, -1.0, np.float32)
        gv[:RPC] = gid[c * RPC:(c + 1) * RPC].astype(np.float32)

        per_core.append({
            "gi1": _wrap16(gi1, epad1 // 16),
            "gi2": _wrap16(gi2, epad2 // 16),
            "dl1": dl1c.T.astype(ml_dtypes.bfloat16).copy(),
            "dl2": dl2c.T.astype(ml_dtypes.bfloat16).copy(),
            "dinv1": dvt,
            "dinv2": (dvt * dvt).copy(),
            "gid": gv.reshape(NB, 128).T.copy(),
        })

    gidn = np.asarray(batch, np.int64)
    cntg = np.bincount(gidn, minlength=G).astype(np.float32)
    inv_cnt_w = (1.0 / np.maximum(cntg, 1.0)).reshape(2, 128).T.copy()

    meta = {"epad1": epad1, "nch1": nch1, "units1": units1,
            "ustart1": ustart1, "ustop1": ustop1, "has1": has1, "nu1p": nu1p,
            "uslot1": uslot1,
            "epad2": epad2, "nch2": nch2, "units2": units2,
            "ustart2": ustart2, "ustop2": ustop2, "has2": has2, "nu2p": nu2p}
    return per_core, inv_cnt_w, dinv, meta


def _build(meta, bias_zero=True, stage=5):
    nc = bacc.Bacc("TRN2", target_bir_lowering=False, debug=False,
                   num_devices=NC, num_swdge_queues=1,
                   dynamic_dma_scratch_size=32768)
    epad1, nch1 = meta["epad1"], meta["nch1"]
    units1, ustart1, ustop1 = meta["units1"], meta["ustart1"], meta["ustop1"]
    has1, nu1p, uslot1 = meta["has1"], meta["nu1p"], meta["uslot1"]
    epad2, nch2 = meta["epad2"], meta["nch2"]
    units2, ustart2, ustop2 = meta["units2"], meta["ustart2"], meta["ustop2"]
    has2, nu2p = meta["has2"], meta["nu2p"]
    ecols1, ecols2 = epad1 // 16, epad2 // 16

    # ------------------------------------------------ I/O declarations
    xb4_d = nc.dram_tensor("xb4", [N // 4, 256], BF16, kind="ExternalInput")
    xst_d = nc.dram_tensor("xst", [IN, NPC], BF16, kind="ExternalInput")
    w1_d = nc.dram_tensor("w1", [IN, HID], BF16, kind="ExternalInput")
    w2_d = nc.dram_tensor("w2", [HID, HID], BF16, kind="ExternalInput")
    wfc_d = nc.dram_tensor("wfc", [HID, HID], F32, kind="ExternalInput")
    bfcr_d = nc.dram_tensor("bfcr", [128, HID], F32, kind="ExternalInput")
    gamr_d = nc.dram_tensor("gamr", [128, HID], F32, kind="ExternalInput")
    betr_d = nc.dram_tensor("betr", [128, HID], F32, kind="ExternalInput")
    if not bias_zero:
        b1r_d = nc.dram_tensor("b1r", [128, HID], F32, kind="ExternalInput")
        b2r_d = nc.dram_tensor("b2r", [128, HID], F32, kind="ExternalInput")
    gi1_d = nc.dram_tensor("gi1", [128, ecols1], I16, kind="ExternalInput")
    gi2_d = nc.dram_tensor("gi2", [128, ecols2], I16, kind="ExternalInput")
    dl1_d = nc.dram_tensor("dl1", [128, nu1p], BF16, kind="ExternalInput")
    dl2_d = nc.dram_tensor("dl2", [128, nu2p], BF16, kind="ExternalInput")
    gid_d = nc.dram_tensor("gid", [128, NB], F32, kind="ExternalInput")
    dinv1_d = nc.dram_tensor("dinv1", [128, NB], F32, kind="ExternalInput")
    dinv2_d = nc.dram_tensor("dinv2", [128, NB], F32, kind="ExternalInput")
    icnt_d = nc.dram_tensor("icnt", [128, 2], F32, kind="ExternalInput")

    hpart = nc.dram_tensor("hpart", [NPC, HID], BF16)
    u2t_k = [nc.dram_tensor(f"u2t{k}", [NC * HID, QLB[k] * DW], FP8)
             for k in range(4)]
    u2o_k = [nc.dram_tensor(f"u2o{k}", [HID, QLB[k] * DW], FP8)
             for k in range(4)]
    pool_loc = nc.dram_tensor("pool_loc", [G, HID], BF16)
    pool_glob = nc.dram_tensor("pool_glob", [G, HID], BF16,
                               addr_space="Shared")
    y_d = nc.dram_tensor("y", [G, HID], F32, kind="ExternalOutput")
    if stage == 2:
        dbg_h = nc.dram_tensor("dbg_h", [NPC, HID], BF16,
                               kind="ExternalOutput")
    if stage == 4:
        dbg_v = [nc.dram_tensor(f"dbg_v{k}", [HID, QLB[k] * DW], FP8,
                                kind="ExternalOutput") for k in range(4)]
    if stage == 3:
        dbg_u = [nc.dram_tensor(f"dbg_u{k}", [NC * HID, QLB[k] * DW], FP8,
                                kind="ExternalOutput") for k in range(4)]

    eye_d = nc.inline_tensor(np.eye(128, dtype=np.float32), name="eye128")
    eyeb_d = nc.inline_tensor(np.eye(128, dtype=ml_dtypes.bfloat16),
                              name="eye128b")
    iotaU_np = np.tile(np.arange(DW, dtype=np.float32),
                       (128, UB)).astype(ml_dtypes.bfloat16)
    iotaU_d = nc.inline_tensor(iotaU_np, name="iotaU")
    iotaF_d = nc.inline_tensor(
        np.tile(np.arange(DW, dtype=np.float32),
                (128, 1)).astype(ml_dtypes.bfloat16), name="iotaF")
    iota256_d = nc.inline_tensor(
        np.tile(np.arange(256, dtype=np.float32), (128, 1)), name="iota256")


    xb4 = xb4_d.ap()
    hp_rows = hpart.ap()
    CORES = [list(range(NC))]

    # persistent SBUF
    gi1_s = nc.alloc_sbuf_tensor("gi1_s", [128, ecols1], I16)
    gi2_s = nc.alloc_sbuf_tensor("gi2_s", [128, ecols2], I16)
    dl1_s = nc.alloc_sbuf_tensor("dl1_s", [128, nu1p], BF16)
    dl2_s = nc.alloc_sbuf_tensor("dl2_s", [128, nu2p], BF16)
    xst_s = nc.alloc_sbuf_tensor("xst_s", [IN, NPC], BF16)
    hpsbT = nc.alloc_sbuf_tensor("hpsbT", [128, NB * 128], BF16)

    with TileContext(nc) as tc:
        with tc.tile_pool(name="init", bufs=1) as ipool:
            nc.sync.dma_start(out=gi1_s[:], in_=gi1_d[:])
            nc.sync.dma_start(out=gi2_s[:], in_=gi2_d[:])
            nc.sync.dma_start(out=dl1_s[:], in_=dl1_d[:])
            nc.sync.dma_start(out=dl2_s[:], in_=dl2_d[:])
            nc.sync.dma_start(out=xst_s[:], in_=xst_d[:])
            eye_t = ipool.tile([128, 128], F32)
            nc.sync.dma_start(out=eye_t[:], in_=eye_d[:])
            eyeb_t = ipool.tile([128, 128], BF16)
            nc.sync.dma_start(out=eyeb_t[:], in_=eyeb_d[:])
            iotaU_t = ipool.tile([128, UB * DW], BF16)
            nc.sync.dma_start(out=iotaU_t[:], in_=iotaU_d[:])
            iotaF_t = ipool.tile([128, DW], BF16)
            nc.sync.dma_start(out=iotaF_t[:], in_=iotaF_d[:])
            iota256_t = ipool.tile([128, 256], F32)
            nc.sync.dma_start(out=iota256_t[:], in_=iota256_d[:])

            w1_t = ipool.tile([IN, HID], BF16)
            nc.sync.dma_start(out=w1_t[:], in_=w1_d[:])
            w2_t = ipool.tile([HID, HID], BF16)
            nc.sync.dma_start(out=w2_t[:], in_=w2_d[:])
            gid_t = ipool.tile([128, NB], F32)
            nc.sync.dma_start(out=gid_t[:], in_=gid_d[:])

            dinv1_t = ipool.tile([128, NB], F32)
            nc.sync.dma_start(out=dinv1_t[:], in_=dinv1_d[:])
            dinv2_t = ipool.tile([128, NB], F32)
            nc.sync.dma_start(out=dinv2_t[:], in_=dinv2_d[:])
            ones_t = ipool.tile([128, 1], BF16)
            nc.vector.memset(ones_t[:], 1.0)
            if not bias_zero:
                b1r_t = ipool.tile([128, HID], F32)
                nc.sync.dma_start(out=b1r_t[:], in_=b1r_d[:])
                b2r_t = ipool.tile([128, HID], F32)
                nc.sync.dma_start(out=b2r_t[:], in_=b2r_d[:])

            hp_r = hpart.ap().rearrange("(p a) f -> p a f", p=128)
            u2t_r = [t.ap().rearrange("(r p) n -> r p n", p=HID)
                     for t in u2t_k]

            # ======================= conv1 =======================
            with (
                tc.tile_pool(name="g1", bufs=4) as gpool,
                tc.tile_pool(name="s1", bufs=6) as spool,
                tc.tile_pool(name="r1", bufs=3) as rpool,
                tc.tile_pool(name="pa1", bufs=3, space="PSUM") as papool,
                tc.tile_pool(name="ph1", bufs=2, space="PSUM") as phpool,
                tc.tile_pool(name="pt1", bufs=3, space="PSUM") as ptpool,
            ):
                mt = {}

                def gather1(w0):
                    wh = min(w0 + W1CH, nch1)
                    t = gpool.tile([128, W1CH, 256], BF16, tag="m1")
                    nc.gpsimd.dma_gather(
                        t[:, :wh - w0, :], xb4, gi1_s[:, w0 * 8:wh * 8],
                        (wh - w0) * 128, (wh - w0) * 128, 256,
                        queue_num=0, single_packet=False)
                    mt.clear()
                    mt[w0] = t

                stile = {}

                def sbuild(u0, nu, dl_s, act_every, sb_i):
                    uh = min(u0 + UB, nu)
                    t = spool.tile([128, UB, DW], BF16, tag="s")
                    if sb_i % act_every == act_every - 1:
                        for j in range(uh - u0):
                            tq = spool.tile([128, DW], BF16, tag="tq")
                            nc.scalar.activation(
                                tq[:], iotaF_t[:],
                                mybir.ActivationFunctionType.Square,
                                bias=dl_s[:, u0 + j:u0 + j + 1], scale=-1.0)
                            nc.scalar.activation(
                                t[:, j, :], tq[:],
                                mybir.ActivationFunctionType.Relu,
                                bias=ones_t[:], scale=-1.0)
                    else:
                        nc.vector.tensor_tensor(
                            t[:, :uh - u0, :],
                            dl_s[:, u0:uh].rearrange("p (u x) -> p u x", x=1)
                            .broadcast_to([128, uh - u0, DW]),
                            iotaU_t[:, :(uh - u0) * DW]
                            .rearrange("p (u x) -> p u x", x=DW),
                            mybir.AluOpType.is_equal)
                    stile.clear()
                    stile[u0] = t

                # conv1: retire group = G1 128-blocks = 2*G1 64-blocks
                B64G = 2 * G1

                r1_i = [0]

                def retire1(g, aggP):
                    b0 = g * G1                      # first 128-block
                    nb_ = min(G1, NB - b0)
                    zs = rpool.tile([64, G1 * 128], BF16, tag="zsb")
                    if aggP is not None:
                        r1_i[0] += 1
                        if r1_i[0] % 3 == 0:
                            nc.vector.tensor_copy(zs[:, :nb_ * 128],
                                                  aggP[:, :nb_ * 128])
                        else:
                            nc.scalar.activation(
                                zs[:, :nb_ * 128], aggP[:, :nb_ * 128],
                                mybir.ActivationFunctionType.Copy)
                    for j64 in range(nb_ * 2):
                        if not has1[g * B64G + j64]:
                            nc.vector.memset(
                                zs[:, j64 * DW:(j64 + 1) * DW], 0.0)
                    hps = phpool.tile([128, G1, 128], F32, tag="hps")
                    hg = rpool.tile([128, G1, 128], BF16, tag="hg")
                    for j in range(nb_):
                        blk = b0 + j
                        nc.tensor.matmul(hps[:, j, :],
                                         zs[:, j * 128:(j + 1) * 128],
                                         w1_t[:], start=True, stop=False)
                        nc.tensor.matmul(
                            hps[:, j, :],
                            xst_s[:, blk * 128:(blk + 1) * 128],
                            w1_t[:], start=False, stop=True)
                        if bias_zero:
                            nc.scalar.activation(
                                hg[:, j, :], hps[:, j, :],
                                mybir.ActivationFunctionType.Relu,
                                scale=dinv2_t[:, blk:blk + 1])
                        else:
                            hb = rpool.tile([128, HID], F32, tag="hb")
                            nc.vector.tensor_scalar(
                                hb[:], hps[:, j, :],
                                dinv1_t[:, blk:blk + 1], None,
                                mybir.AluOpType.mult)
                            nc.vector.tensor_add(hb[:], hb[:], b1r_t[:])
                            hr = rpool.tile([128, HID], F32, tag="hr")
                            nc.scalar.activation(
                                hr[:], hb[:],
                                mybir.ActivationFunctionType.Relu)
                            nc.vector.tensor_scalar(
                                hg[:, j, :], hr[:],
                                dinv1_t[:, blk:blk + 1], None,
                                mybir.AluOpType.mult)
                        tp = ptpool.tile([128, 128], BF16, tag="tp")
                        nc.tensor.transpose(tp[:], hg[:, j, :], eyeb_t[:])
                        if blk % 3 == 2:
                            nc.vector.tensor_copy(
                                hpsbT[:, blk * 128:(blk + 1) * 128], tp[:])
                        else:
                            nc.scalar.activation(
                                hpsbT[:, blk * 128:(blk + 1) * 128], tp[:],
                                mybir.ActivationFunctionType.Copy)
                    nc.sync.dma_start(out=hp_r[:, b0:b0 + nb_, :],
                                        in_=hg[:, :nb_, :])

                aggP = None
                cur_grp = -1
                retired = set()
                sb_i = 0
                for u, (cc, blk, lo, hi) in enumerate(units1):
                    w0 = cc // W1CH * W1CH
                    if w0 not in mt:
                        gather1(w0)
                    u0 = u // UB * UB
                    if u0 not in stile:
                        sbuild(u0, len(units1), dl1_s, ACT1, sb_i)
                        sb_i += 1
                    g = blk // B64G
                    if g != cur_grp:
                        if cur_grp >= 0:
                            retire1(cur_grp, aggP)
                            retired.add(cur_grp)
                        cur_grp = g
                        aggP = papool.tile([64, B64G * DW], F32, tag="agg")
                    j = blk - g * B64G
                    q = uslot1[u]
                    nc.tensor.matmul(
                        aggP[:, j * DW:(j + 1) * DW],
                        mt[w0][:, cc - w0, 64 * q:64 * q + 64],
                        stile[u0][:, u - u0, :],
                        start=ustart1[u], stop=ustop1[u])
                if cur_grp >= 0:
                    retire1(cur_grp, aggP)
                    retired.add(cur_grp)
                for g in range((NB + G1 - 1) // G1):
                    if g not in retired:
                        retire1(g, None)

            if stage == 2:
                nc.sync.dma_start(out=dbg_h[:], in_=hpart[:])

            # ============== conv2 + chunked RS + pipelined mm2 ==========
            if stage >= 3:
                pools2 = [
                    tc.tile_pool(name="g2", bufs=4),
                    tc.tile_pool(name="s2", bufs=6),
                    tc.tile_pool(name="r2", bufs=4),
                    tc.tile_pool(name="pa2", bufs=3, space="PSUM"),
                    tc.tile_pool(name="mm2", bufs=3),
                    tc.tile_pool(name="ps2", bufs=1, space="PSUM"),
                    tc.tile_pool(name="pacc", bufs=1, space="PSUM"),
                ]
                gpool, spool, rpool, papool, mpool, ppool, apool = [
                    p.__enter__() for p in pools2]
                mt = {}
                stile = {}

                def gather2(w0):
                    wh = min(w0 + W2CH, nch2)
                    t = gpool.tile([128, W2CH, HID], BF16, tag="m2")
                    nc.gpsimd.dma_gather(
                        t[:, :wh - w0, :], hp_rows,
                        gi2_s[:, w0 * 8:wh * 8],
                        (wh - w0) * 128, (wh - w0) * 128, HID,
                        queue_num=0, single_packet=False)
                    mt.clear()
                    mt[w0] = t

                def sbuild2(u0, sb_i):
                    uh = min(u0 + UB, len(units2))
                    t = spool.tile([128, UB, DW], BF16, tag="s")
                    if sb_i % ACT2 == ACT2 - 1:
                        for j in range(uh - u0):
                            tq = spool.tile([128, DW], BF16, tag="tq")
                            nc.scalar.activation(
                                tq[:], iotaF_t[:],
                                mybir.ActivationFunctionType.Square,
                                bias=dl2_s[:, u0 + j:u0 + j + 1],
                                scale=-1.0)
                            nc.scalar.activation(
                                t[:, j, :], tq[:],
                                mybir.ActivationFunctionType.Relu,
                                bias=ones_t[:], scale=-1.0)
                    else:
                        nc.vector.tensor_tensor(
                            t[:, :uh - u0, :],
                            dl2_s[:, u0:uh].rearrange("p (u x) -> p u x", x=1)
                            .broadcast_to([128, uh - u0, DW]),
                            iotaU_t[:, :(uh - u0) * DW]
                            .rearrange("p (u x) -> p u x", x=DW),
                            mybir.AluOpType.is_equal)
                    stile.clear()
                    stile[u0] = t

                ret_i = [0]
                qdmas = [[], [], [], []]     # retire DMA insts per quarter
                stage_state = {}             # (k, r) -> [tile, filled_set]

                flushed_qr = set()

                def flush_qr(k, r):
                    tile, filled = stage_state.pop((k, r))
                    ngrp = QLB[k] // G2
                    for gl in range(ngrp):
                        if gl not in filled:
                            nc.vector.memset(
                                tile[:, gl * G2 * DW:(gl + 1) * G2 * DW],
                                0.0)
                    dma = nc.gpsimd.dma_start(out=u2t_r[k][r, :, :],
                                              in_=tile[:, :QLB[k] * DW])
                    qdmas[k].append(dma)
                    flushed_qr.add((k, r))

                def retire2(g, aggP):
                    # g: group index over ordered blocks (G2 64-blocks)
                    ob0 = g * G2
                    b_glob = _B2ORDER[ob0]
                    r = b_glob // NB64
                    lb = b_glob % NB64
                    k = _q_of_lb(lb)
                    gl = (lb - _Q0[k]) // G2         # group within (k, r)
                    ngrp = QLB[k] // G2
                    if (k, r) not in stage_state:
                        tag = "u2sA" if QLB[k] == 56 else "u2sB"
                        st_t = rpool.tile([128, QLB[k] * DW], FP8, tag=tag,
                                          name=tag)
                        stage_state[(k, r)] = [st_t, set()]
                    tile, filled = stage_state[(k, r)]
                    sl = tile[:, gl * G2 * DW:(gl + 1) * G2 * DW]
                    eng = [nc.scalar, nc.scalar, nc.scalar, nc.scalar,
                           nc.vector][ret_i[0] % 5]
                    ret_i[0] += 1
                    if eng is nc.scalar:
                        nc.scalar.activation(
                            sl, aggP[:], mybir.ActivationFunctionType.Copy)
                    else:
                        nc.vector.tensor_copy(sl, aggP[:])
                    for j in range(G2):
                        if not has2[ob0 + j]:
                            nc.vector.memset(
                                tile[:, (gl * G2 + j) * DW:
                                     (gl * G2 + j + 1) * DW], 0.0)
                    filled.add(gl)
                    if len(filled) == ngrp:
                        flush_qr(k, r)

                # ---- mm2 chunk consumer (pooled psum held across chunks)
                pooled = [apool.tile([128, HID], F32, tag=f"pool{h}",
                                     name=f"pooled{h}")
                          for h in range(2)]
                NB128Q = [q // 2 for q in QLB]       # 128-blocks per chunk
                LB128Q = [q // 2 for q in _Q0]       # first 128-block

                def mm2_chunk(k):
                    u2o = u2o_k[k].ap()
                    first = (k == 0)
                    last = (k == 3)
                    nblk = NB128Q[k]
                    for gg in range(nblk // G3):
                        b0 = LB128Q[k] + gg * G3     # absolute 128-block
                        c0 = gg * G3 * 128
                        ga = mpool.tile([128, G3 * 128], FP8, tag="ga")
                        gd = nc.sync.dma_start(out=ga[:],
                                               in_=u2o[:, c0:c0 + G3 * 128])
                        _add_dep_helper(gd.ins, rs_cc[k].ins, True,
                                        f"mm2 chunk {k} reads RS{k}")
                        sel7 = mpool.tile([128, G3, 256], BF16, tag="sel7")
                        nc.vector.tensor_tensor(
                            sel7[:],
                            gid_t[:, b0:b0 + G3]
                            .rearrange("p (u x) -> p u x", x=1)
                            .broadcast_to([128, G3, 256]),
                            iota256_t[:]
                            .rearrange("p (u x) -> p u x", u=1)
                            .broadcast_to([128, G3, 256]),
                            mybir.AluOpType.is_equal)
                        z = mpool.tile([128, G3 * 128], BF16, tag="z2")
                        nc.vector.tensor_add(
                            z[:], ga[:],
                            hpsbT[:, b0 * 128:(b0 + G3) * 128])
                        h2p = ppool.tile([128, G3, HID], F32, tag="h2p")
                        for j in range(G3):
                            blk = b0 + j
                            nc.tensor.matmul(
                                h2p[:, j, :], z[:, j * 128:(j + 1) * 128],
                                w2_t[:], start=True, stop=True)
                            h2s = mpool.tile([128, HID], BF16, tag="h2s")
                            if bias_zero:
                                nc.scalar.activation(
                                    h2s[:], h2p[:, j, :],
                                    mybir.ActivationFunctionType.Relu,
                                    scale=dinv1_t[:, blk:blk + 1])
                            else:
                                hb2 = mpool.tile([128, HID], F32, tag="hb2")
                                nc.vector.tensor_scalar(
                                    hb2[:], h2p[:, j, :],
                                    dinv1_t[:, blk:blk + 1], None,
                                    mybir.AluOpType.mult)
                                nc.vector.tensor_add(hb2[:], hb2[:],
                                                     b2r_t[:])
                                nc.scalar.activation(
                                    h2s[:], hb2[:],
                                    mybir.ActivationFunctionType.Relu)
                            st = first and gg == 0 and j == 0
                            sp = last and gg == nblk // G3 - 1 and j == G3 - 1
                            for hh in range(2):
                                nc.tensor.matmul(
                                    pooled[hh][:],
                                    sel7[:, j, hh * 128:(hh + 1) * 128],
                                    h2s[:], start=st, stop=sp)

                # ---- conv2 main loop with interleaved RS / mm2
                q_last_grp = []                      # last retire group per q
                acc = 0
                for k in range(4):
                    q_last_grp.append((acc + QLB[k]) // G2 - 1)
                    acc += QLB[k]

                rs_emitted = []
                rs_cc = {}

                def emit_rs(k):
                    cc = nc.gpsimd.collective_compute(
                        "ReduceScatter", mybir.AluOpType.add, CORES,
                        [u2t_k[k][:]], [u2o_k[k][:]])
                    for d in qdmas[k]:
                        _add_dep_helper(cc.ins, d.ins, True,
                                        f"RS{k} waits quarter writes")
                    if rs_emitted:
                        _add_dep_helper(cc.ins, rs_cc[rs_emitted[-1]].ins,
                                        True, "collective order")
                    rs_cc[k] = cc
                    rs_emitted.append(k)

                aggP = None
                cur_grp = -1
                retired2 = set()
                sb_i = 0

                rs_ready = []

                def retire_and_track(g, aggP):
                    retire2(g, aggP)
                    retired2.add(g)
                    for k in range(4):
                        if g == q_last_grp[k]:
                            lo_g = q_last_grp[k - 1] + 1 if k else 0
                            if all(gg in retired2
                                   for gg in range(lo_g, g + 1)):
                                rs_ready.append(k)

                def maybe_emit_pending(blk):
                    # fire a ready RS once the loop is half-way through the
                    # NEXT quarter (gather descs pre-generated = DMA runway)
                    if not rs_ready:
                        return
                    k = rs_ready[0]
                    b_glob = _B2ORDER[blk // G2 * G2]
                    kq = _q_of_lb(b_glob % NB64)
                    r = b_glob // NB64
                    if kq > k + 1 or (kq == k + 1 and r >= 4):
                        rs_ready.pop(0)
                        emit_rs(k)
                        if stage >= 5 and k >= 1:
                            mm2_chunk(k - 1)

                for u, (cc, blk, lo, hi) in enumerate(units2):
                    w0 = cc // W2CH * W2CH
                    if w0 not in mt:
                        gather2(w0)
                    u0 = u // UB * UB
                    if u0 not in stile:
                        sbuild2(u0, sb_i)
                        sb_i += 1
                    g = blk // G2
                    maybe_emit_pending(blk)
                    if g != cur_grp:
                        if cur_grp >= 0:
                            retire_and_track(cur_grp, aggP)
                        cur_grp = g
                        aggP = papool.tile([128, G2 * DW], F32, tag="agg2")
                    j = blk - g * G2
                    nc.tensor.matmul(
                        aggP[:, j * DW:(j + 1) * DW], mt[w0][:, cc - w0, :],
                        stile[u0][:, u - u0, :],
                        start=ustart2[u], stop=ustop2[u])
                if cur_grp >= 0:
                    retire_and_track(cur_grp, aggP)
                # flush any incomplete / absent (quarter, range) staging
                for k in list(rs_ready):
                    pass
                for k in range(4):
                    for r in range(NC):
                        if (k, r) in stage_state:
                            flush_qr(k, r)
                        elif (k, r) not in flushed_qr:
                            tag = "u2sA" if QLB[k] == 56 else "u2sB"
                            st_t = rpool.tile([128, QLB[k] * DW], FP8,
                                              tag=tag, name=tag)
                            stage_state[(k, r)] = [st_t, set()]
                            flush_qr(k, r)
                    if k not in rs_emitted:
                        emit_rs(k)
                        if stage >= 5 and k >= 1:
                            mm2_chunk(k - 1)

                if stage == 4:
                    for k in range(4):
                        dd = nc.sync.dma_start(out=dbg_v[k][:],
                                               in_=u2o_k[k][:])
                        _add_dep_helper(dd.ins, rs_cc[k].ins, True, "dbg")
                if stage == 3:
                    for k in range(4):
                        dd = nc.sync.dma_start(out=dbg_u[k][:],
                                               in_=u2t_k[k][:])
                        for d in qdmas[k]:
                            _add_dep_helper(dd.ins, d.ins, True, "dbgu")

                pl_dma = None
                if stage >= 5:
                    mm2_chunk(3)
                    pl_r = pool_loc.ap().rearrange("(h p) f -> p h f", p=128)
                    pl_s = mpool.tile([128, 2, HID], BF16, tag="pls")
                    nc.vector.tensor_copy(pl_s[:, 0, :], pooled[0][:])
                    nc.vector.tensor_copy(pl_s[:, 1, :], pooled[1][:])
                    pl_dma = nc.sync.dma_start(out=pl_r[:], in_=pl_s[:])

                for p in reversed(pools2):
                    p.__exit__(None, None, None)

            if stage >= 5:
                ar_cc = nc.gpsimd.collective_compute(
                    "AllReduce", mybir.AluOpType.add, CORES,
                    [pool_loc[:]], [pool_glob[:]],
                )
                _add_dep_helper(ar_cc.ins, pl_dma.ins, True,
                                "AR waits pooled write")
                _add_dep_helper(ar_cc.ins, rs_cc[3].ins, True,
                                "collective order")

                # ---------------- head: mean-div, fc, LayerNorm (tiny)
                pg_r = pool_glob.ap().rearrange("(h p) f -> p h f", p=128)
                y_r = y_d.ap().rearrange("(h p) f -> p h f", p=128)
                with (
                    tc.tile_pool(name="head", bufs=1) as hpool,
                    tc.tile_pool(name="psh", bufs=2, space="PSUM") as hps,
                ):
                    wfc_t = hpool.tile([HID, HID], F32)
                    nc.sync.dma_start(out=wfc_t[:], in_=wfc_d[:])
                    bfcr_t = hpool.tile([128, HID], F32)
                    nc.sync.dma_start(out=bfcr_t[:], in_=bfcr_d[:])
                    gamr_t = hpool.tile([128, HID], F32)
                    nc.sync.dma_start(out=gamr_t[:], in_=gamr_d[:])
                    betr_t = hpool.tile([128, HID], F32)
                    nc.sync.dma_start(out=betr_t[:], in_=betr_d[:])
                    icnt_t = hpool.tile([128, 2], F32)
                    nc.sync.dma_start(out=icnt_t[:], in_=icnt_d[:])
                    eps_t = hpool.tile([128, 1], F32)
                    nc.vector.memset(eps_t[:], LN_EPS)
                    yo = hpool.tile([128, 2, HID], F32)
                    for hh in range(2):
                        pgb = hpool.tile([128, HID], BF16, tag="pgb")
                        pgd = nc.sync.dma_start(out=pgb[:],
                                                in_=pg_r[:, hh, :])
                        _add_dep_helper(pgd.ins, ar_cc.ins, True,
                                        "head reads AllReduce output")
                        pg_s = hpool.tile([128, HID], F32, tag="pg")
                        nc.vector.tensor_scalar(
                            pg_s[:], pgb[:], icnt_t[:, hh:hh + 1], None,
                            mybir.AluOpType.mult)
                        pgT_p = hps.tile([HID, 128], F32, tag="pgT")
                        nc.tensor.transpose(pgT_p[:], pg_s[:], eye_t[:])
                        pgT_s = hpool.tile([HID, 128], F32, tag="pgTs")
                        nc.vector.tensor_copy(pgT_s[:], pgT_p[:])
                        y_p = hps.tile([128, HID], F32, tag="yp")
                        nc.tensor.matmul(y_p[:], pgT_s[:], wfc_t[:])
                        y_s = hpool.tile([128, HID], F32, tag="ys")
                        nc.vector.tensor_add(y_s[:], y_p[:], bfcr_t[:])
                        mu = hpool.tile([128, 1], F32, tag="mu")
                        nc.vector.tensor_reduce(mu[:], y_s[:],
                                                mybir.AxisListType.XYZW,
                                                mybir.AluOpType.add)
                        nc.vector.tensor_scalar(mu[:], mu[:], -1.0 / HID,
                                                None, mybir.AluOpType.mult)
                        cen = hpool.tile([128, HID], F32, tag="cen")
                        nc.vector.tensor_scalar(cen[:], y_s[:], mu[:], None,
                                                mybir.AluOpType.add)
                        sq = hpool.tile([128, HID], F32, tag="sq")
                        nc.vector.tensor_mul(sq[:], cen[:], cen[:])
                        var = hpool.tile([128, 1], F32, tag="var")
                        nc.vector.tensor_reduce(var[:], sq[:],
                                                mybir.AxisListType.XYZW,
                                                mybir.AluOpType.add)
                        std = hpool.tile([128, 1], F32, tag="std")
                        nc.scalar.activation(
                            std[:], var[:],
                            mybir.ActivationFunctionType.Sqrt,
                            bias=eps_t[:], scale=1.0 / HID)
                        rstd = hpool.tile([128, 1], F32, tag="rstd")
                        nc.vector.reciprocal(rstd[:], std[:])
                        nc.vector.tensor_scalar(cen[:], cen[:], rstd[:],
                                                None, mybir.AluOpType.mult)
                        nc.vector.tensor_mul(cen[:], cen[:], gamr_t[:])
                        nc.vector.tensor_add(yo[:, hh, :], cen[:], betr_t[:])
                    nc.sync.dma_start(out=y_r[:], in_=yo[:])

    nc.compile()
    return nc


_CACHE = {}


def make_in_maps(x, edge_index, batch, W1, b1, W2, b2, Wfc, bfc, gamma, beta,
                 per_core=None, inv_cnt_w=None, dinv=None, meta=None):
    if per_core is None:
        per_core, inv_cnt_w, dinv, meta = _host_prep(
            np.asarray(edge_index), np.asarray(batch))
    x = np.asarray(x, np.float32)
    xp = x * dinv[:, None]
    xb4 = xp.astype(ml_dtypes.bfloat16).reshape(N // 4, 256)
    xself = (xp * dinv[:, None]).astype(np.float32)
    rep = lambda v: np.tile(np.asarray(v, np.float32)[None, :], (128, 1))
    bias_zero = (not np.any(np.asarray(b1))) and (not np.any(np.asarray(b2)))
    shared = {
        "xb4": xb4,
        "w1": np.asarray(W1, np.float32).astype(ml_dtypes.bfloat16),
        "w2": np.asarray(W2, np.float32).astype(ml_dtypes.bfloat16),
        "wfc": np.asarray(Wfc, np.float32),
        "bfcr": rep(bfc),
        "gamr": rep(gamma), "betr": rep(beta),
        "icnt": inv_cnt_w,
    }
    if not bias_zero:
        shared["b1r"] = rep(b1)
        shared["b2r"] = rep(b2)
    in_maps = []
    for c in range(NC):
        m = dict(shared)
        xs = np.zeros((IN, NPC), np.float32)
        xs[:, :RPC] = xself[c * RPC:(c + 1) * RPC].T
        m["xst"] = xs.astype(ml_dtypes.bfloat16)
        for k in ("gi1", "gi2", "dl1", "dl2", "gid",
                  "dinv1", "dinv2"):
            m[k] = per_core[c][k]
        in_maps.append(m)
    return in_maps, bias_zero, meta


def kernel(x, edge_index, batch, W1, b1, W2, b2, Wfc, bfc, gamma, beta,
           _stage=5, _full_results=False):
    per_core, inv_cnt_w, dinv, meta = _host_prep(np.asarray(edge_index),
                                                 np.asarray(batch))
    in_maps, bias_zero, meta = make_in_maps(
        x, edge_index, batch, W1, b1, W2, b2, Wfc, bfc, gamma, beta,
        per_core, inv_cnt_w, dinv, meta)
    key = (meta["epad1"], meta["epad2"], meta["nu1p"], meta["nu2p"],
           bias_zero, _stage)
    if key not in _CACHE:
        _CACHE[key] = _build(meta, bias_zero, _stage)
    nc = _CACHE[key]

    res = run_bass_kernel_spmd(nc, in_maps, list(range(NC)))
    if _full_results:
        return res.results
    return res.results[0]["y"]


# revision 90
# speedup vs baseline: 1.0309x; 1.0036x over previous
"""GCN (2x GCNConv + mean-pool + fc + LayerNorm) on 8 Trainium2 NeuronCores.

One-hot matmul aggregation replaces DMA scatter-adds entirely:

conv1 (pull): per-core in-edges on a cross-core-common segment grid
(64-node dst block x src%4 slot, segment length = max over cores). Each
128-edge chunk is gathered with one 512B descriptor per edge from a
4-row-packed bf16 x' table (the edge's row is slot src%4). For each
(chunk x segment) overlap ("unit"), PE accumulates
zT[64f, 64d] += M[128e, 64f].T @ S[128e, 64d], where S is a one-hot built
from a per-unit dstloc column: batched is_equal-vs-iota on DVE, with a
share built on the Activation engine as relu(1 - (iota - dstloc)^2).
The implicit self-loop is a second matmul against a transposed prescaled
x table, so it never enters the edge stream, and mm1 needs no transpose
(zT is the lhsT directly).

conv2 (push): per-core out-edges on a common global 64-node dst-block
grid, blocks ordered (quarter, range, local) so each node-quarter of the
TRANSPOSED partials u2T completes early; gathers hit the local hpart rows
(h1' = relu(h1)*dinv, prescaled). FOUR chunked ReduceScatters fire as
their quarter's writes land, overlapping the next quarter's compute, and
mm2 consumes each reduced chunk one quarter later (no transposes: z2T is
the lhsT; the self term adds from an SBUF-resident transposed hpsbT).
Pooling is the PSUM-accumulated one-hot graph matmul; pooled sums are
AllReduced; the tiny fc+LayerNorm head is computed redundantly.
"""
import sys

if '/opt/trn_rl_repo' not in sys.path:
    sys.path.insert(0, '/opt/trn_rl_repo')

import ml_dtypes
import numpy as np

import concourse.bacc as bacc
import concourse.mybir as mybir
from concourse.bass import _add_dep_helper
from concourse.tile import TileContext
from concourse.bass_utils import run_bass_kernel_spmd

# ---------------------------------------------------------------- constants
N = 100000
E = 800000
IN = 64
HID = 128
G = 256
NC = 8
RPC = N // NC              # 12500 nodes per core
NB = 98                    # 128-node dst blocks per core (12544 padded)
NPC = NB * 128             # 12544
DW = 64                    # aggregation dst-block width
NB64 = NPC // DW           # 196 64-blocks per core
NBG64 = NC * NB64          # 1568 global 64-blocks
QLB = (56, 56, 56, 28)     # conv2 RS chunks (in 64-blocks per core)
LN_EPS = 1e-5
W1CH = 16                  # conv1 gather window (chunks per call)
W2CH = 24                  # conv2 gather window
UB = 32                    # S-build batch (units per instruction)
ACT1 = 10 ** 9             # conv1: every ACT1-th S-batch goes to Act engine
ACT2 = 10 ** 9             # conv2: every ACT2-th S-batch goes to Act engine
G1 = 4                     # conv1 retire group (128-blocks)
G2 = 7                     # conv2 retire group (64-blocks; divides 56)
G3 = 7                     # mm2 group (128-blocks)
F32 = mybir.dt.float32
FP8 = mybir.dt.float8e4
BF16 = mybir.dt.bfloat16
I16 = mybir.dt.int16


def _wrap16(a, cols):
    """[n] -> [128, cols] int16: element i -> [i%16, i//16], tiled x8."""
    out = np.zeros((16, cols), np.int16)
    w = a.reshape(-1, 16).T
    out[:, : w.shape[1]] = w
    return np.tile(out, (8, 1))


def _common_grid(counts):
    seg = counts.max(axis=0).astype(np.int64)
    off = np.concatenate([[0], np.cumsum(seg)])
    epad = int((off[-1] + 127) // 128 * 128)
    return seg, off, epad


def _make_units(seg, off, nblk, seg_per_blk):
    """(chunk, blk, lo, hi) overlaps of 128-edge chunks with segments.

    Segment k belongs to block k // seg_per_blk (blocks in segment order).
    Returns units, per-unit start/stop (first/last unit of its block), and
    per-block has_units.
    """
    units = []
    blk_first = {}
    blk_last = {}
    for k in range(len(seg)):
        if seg[k] == 0:
            continue
        blk = k // seg_per_blk
        a, b = int(off[k]), int(off[k] + seg[k])
        for cc in range(a // 128, (b - 1) // 128 + 1):
            lo, hi = max(a, cc * 128), min(b, (cc + 1) * 128)
            u = len(units)
            units.append((cc, blk, lo, hi))
            if blk not in blk_first:
                blk_first[blk] = u
            blk_last[blk] = u
    start = [False] * len(units)
    stop = [False] * len(units)
    for blk, u in blk_first.items():
        start[u] = True
    for blk, u in blk_last.items():
        stop[u] = True
    has = [blk in blk_first for blk in range(nblk)]
    return units, start, stop, has


def _q_of_lb(lb):
    """conv2 node-quarter of a local 64-block index."""
    acc = 0
    for q, n in enumerate(QLB):
        if lb < acc + n:
            return q
        acc += n
    raise ValueError(lb)


# conv2 ordered block list: (quarter, range, lb64) lexicographic
_B2ORDER = sorted(range(NBG64),
                  key=lambda b: (_q_of_lb(b % NB64), b // NB64, b % NB64))
_Q0 = [0, 56, 112, 168]            # quarter start lb64


def _host_prep(edge_index, batch):
    src = np.asarray(edge_index[0], np.int64)
    dst = np.asarray(edge_index[1], np.int64)
    deg = (np.bincount(dst, minlength=N) + 1.0).astype(np.float32)
    dinv = (1.0 / np.sqrt(deg)).astype(np.float32)

    # ---------------- conv1: (blk64, slot) grid
    e_core1 = []
    cnt1 = np.zeros((NC, NB64 * 4), np.int64)
    for c in range(NC):
        m = (dst >= c * RPC) & (dst < (c + 1) * RPC)
        s, d = src[m], dst[m] - c * RPC
        key = (d // DW) * 4 + (s % 4)
        cnt1[c] = np.bincount(key, minlength=NB64 * 4)
        order = np.lexsort((d, key))
        e_core1.append((s[order], d[order], key[order]))
    seg1, off1, epad1 = _common_grid(cnt1)
    units1, ustart1, ustop1, has1 = _make_units(seg1, off1, NB64, 4)
    nch1 = epad1 // 128
    nu1 = len(units1)
    nu1p = (nu1 + UB - 1) // UB * UB
    uslot1 = []
    for u, (cc, blk, lo, hi) in enumerate(units1):
        base = blk * 4
        q = 0
        for qq in range(4):
            a, b = off1[base + qq], off1[base + qq] + seg1[base + qq]
            if a <= lo < b:
                q = qq
                break
        uslot1.append(q)

    # ---------------- conv2: global 64-blocks in _B2ORDER
    e_core2 = []
    cnt2 = np.zeros((NC, NBG64), np.int64)   # indexed by ORDER position
    b2pos = np.empty(NBG64, np.int64)        # global blk64 -> order position
    for i, b in enumerate(_B2ORDER):
        b2pos[b] = i
    for c in range(NC):
        m = (src >= c * RPC) & (src < (c + 1) * RPC)
        s, d = src[m] - c * RPC, dst[m]
        rr = d // RPC                      # owner range of the dst
        dloc = d - rr * RPC                # dst local to its owner core
        key = b2pos[rr * NB64 + dloc // DW]
        cnt2[c] = np.bincount(key, minlength=NBG64)
        order = np.lexsort((dloc, key))
        e_core2.append((s[order], dloc[order], key[order]))
    seg2, off2, epad2 = _common_grid(cnt2)
    units2, ustart2, ustop2, has2 = _make_units(seg2, off2, NBG64, 1)
    nch2 = epad2 // 128
    nu2 = len(units2)
    nu2p = (nu2 + UB - 1) // UB * UB

    # ---------------- per-core streams
    per_core = []
    for c in range(NC):
        s, d, key = e_core1[c]
        pos = np.empty(len(s), np.int64)
        ptr = 0
        for k in np.unique(key):
            n = int(cnt1[c][k])
            pos[ptr:ptr + n] = off1[k] + np.arange(n)
            ptr += n
        gi1 = np.zeros(epad1, np.int16)
        gi1[pos] = (s // 4).astype(np.int16)
        sd1 = np.full(epad1, -1.0, np.float32)
        sd1[pos] = (d % DW).astype(np.float32)
        sl1 = np.full(epad1, -1, np.int8)
        sl1[pos] = (s % 4).astype(np.int8)
        dl1c = np.full((nu1p, 128), -1.0, np.float32)
        for u, (cc, blk, lo, hi) in enumerate(units1):
            p0 = lo - cc * 128
            q = uslot1[u]
            dl1c[u, p0:hi - cc * 128] = np.where(
                sl1[lo:hi] == q, sd1[lo:hi], -1.0)

        s2, d2, key2 = e_core2[c]
        pos2 = np.empty(len(s2), np.int64)
        ptr = 0
        for k in np.unique(key2):
            n = int(cnt2[c][k])
            pos2[ptr:ptr + n] = off2[k] + np.arange(n)
            ptr += n
        gi2 = np.zeros(epad2, np.int16)
        gi2[pos2] = ((s2 % 128) * NB + s2 // 128).astype(np.int16)
        sd2 = np.full(epad2, -1.0, np.float32)
        sd2[pos2] = (d2 % DW).astype(np.float32)
        dl2c = np.full((nu2p, 128), -1.0, np.float32)
        for u, (cc, blk, lo, hi) in enumerate(units2):
            p0 = lo - cc * 128
            dl2c[u, p0:hi - cc * 128] = sd2[lo:hi]

        dv = np.zeros(NPC, np.float32)
        dv[:RPC] = dinv[c * RPC:(c + 1) * RPC]
        dvt = dv.reshape(NB, 128).T.copy()
        gid = np.asarray(batch, np.int64)
        gv = np.full(NPC, -1.0, np.float32)
        gv[:RPC] = gid[c * RPC:(c + 1) * RPC].astype(np.float32)

        per_core.append({
            "gi1": _wrap16(gi1, epad1 // 16),
            "gi2": _wrap16(gi2, epad2 // 16),
            "dl1": dl1c.T.astype(ml_dtypes.bfloat16).copy(),
            "dl2": dl2c.T.astype(ml_dtypes.bfloat16).copy(),
            "dinv1": dvt,
            "dinv2": (dvt * dvt).copy(),
            "gid": gv.reshape(NB, 128).T.copy(),
        })

    gidn = np.asarray(batch, np.int64)
    cntg = np.bincount(gidn, minlength=G).astype(np.float32)
    inv_cnt_w = (1.0 / np.maximum(cntg, 1.0)).reshape(2, 128).T.copy()

    meta = {"epad1": epad1, "nch1": nch1, "units1": units1,
            "ustart1": ustart1, "ustop1": ustop1, "has1": has1, "nu1p": nu1p,
            "uslot1": uslot1,
            "epad2": epad2, "nch2": nch2, "units2": units2,
            "ustart2": ustart2, "ustop2": ustop2, "has2": has2, "nu2p": nu2p}
    return per_core, inv_cnt_w, dinv, meta


def _build(meta, bias_zero=True, stage=5):
    nc = bacc.Bacc("TRN2", target_bir_lowering=False, debug=False,
                   num_devices=NC, num_swdge_queues=1,
                   dynamic_dma_scratch_size=32768)
    epad1, nch1 = meta["epad1"], meta["nch1"]
    units1, ustart1, ustop1 = meta["units1"], meta["ustart1"], meta["ustop1"]
    has1, nu1p, uslot1 = meta["has1"], meta["nu1p"], meta["uslot1"]
    epad2, nch2 = meta["epad2"], meta["nch2"]
    units2, ustart2, ustop2 = meta["units2"], meta["ustart2"], meta["ustop2"]
    has2, nu2p = meta["has2"], meta["nu2p"]
    ecols1, ecols2 = epad1 // 16, epad2 // 16

    # ------------------------------------------------ I/O declarations
    xb4_d = nc.dram_tensor("xb4", [N // 4, 256], BF16, kind="ExternalInput")
    xst_d = nc.dram_tensor("xst", [IN, NPC], BF16, kind="ExternalInput")
    w1_d = nc.dram_tensor("w1", [IN, HID], BF16, kind="ExternalInput")
    w2_d = nc.dram_tensor("w2", [HID, HID], BF16, kind="ExternalInput")
    wfc_d = nc.dram_tensor("wfc", [HID, HID], F32, kind="ExternalInput")
    bfcr_d = nc.dram_tensor("bfcr", [128, HID], F32, kind="ExternalInput")
    gamr_d = nc.dram_tensor("gamr", [128, HID], F32, kind="ExternalInput")
    betr_d = nc.dram_tensor("betr", [128, HID], F32, kind="ExternalInput")
    if not bias_zero:
        b1r_d = nc.dram_tensor("b1r", [128, HID], F32, kind="ExternalInput")
        b2r_d = nc.dram_tensor("b2r", [128, HID], F32, kind="ExternalInput")
    gi1_d = nc.dram_tensor("gi1", [128, ecols1], I16, kind="ExternalInput")
    gi2_d = nc.dram_tensor("gi2", [128, ecols2], I16, kind="ExternalInput")
    dl1_d = nc.dram_tensor("dl1", [128, nu1p], BF16, kind="ExternalInput")
    dl2_d = nc.dram_tensor("dl2", [128, nu2p], BF16, kind="ExternalInput")
    gid_d = nc.dram_tensor("gid", [128, NB], F32, kind="ExternalInput")
    dinv1_d = nc.dram_tensor("dinv1", [128, NB], F32, kind="ExternalInput")
    dinv2_d = nc.dram_tensor("dinv2", [128, NB], F32, kind="ExternalInput")
    icnt_d = nc.dram_tensor("icnt", [128, 2], F32, kind="ExternalInput")

    hpart = nc.dram_tensor("hpart", [NPC, HID], BF16)
    u2t_k = [nc.dram_tensor(f"u2t{k}", [NC * HID, QLB[k] * DW], FP8)
             for k in range(4)]
    u2o_k = [nc.dram_tensor(f"u2o{k}", [HID, QLB[k] * DW], FP8)
             for k in range(4)]
    pool_loc = nc.dram_tensor("pool_loc", [G, HID], BF16)
    pool_glob = nc.dram_tensor("pool_glob", [G, HID], BF16,
                               addr_space="Shared")
    y_d = nc.dram_tensor("y", [G, HID], F32, kind="ExternalOutput")
    if stage == 2:
        dbg_h = nc.dram_tensor("dbg_h", [NPC, HID], BF16,
                               kind="ExternalOutput")
    if stage == 4:
        dbg_v = [nc.dram_tensor(f"dbg_v{k}", [HID, QLB[k] * DW], FP8,
                                kind="ExternalOutput") for k in range(4)]
    if stage == 3:
        dbg_u = [nc.dram_tensor(f"dbg_u{k}", [NC * HID, QLB[k] * DW], FP8,
                                kind="ExternalOutput") for k in range(4)]

    eye_d = nc.inline_tensor(np.eye(128, dtype=np.float32), name="eye128")
    eyeb_d = nc.inline_tensor(np.eye(128, dtype=ml_dtypes.bfloat16),
                              name="eye128b")
    iotaU_np = np.tile(np.arange(DW, dtype=np.float32),
                       (128, UB)).astype(ml_dtypes.bfloat16)
    iotaU_d = nc.inline_tensor(iotaU_np, name="iotaU")
    iotaF_d = nc.inline_tensor(
        np.tile(np.arange(DW, dtype=np.float32),
                (128, 1)).astype(ml_dtypes.bfloat16), name="iotaF")
    iota256_d = nc.inline_tensor(
        np.tile(np.arange(256, dtype=np.float32), (128, 1)), name="iota256")


    xb4 = xb4_d.ap()
    hp_rows = hpart.ap()
    CORES = [list(range(NC))]

    # persistent SBUF
    gi1_s = nc.alloc_sbuf_tensor("gi1_s", [128, ecols1], I16)
    gi2_s = nc.alloc_sbuf_tensor("gi2_s", [128, ecols2], I16)
    dl1_s = nc.alloc_sbuf_tensor("dl1_s", [128, nu1p], BF16)
    dl2_s = nc.alloc_sbuf_tensor("dl2_s", [128, nu2p], BF16)
    xst_s = nc.alloc_sbuf_tensor("xst_s", [IN, NPC], BF16)
    hpsbT = nc.alloc_sbuf_tensor("hpsbT", [128, NB * 128], BF16)

    with TileContext(nc) as tc:
        with tc.tile_pool(name="init", bufs=1) as ipool:
            nc.sync.dma_start(out=gi1_s[:], in_=gi1_d[:])
            nc.sync.dma_start(out=gi2_s[:], in_=gi2_d[:])
            nc.sync.dma_start(out=dl1_s[:], in_=dl1_d[:])
            nc.sync.dma_start(out=dl2_s[:], in_=dl2_d[:])
            nc.sync.dma_start(out=xst_s[:], in_=xst_d[:])
            eye_t = ipool.tile([128, 128], F32)
            nc.sync.dma_start(out=eye_t[:], in_=eye_d[:])
            eyeb_t = ipool.tile([128, 128], BF16)
            nc.sync.dma_start(out=eyeb_t[:], in_=eyeb_d[:])
            iotaU_t = ipool.tile([128, UB * DW], BF16)
            nc.sync.dma_start(out=iotaU_t[:], in_=iotaU_d[:])
            iotaF_t = ipool.tile([128, DW], BF16)
            nc.sync.dma_start(out=iotaF_t[:], in_=iotaF_d[:])
            iota256_t = ipool.tile([128, 256], F32)
            nc.sync.dma_start(out=iota256_t[:], in_=iota256_d[:])

            w1_t = ipool.tile([IN, HID], BF16)
            nc.sync.dma_start(out=w1_t[:], in_=w1_d[:])
            w2_t = ipool.tile([HID, HID], BF16)
            nc.sync.dma_start(out=w2_t[:], in_=w2_d[:])
            gid_t = ipool.tile([128, NB], F32)
            nc.sync.dma_start(out=gid_t[:], in_=gid_d[:])

            dinv1_t = ipool.tile([128, NB], F32)
            nc.sync.dma_start(out=dinv1_t[:], in_=dinv1_d[:])
            dinv2_t = ipool.tile([128, NB], F32)
            nc.sync.dma_start(out=dinv2_t[:], in_=dinv2_d[:])
            ones_t = ipool.tile([128, 1], BF16)
            nc.vector.memset(ones_t[:], 1.0)
            if not bias_zero:
                b1r_t = ipool.tile([128, HID], F32)
                nc.sync.dma_start(out=b1r_t[:], in_=b1r_d[:])
                b2r_t = ipool.tile([128, HID], F32)
                nc.sync.dma_start(out=b2r_t[:], in_=b2r_d[:])

            hp_r = hpart.ap().rearrange("(p a) f -> p a f", p=128)
            u2t_r = [t.ap().rearrange("(r p) n -> r p n", p=HID)
                     for t in u2t_k]

            # ======================= conv1 =======================
            with (
                tc.tile_pool(name="g1", bufs=4) as gpool,
                tc.tile_pool(name="s1", bufs=6) as spool,
                tc.tile_pool(name="r1", bufs=3) as rpool,
                tc.tile_pool(name="pa1", bufs=3, space="PSUM") as papool,
                tc.tile_pool(name="ph1", bufs=2, space="PSUM") as phpool,
                tc.tile_pool(name="pt1", bufs=3, space="PSUM") as ptpool,
            ):
                mt = {}

                def gather1(w0):
                    wh = min(w0 + W1CH, nch1)
                    t = gpool.tile([128, W1CH, 256], BF16, tag="m1")
                    nc.gpsimd.dma_gather(
                        t[:, :wh - w0, :], xb4, gi1_s[:, w0 * 8:wh * 8],
                        (wh - w0) * 128, (wh - w0) * 128, 256,
                        queue_num=0, single_packet=False)
                    mt.clear()
                    mt[w0] = t

                stile = {}

                def sbuild(u0, nu, dl_s, act_every, sb_i):
                    uh = min(u0 + UB, nu)
                    t = spool.tile([128, UB, DW], BF16, tag="s")
                    if sb_i % act_every == act_every - 1:
                        for j in range(uh - u0):
                            tq = spool.tile([128, DW], BF16, tag="tq")
                            nc.scalar.activation(
                                tq[:], iotaF_t[:],
                                mybir.ActivationFunctionType.Square,
                                bias=dl_s[:, u0 + j:u0 + j + 1], scale=-1.0)
                            nc.scalar.activation(
                                t[:, j, :], tq[:],
                                mybir.ActivationFunctionType.Relu,
                                bias=ones_t[:], scale=-1.0)
                    else:
                        nc.vector.tensor_tensor(
                            t[:, :uh - u0, :],
                            dl_s[:, u0:uh].rearrange("p (u x) -> p u x", x=1)
                            .broadcast_to([128, uh - u0, DW]),
                            iotaU_t[:, :(uh - u0) * DW]
                            .rearrange("p (u x) -> p u x", x=DW),
                            mybir.AluOpType.is_equal)
                    stile.clear()
                    stile[u0] = t

                # conv1: retire group = G1 128-blocks = 2*G1 64-blocks
                B64G = 2 * G1

                r1_i = [0]

                def retire1(g, aggP):
                    b0 = g * G1                      # first 128-block
                    nb_ = min(G1, NB - b0)
                    zs = rpool.tile([64, G1 * 128], BF16, tag="zsb")
                    if aggP is not None:
                        r1_i[0] += 1
                        if r1_i[0] % 3 == 0:
                            nc.vector.tensor_copy(zs[:, :nb_ * 128],
                                                  aggP[:, :nb_ * 128])
                        else:
                            nc.scalar.activation(
                                zs[:, :nb_ * 128], aggP[:, :nb_ * 128],
                                mybir.ActivationFunctionType.Copy)
                    for j64 in range(nb_ * 2):
                        if not has1[g * B64G + j64]:
                            nc.vector.memset(
                                zs[:, j64 * DW:(j64 + 1) * DW], 0.0)
                    hps = phpool.tile([128, G1, 128], F32, tag="hps")
                    hg = rpool.tile([128, G1, 128], BF16, tag="hg")
                    for j in range(nb_):
                        blk = b0 + j
                        nc.tensor.matmul(hps[:, j, :],
                                         zs[:, j * 128:(j + 1) * 128],
                                         w1_t[:], start=True, stop=False)
                        nc.tensor.matmul(
                            hps[:, j, :],
                            xst_s[:, blk * 128:(blk + 1) * 128],
                            w1_t[:], start=False, stop=True)
                        if bias_zero:
                            nc.scalar.activation(
                                hg[:, j, :], hps[:, j, :],
                                mybir.ActivationFunctionType.Relu,
                                scale=dinv2_t[:, blk:blk + 1])
                        else:
                            hb = rpool.tile([128, HID], F32, tag="hb")
                            nc.vector.tensor_scalar(
                                hb[:], hps[:, j, :],
                                dinv1_t[:, blk:blk + 1], None,
                                mybir.AluOpType.mult)
                            nc.vector.tensor_add(hb[:], hb[:], b1r_t[:])
                            hr = rpool.tile([128, HID], F32, tag="hr")
                            nc.scalar.activation(
                                hr[:], hb[:],
                                mybir.ActivationFunctionType.Relu)
                            nc.vector.tensor_scalar(
                                hg[:, j, :], hr[:],
                                dinv1_t[:, blk:blk + 1], None,
                                mybir.AluOpType.mult)
                        tp = ptpool.tile([128, 128], BF16, tag="tp")
                        nc.tensor.transpose(tp[:], hg[:, j, :], eyeb_t[:])
                        if blk % 3 == 2:
                            nc.vector.tensor_copy(
                                hpsbT[:, blk * 128:(blk + 1) * 128], tp[:])
                        else:
                            nc.scalar.activation(
                                hpsbT[:, blk * 128:(blk + 1) * 128], tp[:],
                                mybir.ActivationFunctionType.Copy)
                    nc.sync.dma_start(out=hp_r[:, b0:b0 + nb_, :],
                                        in_=hg[:, :nb_, :])

                aggP = None
                cur_grp = -1
                retired = set()
                sb_i = 0
                for u, (cc, blk, lo, hi) in enumerate(units1):
                    w0 = cc // W1CH * W1CH
                    if w0 not in mt:
                        gather1(w0)
                    u0 = u // UB * UB
                    if u0 not in stile:
                        sbuild(u0, len(units1), dl1_s, ACT1, sb_i)
                        sb_i += 1
                    g = blk // B64G
                    if g != cur_grp:
                        if cur_grp >= 0:
                            retire1(cur_grp, aggP)
                            retired.add(cur_grp)
                        cur_grp = g
                        aggP = papool.tile([64, B64G * DW], F32, tag="agg")
                    j = blk - g * B64G
                    q = uslot1[u]
                    nc.tensor.matmul(
                        aggP[:, j * DW:(j + 1) * DW],
                        mt[w0][:, cc - w0, 64 * q:64 * q + 64],
                        stile[u0][:, u - u0, :],
                        start=ustart1[u], stop=ustop1[u])
                if cur_grp >= 0:
                    retire1(cur_grp, aggP)
                    retired.add(cur_grp)
                for g in range((NB + G1 - 1) // G1):
                    if g not in retired:
                        retire1(g, None)

            if stage == 2:
                nc.sync.dma_start(out=dbg_h[:], in_=hpart[:])

            # ============== conv2 + chunked RS + pipelined mm2 ==========
            if stage >= 3:
                pools2 = [
                    tc.tile_pool(name="g2", bufs=4),
                    tc.tile_pool(name="s2", bufs=6),
                    tc.tile_pool(name="r2", bufs=4),
                    tc.tile_pool(name="pa2", bufs=3, space="PSUM"),
                    tc.tile_pool(name="mm2", bufs=3),
                    tc.tile_pool(name="ps2", bufs=1, space="PSUM"),
                    tc.tile_pool(name="pacc", bufs=1, space="PSUM"),
                ]
                gpool, spool, rpool, papool, mpool, ppool, apool = [
                    p.__enter__() for p in pools2]
                mt = {}
                stile = {}

                def gather2(w0):
                    wh = min(w0 + W2CH, nch2)
                    t = gpool.tile([128, W2CH, HID], BF16, tag="m2")
                    nc.gpsimd.dma_gather(
                        t[:, :wh - w0, :], hp_rows,
                        gi2_s[:, w0 * 8:wh * 8],
                        (wh - w0) * 128, (wh - w0) * 128, HID,
                        queue_num=0, single_packet=False)
                    mt.clear()
                    mt[w0] = t

                def sbuild2(u0, sb_i):
                    uh = min(u0 + UB, len(units2))
                    t = spool.tile([128, UB, DW], BF16, tag="s")
                    if sb_i % ACT2 == ACT2 - 1:
                        for j in range(uh - u0):
                            tq = spool.tile([128, DW], BF16, tag="tq")
                            nc.scalar.activation(
                                tq[:], iotaF_t[:],
                                mybir.ActivationFunctionType.Square,
                                bias=dl2_s[:, u0 + j:u0 + j + 1],
                                scale=-1.0)
                            nc.scalar.activation(
                                t[:, j, :], tq[:],
                                mybir.ActivationFunctionType.Relu,
                                bias=ones_t[:], scale=-1.0)
                    else:
                        nc.vector.tensor_tensor(
                            t[:, :uh - u0, :],
                            dl2_s[:, u0:uh].rearrange("p (u x) -> p u x", x=1)
                            .broadcast_to([128, uh - u0, DW]),
                            iotaU_t[:, :(uh - u0) * DW]
                            .rearrange("p (u x) -> p u x", x=DW),
                            mybir.AluOpType.is_equal)
                    stile.clear()
                    stile[u0] = t

                ret_i = [0]
                qdmas = [[], [], [], []]     # retire DMA insts per quarter
                stage_state = {}             # (k, r) -> [tile, filled_set]

                flushed_qr = set()

                def flush_qr(k, r):
                    tile, filled = stage_state.pop((k, r))
                    ngrp = QLB[k] // G2
                    for gl in range(ngrp):
                        if gl not in filled:
                            nc.vector.memset(
                                tile[:, gl * G2 * DW:(gl + 1) * G2 * DW],
                                0.0)
                    dma = nc.gpsimd.dma_start(out=u2t_r[k][r, :, :],
                                              in_=tile[:, :QLB[k] * DW])
                    qdmas[k].append(dma)
                    flushed_qr.add((k, r))

                def retire2(g, aggP):
                    # g: group index over ordered blocks (G2 64-blocks)
                    ob0 = g * G2
                    b_glob = _B2ORDER[ob0]
                    r = b_glob // NB64
                    lb = b_glob % NB64
                    k = _q_of_lb(lb)
                    gl = (lb - _Q0[k]) // G2         # group within (k, r)
                    ngrp = QLB[k] // G2
                    if (k, r) not in stage_state:
                        tag = "u2sA" if QLB[k] == 56 else "u2sB"
                        st_t = rpool.tile([128, QLB[k] * DW], FP8, tag=tag,
                                          name=tag)
                        stage_state[(k, r)] = [st_t, set()]
                    tile, filled = stage_state[(k, r)]
                    sl = tile[:, gl * G2 * DW:(gl + 1) * G2 * DW]
                    eng = [nc.scalar, nc.scalar, nc.scalar, nc.scalar,
                           nc.vector][ret_i[0] % 5]
                    ret_i[0] += 1
                    if eng is nc.scalar:
                        nc.scalar.activation(
                            sl, aggP[:], mybir.ActivationFunctionType.Copy)
                    else:
                        nc.vector.tensor_copy(sl, aggP[:])
                    for j in range(G2):
                        if not has2[ob0 + j]:
                            nc.vector.memset(
                                tile[:, (gl * G2 + j) * DW:
                                     (gl * G2 + j + 1) * DW], 0.0)
                    filled.add(gl)
                    if len(filled) == ngrp:
                        flush_qr(k, r)

                # ---- mm2 chunk consumer (pooled psum held across chunks)
                pooled = [apool.tile([128, HID], F32, tag=f"pool{h}",
                                     name=f"pooled{h}")
                          for h in range(2)]
                NB128Q = [q // 2 for q in QLB]       # 128-blocks per chunk
                LB128Q = [q // 2 for q in _Q0]       # first 128-block

                def mm2_chunk(k):
                    u2o = u2o_k[k].ap()
                    first = (k == 0)
                    last = (k == 3)
                    nblk = NB128Q[k]
                    for gg in range(nblk // G3):
                        b0 = LB128Q[k] + gg * G3     # absolute 128-block
                        c0 = gg * G3 * 128
                        ga = mpool.tile([128, G3 * 128], FP8, tag="ga")
                        gd = nc.sync.dma_start(out=ga[:],
                                               in_=u2o[:, c0:c0 + G3 * 128])
                        _add_dep_helper(gd.ins, rs_cc[k].ins, True,
                                        f"mm2 chunk {k} reads RS{k}")
                        sel7 = mpool.tile([128, G3, 256], BF16, tag="sel7")
                        nc.vector.tensor_tensor(
                            sel7[:],
                            gid_t[:, b0:b0 + G3]
                            .rearrange("p (u x) -> p u x", x=1)
                            .broadcast_to([128, G3, 256]),
                            iota256_t[:]
                            .rearrange("p (u x) -> p u x", u=1)
                            .broadcast_to([128, G3, 256]),
                            mybir.AluOpType.is_equal)
                        z = mpool.tile([128, G3 * 128], BF16, tag="z2")
                        nc.vector.tensor_add(
                            z[:], ga[:],
                            hpsbT[:, b0 * 128:(b0 + G3) * 128])
                        h2p = ppool.tile([128, G3, HID], F32, tag="h2p")
                        for j in range(G3):
                            blk = b0 + j
                            nc.tensor.matmul(
                                h2p[:, j, :], z[:, j * 128:(j + 1) * 128],
                                w2_t[:], start=True, stop=True)
                            h2s = mpool.tile([128, HID], BF16, tag="h2s")
                            if bias_zero:
                                nc.scalar.activation(
                                    h2s[:], h2p[:, j, :],
                                    mybir.ActivationFunctionType.Relu,
                                    scale=dinv1_t[:, blk:blk + 1])
                            else:
                                hb2 = mpool.tile([128, HID], F32, tag="hb2")
                                nc.vector.tensor_scalar(
                                    hb2[:], h2p[:, j, :],
                                    dinv1_t[:, blk:blk + 1], None,
                                    mybir.AluOpType.mult)
                                nc.vector.tensor_add(hb2[:], hb2[:],
                                                     b2r_t[:])
                                nc.scalar.activation(
                                    h2s[:], hb2[:],
                                    mybir.ActivationFunctionType.Relu)
                            st = first and gg == 0 and j == 0
                            sp = last and gg == nblk // G3 - 1 and j == G3 - 1
                            for hh in range(2):
                                nc.tensor.matmul(
                                    pooled[hh][:],
                                    sel7[:, j, hh * 128:(hh + 1) * 128],
                                    h2s[:], start=st, stop=sp)

                # ---- conv2 main loop with interleaved RS / mm2
                q_last_grp = []                      # last retire group per q
                acc = 0
                for k in range(4):
                    q_last_grp.append((acc + QLB[k]) // G2 - 1)
                    acc += QLB[k]

                rs_emitted = []
                rs_cc = {}

                def emit_rs(k):
                    cc = nc.gpsimd.collective_compute(
                        "ReduceScatter", mybir.AluOpType.add, CORES,
                        [u2t_k[k][:]], [u2o_k[k][:]])
                    for d in qdmas[k]:
                        _add_dep_helper(cc.ins, d.ins, True,
                                        f"RS{k} waits quarter writes")
                    if rs_emitted:
                        _add_dep_helper(cc.ins, rs_cc[rs_emitted[-1]].ins,
                                        True, "collective order")
                    rs_cc[k] = cc
                    rs_emitted.append(k)

                aggP = None
                cur_grp = -1
                retired2 = set()
                sb_i = 0

                rs_ready = []

                def retire_and_track(g, aggP):
                    retire2(g, aggP)
                    retired2.add(g)
                    for k in range(4):
                        if g == q_last_grp[k]:
                            lo_g = q_last_grp[k - 1] + 1 if k else 0
                            if all(gg in retired2
                                   for gg in range(lo_g, g + 1)):
                                rs_ready.append(k)

                def maybe_emit_pending(blk):
                    # fire a ready RS once the loop is half-way through the
                    # NEXT quarter (gather descs pre-generated = DMA runway)
                    if not rs_ready:
                        return
                    k = rs_ready[0]
                    b_glob = _B2ORDER[blk // G2 * G2]
                    kq = _q_of_lb(b_glob % NB64)
                    r = b_glob // NB64
                    if kq > k + 1 or (kq == k + 1 and r >= 4):
                        rs_ready.pop(0)
                        emit_rs(k)
                        if stage >= 5 and k >= 1:
                            mm2_chunk(k - 1)

                for u, (cc, blk, lo, hi) in enumerate(units2):
                    w0 = cc // W2CH * W2CH
                    if w0 not in mt:
                        gather2(w0)
                    u0 = u // UB * UB
                    if u0 not in stile:
                        sbuild2(u0, sb_i)
                        sb_i += 1
                    g = blk // G2
                    maybe_emit_pending(blk)
                    if g != cur_grp:
                        if cur_grp >= 0:
                            retire_and_track(cur_grp, aggP)
                        cur_grp = g
                        aggP = papool.tile([128, G2 * DW], F32, tag="agg2")
                    j = blk - g * G2
                    nc.tensor.matmul(
                        aggP[:, j * DW:(j + 1) * DW], mt[w0][:, cc - w0, :],
                        stile[u0][:, u - u0, :],
                        start=ustart2[u], stop=ustop2[u])
                if cur_grp >= 0:
                    retire_and_track(cur_grp, aggP)
                # flush any incomplete / absent (quarter, range) staging
                for k in list(rs_ready):
                    pass
                for k in range(4):
                    for r in range(NC):
                        if (k, r) in stage_state:
                            flush_qr(k, r)
                        elif (k, r) not in flushed_qr:
                            tag = "u2sA" if QLB[k] == 56 else "u2sB"
                            st_t = rpool.tile([128, QLB[k] * DW], FP8,
                                              tag=tag, name=tag)
                            stage_state[(k, r)] = [st_t, set()]
                            flush_qr(k, r)
                    if k not in rs_emitted:
                        emit_rs(k)
                        if stage >= 5 and k >= 1:
                            mm2_chunk(k - 1)

                if stage == 4:
                    for k in range(4):
                        dd = nc.sync.dma_start(out=dbg_v[k][:],
                                               in_=u2o_k[k][:])
                        _add_dep_helper(dd.ins, rs_cc[k].ins, True, "dbg")
                if stage == 3:
                    for k in range(4):
                        dd = nc.sync.dma_start(out=dbg_u[k][:],
                                               in_=u2t_k[k][:])
                        for d in qdmas[k]:
                            _add_dep_helper(dd.ins, d.ins, True, "dbgu")

                pl_dma = None
                if stage >= 5:
                    mm2_chunk(3)
                    pl_r = pool_loc.ap().rearrange("(h p) f -> p h f", p=128)
                    pl_s = mpool.tile([128, 2, HID], BF16, tag="pls")
                    nc.vector.tensor_copy(pl_s[:, 0, :], pooled[0][:])
                    nc.vector.tensor_copy(pl_s[:, 1, :], pooled[1][:])
                    pl_dma = nc.sync.dma_start(out=pl_r[:], in_=pl_s[:])

                for p in reversed(pools2):
                    p.__exit__(None, None, None)

            if stage >= 5:
                ar_cc = nc.gpsimd.collective_compute(
                    "AllReduce", mybir.AluOpType.add, CORES,
                    [pool_loc[:]], [pool_glob[:]],
                )
                _add_dep_helper(ar_cc.ins, pl_dma.ins, True,
                                "AR waits pooled write")
                _add_dep_helper(ar_cc.ins, rs_cc[3].ins, True,
                                "collective order")

                # ---------------- head: mean-div, fc, LayerNorm (tiny)
                pg_r = pool_glob.ap().rearrange("(h p) f -> p h f", p=128)
                y_r = y_d.ap().rearrange("(h p) f -> p h f", p=128)
                with (
                    tc.tile_pool(name="head", bufs=1) as hpool,
                    tc.tile_pool(name="psh", bufs=2, space="PSUM") as hps,
                ):
                    wfc_t = hpool.tile([HID, HID], F32)
                    nc.sync.dma_start(out=wfc_t[:], in_=wfc_d[:])
                    bfcr_t = hpool.tile([128, HID], F32)
                    nc.sync.dma_start(out=bfcr_t[:], in_=bfcr_d[:])
                    gamr_t = hpool.tile([128, HID], F32)
                    nc.sync.dma_start(out=gamr_t[:], in_=gamr_d[:])
                    betr_t = hpool.tile([128, HID], F32)
                    nc.sync.dma_start(out=betr_t[:], in_=betr_d[:])
                    icnt_t = hpool.tile([128, 2], F32)
                    nc.sync.dma_start(out=icnt_t[:], in_=icnt_d[:])
                    eps_t = hpool.tile([128, 1], F32)
                    nc.vector.memset(eps_t[:], LN_EPS)
                    yo = hpool.tile([128, 2, HID], F32)
                    for hh in range(2):
                        pgb = hpool.tile([128, HID], BF16, tag="pgb")
                        pgd = nc.sync.dma_start(out=pgb[:],
                                                in_=pg_r[:, hh, :])
                        _add_dep_helper(pgd.ins, ar_cc.ins, True,
                                        "head reads AllReduce output")
                        pg_s = hpool.tile([128, HID], F32, tag="pg")
                        nc.vector.tensor_scalar(
                            pg_s[:], pgb[:], icnt_t[:, hh:hh + 1], None,
                            mybir.AluOpType.mult)
                        pgT_p = hps.tile([HID, 128], F32, tag="pgT")
                        nc.tensor.transpose(pgT_p[:], pg_s[:], eye_t[:])
                        pgT_s = hpool.tile([HID, 128], F32, tag="pgTs")
                        nc.vector.tensor_copy(pgT_s[:], pgT_p[:])
                        y_p = hps.tile([128, HID], F32, tag="yp")
                        nc.tensor.matmul(y_p[:], pgT_s[:], wfc_t[:])
                        y_s = hpool.tile([128, HID], F32, tag="ys")
                        nc.vector.tensor_add(y_s[:], y_p[:], bfcr_t[:])
                        mu = hpool.tile([128, 1], F32, tag="mu")
                        nc.vector.tensor_reduce(mu[:], y_s[:],
                                                mybir.AxisListType.XYZW,
                                                mybir.AluOpType.add)
                        nc.vector.tensor_scalar(mu[:], mu[:], -1.0 / HID,
                                                None, mybir.AluOpType.mult)
                        cen = hpool.tile([128, HID], F32, tag="cen")
                        nc.vector.tensor_scalar(cen[:], y_s[:], mu[:], None,
                                                mybir.AluOpType.add)
                        sq = hpool.tile([128, HID], F32, tag="sq")
                        nc.vector.tensor_mul(sq[:], cen[:], cen[:])
                        var = hpool.tile([128, 1], F32, tag="var")
                        nc.vector.tensor_reduce(var[:], sq[:],
                                                mybir.AxisListType.XYZW,
                                                mybir.AluOpType.add)
                        std = hpool.tile([128, 1], F32, tag="std")
                        nc.scalar.activation(
                            std[:], var[:],
                            mybir.ActivationFunctionType.Sqrt,
                            bias=eps_t[:], scale=1.0 / HID)
                        rstd = hpool.tile([128, 1], F32, tag="rstd")
                        nc.vector.reciprocal(rstd[:], std[:])
                        nc.vector.tensor_scalar(cen[:], cen[:], rstd[:],
                                                None, mybir.AluOpType.mult)
                        nc.vector.tensor_mul(cen[:], cen[:], gamr_t[:])
                        nc.vector.tensor_add(yo[:, hh, :], cen[:], betr_t[:])
                    nc.sync.dma_start(out=y_r[:], in_=yo[:])

    nc.compile()
    return nc


_CACHE = {}


def make_in_maps(x, edge_index, batch, W1, b1, W2, b2, Wfc, bfc, gamma, beta,
                 per_core=None, inv_cnt_w=None, dinv=None, meta=None):
    if per_core is None:
        per_core, inv_cnt_w, dinv, meta = _host_prep(
            np.asarray(edge_index), np.asarray(batch))
    x = np.asarray(x, np.float32)
    xp = x * dinv[:, None]
    xb4 = xp.astype(ml_dtypes.bfloat16).reshape(N // 4, 256)
    xself = (xp * dinv[:, None]).astype(np.float32)
    rep = lambda v: np.tile(np.asarray(v, np.float32)[None, :], (128, 1))
    bias_zero = (not np.any(np.asarray(b1))) and (not np.any(np.asarray(b2)))
    shared = {
        "xb4": xb4,
        "w1": np.asarray(W1, np.float32).astype(ml_dtypes.bfloat16),
        "w2": np.asarray(W2, np.float32).astype(ml_dtypes.bfloat16),
        "wfc": np.asarray(Wfc, np.float32),
        "bfcr": rep(bfc),
        "gamr": rep(gamma), "betr": rep(beta),
        "icnt": inv_cnt_w,
    }
    if not bias_zero:
        shared["b1r"] = rep(b1)
        shared["b2r"] = rep(b2)
    in_maps = []
    for c in range(NC):
        m = dict(shared)
        xs = np.zeros((IN, NPC), np.float32)
        xs[:, :RPC] = xself[c * RPC:(c + 1) * RPC].T
        m["xst"] = xs.astype(ml_dtypes.bfloat16)
        for k in ("gi1", "gi2", "dl1", "dl2", "gid",
                  "dinv1", "dinv2"):
            m[k] = per_core[c][k]
        in_maps.append(m)
    return in_maps, bias_zero, meta


def kernel(x, edge_index, batch, W1, b1, W2, b2, Wfc, bfc, gamma, beta,
           _stage=5, _full_results=False):
    per_core, inv_cnt_w, dinv, meta = _host_prep(np.asarray(edge_index),
                                                 np.asarray(batch))
    in_maps, bias_zero, meta = make_in_maps(
        x, edge_index, batch, W1, b1, W2, b2, Wfc, bfc, gamma, beta,
        per_core, inv_cnt_w, dinv, meta)
    key = (meta["epad1"], meta["epad2"], meta["nu1p"], meta["nu2p"],
           bias_zero, _stage)
    if key not in _CACHE:
        _CACHE[key] = _build(meta, bias_zero, _stage)
    nc = _CACHE[key]

    res = run_bass_kernel_spmd(nc, in_maps, list(range(NC)))
    if _full_results:
        return res.results
    return res.results[0]["y"]
